# revision 1
# baseline (speedup 1.0000x reference)
"""DSA sparse MLA attention kernel for TRN2, 8 NeuronCores.

Sharding: sequence-parallel over query rows. Core c owns query rows
[256c, 256(c+1)). Every core replicates the shared KV/indexer-key
expansion over all 2048 keys (no collectives -- they are far slower
than recompute on this chip). Per-core program is identical (SPMD);
only the per-core inputs (query-block slices) differ.

Pipeline per core:
  P1: stream x^T tiles; ckv = rmsnorm(x@wkv_a[:512]) -> ckvT; k_pe
      (rope) -> kpeT; ki = layernorm(x@idx_wk) + rope -> kiT.
  P2: block: qr = rmsnorm(x_b@wq_a) -> qrT; gate; q = qr@wq_b (+rope,
      *scale) -> qTn/qTp; qi = qr@idx_wq_b (+rope, *gate*scale) -> qiT.
  P3: index scores ISC = sum_h qiT_h . kiT + attn_mask; per-row top-256
      threshold via sampled init + secant iterations on fused
      compare+count (tensor_scalar accum_out); maskNEG = (ISC<t)*-1e9
      + attn_mask.
  P4: per MLA head: expand kT_h, v_h from ckvT; scores; +maskNEG; exp
      (ACT, accum denom); normalize; bf16; DMA-transpose -> probsT;
      PV matmul -> out_hT.
  P5: outT = sum_h wo_h^T @ out_hT -> DRAM.
"""

import os
import numpy as np

import concourse.bass as bass
import concourse.bacc as bacc
import concourse.mybir as mybir
from concourse.tile import TileContext

F32 = mybir.dt.float32
F32R = mybir.dt.float32r
BF16 = mybir.dt.bfloat16

S, HID = 2048, 2048
H, DN, DR, DV = 16, 128, 64, 128
QLR, KVLR = 1024, 512
IH, IHD, TOPK = 8, 64, 256
NEG = -1e9
NB = 256            # query rows per core
NCORES = 8
NT = S // 128       # 16 token tiles
NQT = NB // 128     # 2 query tiles per core
SEL_ITERS = 12      # secant iterations for threshold
SCALE_MLA = float((DN + DR) ** -0.5)
SCALE_IDX = float(IHD ** -0.5)
SCALE_GATE = float(IH ** -0.5)


def _bcast(ap, parts=128):
    """Partition-broadcast view of a 1-D (or row) DRAM AP."""
    return bass.AP(tensor=ap.tensor, offset=ap.offset,
                   ap=[[0, parts]] + list(ap.ap))


def _rmsnorm_from_psum(nc, pool, out_sb, psums, wb, d, eps=1e-6):
    """out_sb[p, d] = psum * rsqrt(mean(psum^2)+eps) * w  (psums: list of
    [128, chunk] PSUM APs covering d columns; wb: [128, d] bcast weights)."""
    ssq = pool.tile([128, len(psums)], F32)
    off = 0
    for i, ps in enumerate(psums):
        w = ps.shape[-1]
        scr = pool.tile([128, 512], F32, tag="rms_scr")
        nc.scalar.activation(out=scr[:, :w], in_=ps,
                             func=mybir.ActivationFunctionType.Square,
                             accum_out=ssq[:, i:i + 1])
        off += w
    tot = pool.tile([128, 1], F32)
    if len(psums) == 1:
        nc.vector.tensor_scalar(out=tot, in0=ssq, scalar1=1.0 / d,
                                scalar2=eps, op0=mybir.AluOpType.mult,
                                op1=mybir.AluOpType.add)
    else:
        nc.vector.tensor_reduce(out=tot, in_=ssq, axis=mybir.AxisListType.X,
                                op=mybir.AluOpType.add)
        nc.vector.tensor_scalar(out=tot, in0=tot, scalar1=1.0 / d,
                                scalar2=eps, op0=mybir.AluOpType.mult,
                                op1=mybir.AluOpType.add)
    nc.scalar.activation(out=tot, in_=tot,
                         func=mybir.ActivationFunctionType.Sqrt)
    rinv = pool.tile([128, 1], F32)
    nc.vector.reciprocal(out=rinv, in_=tot)
    off = 0
    for ps in psums:
        w = ps.shape[-1]
        nc.vector.tensor_scalar(out=out_sb[:, off:off + w], in0=ps,
                                scalar1=rinv, scalar2=None,
                                op0=mybir.AluOpType.mult)
        off += w
    nc.vector.tensor_mul(out_sb[:, :d], out_sb[:, :d], wb[:, :d])


def _rope_int(nc, out, in_, cos, sin):
    """Interleaved (GPT-J) rope, token-major [128, 64] -> out[128, 64].
    cos/sin: [128, 64] token-major tiles (first 32 cols used)."""
    xp = in_.rearrange("p (a b) -> p a b", b=2)
    op = out.rearrange("p (a b) -> p a b", b=2)
    c, s = cos[:, 0:32], sin[:, 0:32]
    x1, x2 = xp[:, :, 0], xp[:, :, 1]
    nc.vector.tensor_mul(op[:, :, 0], x1, c)
    nc.vector.tensor_mul(op[:, :, 1], x2, c)
    t = nc._rope_scr.tile([128, 32], F32, tag="rope_t")
    nc.vector.tensor_mul(t, x2, s)
    nc.vector.tensor_sub(op[:, :, 0], op[:, :, 0], t)
    nc.vector.tensor_mul(t, x1, s)
    nc.vector.tensor_add(op[:, :, 1], op[:, :, 1], t)


def _rope_ni(nc, out, in_, cos, sin):
    """Non-interleaved (rotate_half) rope, [128, 64]."""
    x1, x2 = in_[:, 0:32], in_[:, 32:64]
    c1, c2 = cos[:, 0:32], cos[:, 32:64]
    s1, s2 = sin[:, 0:32], sin[:, 32:64]
    nc.vector.tensor_mul(out[:, 0:32], x1, c1)
    nc.vector.tensor_mul(out[:, 32:64], x2, c2)
    t = nc._rope_scr.tile([128, 32], F32, tag="rope_t")
    nc.vector.tensor_mul(t, x2, s1)
    nc.vector.tensor_sub(out[:, 0:32], out[:, 0:32], t)
    nc.vector.tensor_mul(t, x1, s2)
    nc.vector.tensor_add(out[:, 32:64], out[:, 32:64], t)


def build_nc():
    nc = bacc.Bacc("TRN2", target_bir_lowering=False, debug=False)

    xT = nc.dram_tensor("xT", [HID, S], F32R, kind="ExternalInput").ap()
    xTb = nc.dram_tensor("xTb", [HID, NB], F32R, kind="ExternalInput").ap()
    cos_d = nc.dram_tensor("cos_t", [S, DR], F32, kind="ExternalInput").ap()
    sin_d = nc.dram_tensor("sin_t", [S, DR], F32, kind="ExternalInput").ap()
    cosb_d = nc.dram_tensor("cosb", [NB, DR], F32, kind="ExternalInput").ap()
    sinb_d = nc.dram_tensor("sinb", [NB, DR], F32, kind="ExternalInput").ap()
    amask_d = nc.dram_tensor("amask", [NB, S], F32, kind="ExternalInput").ap()
    wq_a = nc.dram_tensor("wq_a", [HID, QLR], F32R, kind="ExternalInput").ap()
    wq_b = nc.dram_tensor("wq_b", [QLR, H * (DN + DR)], F32R,
                          kind="ExternalInput").ap()
    wkv_a = nc.dram_tensor("wkv_a", [HID, KVLR + DR], F32R,
                           kind="ExternalInput").ap()
    wkv_b = nc.dram_tensor("wkv_b", [KVLR, H * (DN + DV)], F32R,
                           kind="ExternalInput").ap()
    wo = nc.dram_tensor("wo", [H * DV, HID], F32R, kind="ExternalInput").ap()
    iwqb = nc.dram_tensor("idx_wq_b", [QLR, IH * IHD], F32R,
                          kind="ExternalInput").ap()
    iwk = nc.dram_tensor("idx_wk", [HID, IHD], F32R, kind="ExternalInput").ap()
    igate = nc.dram_tensor("idx_gate", [HID, IH], F32R,
                           kind="ExternalInput").ap()
    qnw_d = nc.dram_tensor("q_norm_w", [QLR], F32, kind="ExternalInput").ap()
    kvnw_d = nc.dram_tensor("kv_norm_w", [KVLR], F32,
                            kind="ExternalInput").ap()
    knw_d = nc.dram_tensor("idx_knorm_w", [IHD], F32,
                           kind="ExternalInput").ap()
    knb_d = nc.dram_tensor("idx_knorm_b", [IHD], F32,
                           kind="ExternalInput").ap()
    ident_d = nc.dram_tensor("ident", [128, 128], F32,
                             kind="ExternalInput").ap()
    outT = nc.dram_tensor("outT", [HID, NB], F32, kind="ExternalOutput").ap()

    with TileContext(nc) as tc:
        consts = tc.alloc_tile_pool(name="consts", bufs=1)
        nc._rope_scr = consts

        ident = consts.tile([128, 128], F32)
        nc.gpsimd.dma_start(out=ident, in_=ident_d)
        kvnw = consts.tile([128, KVLR], F32)
        nc.gpsimd.dma_start(out=kvnw, in_=_bcast(kvnw_d))
        knw = consts.tile([128, IHD], F32)
        nc.gpsimd.dma_start(out=knw, in_=_bcast(knw_d))
        knb = consts.tile([128, IHD], F32)
        nc.gpsimd.dma_start(out=knb, in_=_bcast(knb_d))

        ckvT = consts.tile([128, 4, S], F32R)      # [ckv_chunk, 4, tok]
        kpeT = consts.tile([64, S], F32R)
        kiT = consts.tile([64, S], F32R)

        # ---------------- P1: KV / indexer-key expansion ----------------
        with tc.tile_pool(name="p1w", bufs=1) as p1w, \
             tc.tile_pool(name="p1", bufs=3) as p1, \
             tc.tile_pool(name="p1ps", bufs=2, space="PSUM") as p1ps, \
             tc.tile_pool(name="p1tr", bufs=2, space="PSUM") as p1tr:
            cos_t = p1w.tile([128, NT, DR], F32)
            sin_t = p1w.tile([128, NT, DR], F32)
            cr = cos_d.rearrange("(t p) d -> p t d", p=128)
            sr = sin_d.rearrange("(t p) d -> p t d", p=128)
            wkva_sb = p1w.tile([128, NT, KVLR + DR], F32R)
            iwk_sb = p1w.tile([128, NT, IHD], F32R)
            wr = wkv_a.rearrange("(c p) n -> p c n", p=128)
            ir = iwk.rearrange("(c p) n -> p c n", p=128)
            for c in range(NT):
                nc.gpsimd.dma_start(out=cos_t[:, c, :], in_=cr[:, c, :])
                nc.gpsimd.dma_start(out=sin_t[:, c, :], in_=sr[:, c, :])
                nc.gpsimd.dma_start(out=wkva_sb[:, c, :], in_=wr[:, c, :])
                nc.gpsimd.dma_start(out=iwk_sb[:, c, :], in_=ir[:, c, :])

            for t in range(NT):
                xt = p1.tile([128, NT, 128], F32R, tag="xt")
                xr = xT.rearrange("(c p) (u q) -> p c u q", p=128, q=128)
                for c in range(NT):
                    nc.gpsimd.dma_start(out=xt[:, c, :], in_=xr[:, c, t, :])
                ps_kv = p1ps.tile([128, KVLR], F32, tag="ps_kv")
                ps_pe = p1ps.tile([128, DR], F32, tag="ps_pe")
                ps_ki = p1ps.tile([128, IHD], F32, tag="ps_ki")
                for f in range(NT):
                    st, sp = (f == 0), (f == NT - 1)
                    lhs = xt[:, f, :]
                    nc.tensor.matmul(ps_kv, lhs,
                                     wkva_sb[:, f, 0:KVLR],
                                     start=st, stop=sp)
                    nc.tensor.matmul(ps_pe, lhs,
                                     wkva_sb[:, f, KVLR:],
                                     start=st, stop=sp)
                    nc.tensor.matmul(ps_ki, lhs,
                                     iwk_sb[:, f, :],
                                     start=st, stop=sp)
                # ckv rmsnorm -> token-major sbuf -> transpose to ckvT
                ckv_sb = p1.tile([128, KVLR], F32, tag="ckv_sb")
                _rmsnorm_from_psum(nc, p1, ckv_sb, [ps_kv], kvnw, KVLR)
                for ch in range(4):
                    ptr = p1tr.tile([128, 128], F32, tag="ptr")
                    nc.tensor.transpose(ptr, ckv_sb[:, ch * 128:(ch + 1) * 128],
                                        ident)
                    nc.scalar.copy(out=ckvT[:, ch, t * 128:(t + 1) * 128],
                                   in_=ptr)
                # k_pe rope (token-major) -> transpose into kpeT
                pe_sb = p1.tile([128, DR], F32, tag="pe_sb")
                _rope_int(nc, pe_sb, ps_pe, cos_t[:, t, :], sin_t[:, t, :])
                ptr = p1tr.tile([128, 128], F32, tag="ptr")
                nc.tensor.transpose(ptr[:64, :], pe_sb, ident)
                nc.scalar.copy(out=kpeT[:, t * 128:(t + 1) * 128],
                               in_=ptr[:64, :])
                # ki layernorm + rope -> transpose into kiT
                s1 = p1.tile([128, 2], F32, tag="ki_s")
                scr = p1.tile([128, IHD], F32, tag="ki_scr")
                nc.scalar.activation(out=scr, in_=ps_ki,
                                     func=mybir.ActivationFunctionType.Copy,
                                     accum_out=s1[:, 0:1])
                nc.scalar.activation(out=scr, in_=ps_ki,
                                     func=mybir.ActivationFunctionType.Square,
                                     accum_out=s1[:, 1:2])
                mom = p1.tile([128, 4], F32, tag="ki_m")
                nc.vector.tensor_scalar(out=mom[:, 0:1], in0=s1[:, 0:1],
                                        scalar1=1.0 / IHD, scalar2=None,
                                        op0=mybir.AluOpType.mult)
                nc.vector.tensor_scalar(out=mom[:, 1:2], in0=s1[:, 1:2],
                                        scalar1=1.0 / IHD, scalar2=None,
                                        op0=mybir.AluOpType.mult)
                nc.vector.tensor_mul(mom[:, 2:3], mom[:, 0:1], mom[:, 0:1])
                nc.vector.tensor_sub(mom[:, 2:3], mom[:, 1:2], mom[:, 2:3])
                nc.vector.tensor_scalar(out=mom[:, 2:3], in0=mom[:, 2:3],
                                        scalar1=1e-5, scalar2=None,
                                        op0=mybir.AluOpType.add)
                nc.scalar.activation(out=mom[:, 2:3], in_=mom[:, 2:3],
                                     func=mybir.ActivationFunctionType.Sqrt)
                nc.vector.reciprocal(out=mom[:, 3:4], in_=mom[:, 2:3])
                ki_n = p1.tile([128, IHD], F32, tag="ki_n")
                nc.vector.tensor_scalar(out=ki_n, in0=ps_ki,
                                        scalar1=mom[:, 0:1],
                                        scalar2=mom[:, 3:4],
                                        op0=mybir.AluOpType.subtract,
                                        op1=mybir.AluOpType.mult)
                nc.vector.tensor_mul(ki_n, ki_n, knw)
                nc.vector.tensor_add(ki_n, ki_n, knb)
                ki_r = p1.tile([128, IHD], F32, tag="ki_r")
                _rope_ni(nc, ki_r, ki_n, cos_t[:, t, :], sin_t[:, t, :])
                ptr = p1tr.tile([128, 128], F32, tag="ptr")
                nc.tensor.transpose(ptr[:64, :], ki_r, ident)
                nc.scalar.copy(out=kiT[:, t * 128:(t + 1) * 128],
                               in_=ptr[:64, :])

        # ---------------- P2: query-block projections ----------------
        mid = tc.alloc_tile_pool(name="mid", bufs=1)
        qTn = mid.tile([128, H, NB], F32R)       # nope part, feature-major
        qTp = mid.tile([64, H, NB], F32R)        # rope part
        qiT = mid.tile([64, IH, NB], F32R)       # indexer q, gated+scaled

        with tc.tile_pool(name="p2w", bufs=2) as p2w, \
             tc.tile_pool(name="p2", bufs=2) as p2, \
             tc.tile_pool(name="p2ps", bufs=1, space="PSUM") as p2ps, \
             tc.tile_pool(name="p2tr", bufs=1, space="PSUM") as p2tr:
            cosb = p2.tile([128, NQT, DR], F32, tag="cosb", bufs=1)
            sinb = p2.tile([128, NQT, DR], F32, tag="sinb", bufs=1)
            nc.gpsimd.dma_start(out=cosb, in_=cosb_d.rearrange(
                "(t p) d -> p t d", p=128))
            nc.gpsimd.dma_start(out=sinb, in_=sinb_d.rearrange(
                "(t p) d -> p t d", p=128))
            qnw = p2.tile([128, QLR], F32, tag="qnw", bufs=1)
            nc.gpsimd.dma_start(out=qnw, in_=_bcast(qnw_d))
            xtb_r = xTb.rearrange("(c p) n -> p c n", p=128)
            ps_qr = [p2ps.tile([128, 512], F32, tag=f"ps_qr{q}{i}",
                               name=f"ps_qr{q}{i}")
                     for q in range(NQT) for i in range(2)]
            ps_g = [p2ps.tile([128, IH], F32, tag=f"ps_g{q}",
                              name=f"ps_g{q}") for q in range(NQT)]
            xtb_tiles = []
            for f in range(NT):
                wqa_f = p2w.tile([128, QLR], F32R, tag="wqa_f")
                nc.gpsimd.dma_start(out=wqa_f,
                                  in_=wq_a[f * 128:(f + 1) * 128, :])
                ig_f = p2w.tile([128, IH], F32R, tag="ig_f")
                nc.gpsimd.dma_start(out=ig_f,
                                  in_=igate[f * 128:(f + 1) * 128, :])
                xtb_f = p2w.tile([128, NB], F32R, tag="xtb_f", bufs=3)
                nc.gpsimd.dma_start(out=xtb_f, in_=xtb_r[:, f, :])
                st, sp = (f == 0), (f == NT - 1)
                for q in range(NQT):
                    lhs = xtb_f[:, q * 128:(q + 1) * 128]
                    nc.tensor.matmul(ps_qr[2 * q], lhs,
                                     wqa_f[:, 0:512],
                                     start=st, stop=sp)
                    nc.tensor.matmul(ps_qr[2 * q + 1], lhs,
                                     wqa_f[:, 512:1024],
                                     start=st, stop=sp)
                    nc.tensor.matmul(ps_g[q], lhs, ig_f,
                                     start=st, stop=sp)
            qrT = p2.tile([128, 8, NB], F32R, tag="qrT", bufs=1)
            gate_sb = p2.tile([128, NQT, IH], F32, tag="gate_sb", bufs=1)
            for q in range(NQT):
                qr_sb = p2.tile([128, QLR], F32, tag="qr_sb")
                _rmsnorm_from_psum(nc, p2, qr_sb,
                                   [ps_qr[2 * q], ps_qr[2 * q + 1]], qnw, QLR)
                nc.vector.tensor_scalar(out=gate_sb[:, q, :], in0=ps_g[q],
                                        scalar1=SCALE_GATE * SCALE_IDX,
                                        scalar2=None,
                                        op0=mybir.AluOpType.mult)
                for ch in range(8):
                    ptr = p2tr.tile([128, 128], F32, tag="ptr2")
                    nc.tensor.transpose(ptr, qr_sb[:, ch * 128:(ch + 1) * 128],
                                        ident)
                    nc.scalar.copy(out=qrT[:, ch, q * 128:(q + 1) * 128],
                                   in_=ptr)
            # q projection per MLA head: token-major [128, 192] -> rope/scale
            # -> transpose to qTn/qTp
            for h in range(H):
                wqb_h = p2w.tile([128, 8, DN + DR], F32R, tag="wqb_h")
                wqbr = wq_b.rearrange("(c p) n -> p c n", p=128)
                for c in range(8):
                    nc.gpsimd.dma_start(
                        out=wqb_h[:, c, :],
                        in_=wqbr[:, c, h * (DN + DR):(h + 1) * (DN + DR)])
                for q in range(NQT):
                    ps_q = p2ps.tile([128, DN + DR], F32, tag="ps_q")
                    for ch in range(8):
                        nc.tensor.matmul(
                            ps_q, qrT[:, ch, q * 128:(q + 1) * 128],
                            wqb_h[:, ch, :],
                            start=(ch == 0), stop=(ch == 7))
                    q_sb = p2.tile([128, DN + DR], F32, tag="q_sb")
                    nc.vector.tensor_scalar(out=q_sb[:, 0:DN],
                                            in0=ps_q[:, 0:DN],
                                            scalar1=SCALE_MLA, scalar2=None,
                                            op0=mybir.AluOpType.mult)
                    _rope_int(nc, q_sb[:, DN:], ps_q[:, DN:],
                              cosb[:, q, :], sinb[:, q, :])
                    nc.vector.tensor_scalar(out=q_sb[:, DN:], in0=q_sb[:, DN:],
                                            scalar1=SCALE_MLA, scalar2=None,
                                            op0=mybir.AluOpType.mult)
                    ptr = p2tr.tile([128, 128], F32, tag="ptr2")
                    nc.tensor.transpose(ptr, q_sb[:, 0:DN], ident)
                    nc.scalar.copy(out=qTn[:, h, q * 128:(q + 1) * 128],
                                   in_=ptr)
                    ptr = p2tr.tile([128, 128], F32, tag="ptr2")
                    nc.tensor.transpose(ptr[:64, :], q_sb[:, DN:], ident)
                    nc.scalar.copy(out=qTp[:, h, q * 128:(q + 1) * 128],
                                   in_=ptr[:64, :])
            # indexer q heads: rope, * gate * scale, transpose
            for ih in range(IH):
                wiq_h = p2w.tile([128, 8, IHD], F32R, tag="wiq_h")
                wiqr = iwqb.rearrange("(c p) n -> p c n", p=128)
                for c in range(8):
                    nc.gpsimd.dma_start(
                        out=wiq_h[:, c, :],
                        in_=wiqr[:, c, ih * IHD:(ih + 1) * IHD])
                for q in range(NQT):
                    ps_qi_full = p2ps.tile([128, DN + DR], F32, tag="ps_q")
                    ps_qi = ps_qi_full[:, 0:IHD]
                    for ch in range(8):
                        nc.tensor.matmul(
                            ps_qi,
                            qrT[:, ch, q * 128:(q + 1) * 128],
                            wiq_h[:, ch, :],
                            start=(ch == 0), stop=(ch == 7))
                    qi_sb = p2.tile([128, IHD], F32, tag="qi_sb")
                    _rope_ni(nc, qi_sb, ps_qi, cosb[:, q, :], sinb[:, q, :])
                    nc.vector.tensor_scalar(out=qi_sb, in0=qi_sb,
                                            scalar1=gate_sb[:, q, ih:ih + 1],
                                            scalar2=None,
                                            op0=mybir.AluOpType.mult)
                    ptr = p2tr.tile([128, 128], F32, tag="ptr2")
                    nc.tensor.transpose(ptr[:64, :], qi_sb, ident)
                    nc.scalar.copy(out=qiT[:, ih, q * 128:(q + 1) * 128],
                                   in_=ptr[:64, :])

        # ---------------- P3: index scores + top-k threshold ----------------
        maskNEG = mid.tile([128, NQT, S], F32)
        with tc.tile_pool(name="p3", bufs=1) as p3, \
             tc.tile_pool(name="p3ps", bufs=4, space="PSUM") as p3ps:
            amask = p3.tile([128, NQT, S], F32)
            nc.gpsimd.dma_start(out=amask, in_=amask_d.rearrange(
                "(t p) s -> p t s", p=128))
            for q in range(NQT):
                isc = p3.tile([128, S], F32, tag="isc")
                for kc in range(4):
                    ps = p3ps.tile([128, 512], F32, tag="ps_isc")
                    for ih in range(IH):
                        nc.tensor.matmul(
                            ps, qiT[:, ih, q * 128:(q + 1) * 128],
                            kiT[:, kc * 512:(kc + 1) * 512],
                            start=(ih == 0), stop=(ih == IH - 1))
                    nc.vector.tensor_add(isc[:, kc * 512:(kc + 1) * 512], ps,
                                         amask[:, q, kc * 512:(kc + 1) * 512])
                # clamp masked scores to -200 so secant operates in a
                # uniform value range (attn_mask re-kills them later)
                nc.vector.tensor_scalar(out=isc, in0=isc, scalar1=-200.0,
                                        scalar2=None, op0=mybir.AluOpType.max)
                # bracket probes from stride-8 sample: rank38 / rank26
                samp = p3.tile([128, 256], F32, tag="samp")
                nc.vector.tensor_copy(
                    samp, isc.rearrange("p (a b) -> p a b", b=8)[:, :, 0])
                mx = p3.tile([128, 8], F32, tag="mx")
                probe_hi = p3.tile([128, 1], F32, tag="probe_hi")
                for r in range(5):
                    nc.vector.max(out=mx, in_=samp)
                    if r == 3:  # ranks 25..32; idx1 = rank 26
                        nc.vector.tensor_copy(probe_hi, mx[:, 1:2])
                    if r < 4:
                        nc.vector.match_replace(out=samp, in_to_replace=mx,
                                                in_values=samp,
                                                imm_value=-3e9)
                # st cols: 0 lo, 1 hi, 2 flo, 3 fhi, 4 t, 5 c, 6 p, 7 np, 8 last
                st = p3.tile([128, 9], F32, tag="st")
                nc.vector.memset(st[:, 0:1], -300.0)
                nc.vector.memset(st[:, 1:2], 200.0)
                nc.vector.memset(st[:, 2:3], float(S - TOPK))
                nc.vector.memset(st[:, 3:4], -float(TOPK))
                nc.vector.memset(st[:, 8:9], 0.0)
                nc.vector.tensor_copy(st[:, 4:5], mx[:, 5:6])  # rank 38
                scr = p3.tile([128, S], F32, tag="cnt_scr")
                d3 = p3.tile([128, 3], F32, tag="d3")
                predu = p3.tile([128, 4], mybir.dt.uint8, tag="predu")
                for it in range(SEL_ITERS):
                    nc.vector.tensor_scalar(out=scr, in0=isc,
                                            scalar1=st[:, 4:5], scalar2=None,
                                            op0=mybir.AluOpType.is_ge,
                                            op1=mybir.AluOpType.add,
                                            accum_out=st[:, 5:6])
                    # f = c - K; p = f >= 0
                    nc.vector.tensor_scalar(out=d3[:, 0:1], in0=st[:, 5:6],
                                            scalar1=-float(TOPK), scalar2=None,
                                            op0=mybir.AluOpType.add)
                    nc.vector.tensor_scalar(out=st[:, 6:7], in0=d3[:, 0:1],
                                            scalar1=0.0, scalar2=None,
                                            op0=mybir.AluOpType.is_ge)
                    nc.vector.tensor_scalar(out=st[:, 7:8], in0=d3[:, 0:1],
                                            scalar1=0.0, scalar2=None,
                                            op0=mybir.AluOpType.is_lt)
                    # Illinois damping: same side twice -> halve other f
                    nc.vector.tensor_scalar(out=d3[:, 1:2], in0=st[:, 8:9],
                                            scalar1=0.0, scalar2=None,
                                            op0=mybir.AluOpType.is_gt)
                    nc.vector.tensor_mul(d3[:, 1:2], d3[:, 1:2], st[:, 6:7])
                    nc.vector.tensor_copy(predu[:, 2:3], d3[:, 1:2])
                    nc.vector.tensor_scalar(out=d3[:, 2:3], in0=st[:, 3:4],
                                            scalar1=0.5, scalar2=None,
                                            op0=mybir.AluOpType.mult)
                    nc.vector.copy_predicated(st[:, 3:4], predu[:, 2:3],
                                              d3[:, 2:3])
                    nc.vector.tensor_scalar(out=d3[:, 1:2], in0=st[:, 8:9],
                                            scalar1=0.0, scalar2=None,
                                            op0=mybir.AluOpType.is_lt)
                    nc.vector.tensor_mul(d3[:, 1:2], d3[:, 1:2], st[:, 7:8])
                    nc.vector.tensor_copy(predu[:, 3:4], d3[:, 1:2])
                    nc.vector.tensor_scalar(out=d3[:, 2:3], in0=st[:, 2:3],
                                            scalar1=0.5, scalar2=None,
                                            op0=mybir.AluOpType.mult)
                    nc.vector.copy_predicated(st[:, 2:3], predu[:, 3:4],
                                              d3[:, 2:3])
                    # bracket updates
                    nc.vector.tensor_copy(predu[:, 0:1], st[:, 6:7])
                    nc.vector.tensor_copy(predu[:, 1:2], st[:, 7:8])
                    nc.vector.copy_predicated(st[:, 0:1], predu[:, 0:1],
                                              st[:, 4:5])
                    nc.vector.copy_predicated(st[:, 2:3], predu[:, 0:1],
                                              d3[:, 0:1])
                    nc.vector.copy_predicated(st[:, 1:2], predu[:, 1:2],
                                              st[:, 4:5])
                    nc.vector.copy_predicated(st[:, 3:4], predu[:, 1:2],
                                              d3[:, 0:1])
                    nc.vector.tensor_sub(st[:, 8:9], st[:, 6:7], st[:, 7:8])
                    if it == SEL_ITERS - 1:
                        break
                    if it == 0:
                        nc.vector.tensor_copy(st[:, 4:5], probe_hi)
                        continue
                    # t = hi - fhi*(hi-lo)/(fhi-flo)
                    nc.vector.tensor_sub(d3[:, 1:2], st[:, 1:2], st[:, 0:1])
                    nc.vector.tensor_mul(d3[:, 1:2], d3[:, 1:2], st[:, 3:4])
                    nc.vector.tensor_sub(d3[:, 2:3], st[:, 3:4], st[:, 2:3])
                    nc.vector.reciprocal(out=d3[:, 2:3], in_=d3[:, 2:3])
                    nc.vector.tensor_mul(d3[:, 1:2], d3[:, 1:2], d3[:, 2:3])
                    nc.vector.tensor_sub(st[:, 4:5], st[:, 1:2], d3[:, 1:2])
                # final threshold = lo (count >= K guaranteed)
                nc.vector.tensor_scalar(out=maskNEG[:, q, :], in0=isc,
                                        scalar1=st[:, 0:1], scalar2=NEG,
                                        op0=mybir.AluOpType.is_lt,
                                        op1=mybir.AluOpType.mult)
                nc.vector.tensor_add(maskNEG[:, q, :], maskNEG[:, q, :],
                                     amask[:, q, :])

        # ---------------- P4: sparse MLA attention per head ----------------
        out_hT = mid.tile([128, H, NB], F32R)
        with tc.tile_pool(name="p4w", bufs=2) as p4w, \
             tc.tile_pool(name="p4k", bufs=2) as p4k, \
             tc.tile_pool(name="p4p", bufs=2) as p4p, \
             tc.tile_pool(name="p4ps", bufs=2, space="PSUM") as p4ps, \
             tc.tile_pool(name="p4po", bufs=2, space="PSUM") as p4po:
            for h in range(H):
                wb_k = p4w.tile([128, 4, DN], F32R, tag="wb_k")
                wb_v = p4w.tile([128, 4, DV], F32R, tag="wb_v")
                wbr = wkv_b.rearrange("(c p) n -> p c n", p=128)
                for c in range(4):
                    nc.gpsimd.dma_start(
                        out=wb_k[:, c, :],
                        in_=wbr[:, c, h * (DN + DV):h * (DN + DV) + DN])
                    nc.gpsimd.dma_start(
                        out=wb_v[:, c, :],
                        in_=wbr[:, c, h * (DN + DV) + DN:(h + 1) * (DN + DV)])
                knT = p4k.tile([128, S], F32R, tag="knT")
                for kc in range(4):
                    ps = p4ps.tile([128, 512], F32, tag="ps_kn")
                    for c in range(4):
                        nc.tensor.matmul(
                            ps, wb_k[:, c, :],
                            ckvT[:, c, kc * 512:(kc + 1) * 512],
                            start=(c == 0), stop=(c == 3))
                    nc.scalar.copy(out=knT[:, kc * 512:(kc + 1) * 512], in_=ps)
                v_sb = p4k.tile([128, NT, DV], BF16, tag="v_sb")
                for kt in range(NT):
                    ps = p4ps.tile([128, DV], F32, tag="ps_v")
                    for c in range(4):
                        nc.tensor.matmul(
                            ps,
                            ckvT[:, c, kt * 128:(kt + 1) * 128],
                            wb_v[:, c, :],
                            start=(c == 0), stop=(c == 3))
                    nc.scalar.copy(out=v_sb[:, kt, :], in_=ps)
                ps_o = p4po.tile([128, NB], F32, tag="ps_o")
                for q in range(NQT):
                    probs = p4p.tile([128, S], F32, tag="probs", bufs=1)
                    for kc in range(4):
                        ps = p4ps.tile([128, 512], F32, tag="ps_s")
                        nc.tensor.matmul(
                            ps, qTn[:, h, q * 128:(q + 1) * 128],
                            knT[:, kc * 512:(kc + 1) * 512],
                            start=True, stop=False)
                        nc.tensor.matmul(
                            ps, qTp[:, h, q * 128:(q + 1) * 128],
                            kpeT[:, kc * 512:(kc + 1) * 512],
                            start=False, stop=True)
                        nc.vector.tensor_add(
                            probs[:, kc * 512:(kc + 1) * 512], ps,
                            maskNEG[:, q, kc * 512:(kc + 1) * 512])
                    den = p4p.tile([128, 2], F32, tag="den")
                    nc.scalar.activation(out=probs, in_=probs,
                                         func=mybir.ActivationFunctionType.Exp,
                                         accum_out=den[:, 0:1])
                    nc.vector.reciprocal(out=den[:, 1:2], in_=den[:, 0:1])
                    pb = p4p.tile([128, S], BF16, tag="pb")
                    nc.vector.tensor_scalar(out=pb, in0=probs,
                                            scalar1=den[:, 1:2], scalar2=None,
                                            op0=mybir.AluOpType.mult)
                    pT = p4p.tile([128, NT, 128], BF16, tag="pT", bufs=1)
                    for kt in range(NT):
                        nc.scalar.dma_start_transpose(
                            out=pT[:, kt, :],
                            in_=pb[:, kt * 128:(kt + 1) * 128])
                    for kt in range(NT):
                        nc.tensor.matmul(
                            ps_o[:, q * 128:(q + 1) * 128],
                            v_sb[:, kt, :], pT[:, kt, :],
                            start=(kt == 0), stop=(kt == NT - 1))
                nc.scalar.copy(out=out_hT[:, h, :], in_=ps_o)

        # ---------------- P5: output projection ----------------
        with tc.tile_pool(name="p5w", bufs=3) as p5w, \
             tc.tile_pool(name="p5", bufs=3) as p5, \
             tc.tile_pool(name="p5ps", bufs=4, space="PSUM") as p5ps:
            for g in range(NT):
                wo_g = p5w.tile([128, H, 128], F32R, tag="wo_g")
                wor = wo.rearrange("(hh p) n -> p hh n", p=128)
                for c in range(H):
                    nc.gpsimd.dma_start(
                        out=wo_g[:, c, :],
                        in_=wor[:, c, g * 128:(g + 1) * 128])
                ps = p5ps.tile([128, NB], F32, tag="ps_w")
                for h in range(H):
                    nc.tensor.matmul(ps, wo_g[:, h, :],
                                     out_hT[:, h, :],
                                     start=(h == 0), stop=(h == H - 1))
                ot = p5.tile([128, NB], F32, tag="ot")
                nc.scalar.copy(out=ot, in_=ps)
                nc.gpsimd.dma_start(out=outT[g * 128:(g + 1) * 128, :], in_=ot)

        mid.release()
        consts.release()
    nc.compile()
    return nc


_NC_CACHE = None


def _get_nc():
    global _NC_CACHE
    if _NC_CACHE is None:
        _NC_CACHE = build_nc()
    return _NC_CACHE


def make_core_inputs(x, cos, sin, attn_mask, wq_a, q_norm_w, wq_b, wkv_a,
                     kv_norm_w, wkv_b, wo, idx_wq_b, idx_wk, idx_knorm_w,
                     idx_knorm_b, idx_gate):
    x2 = np.ascontiguousarray(x[0].astype(np.float32))        # [S, HID]
    xT = np.ascontiguousarray(x2.T)                           # [HID, S]
    cos2 = np.ascontiguousarray(cos[0].astype(np.float32))
    sin2 = np.ascontiguousarray(sin[0].astype(np.float32))
    am = np.ascontiguousarray(attn_mask[0, 0].astype(np.float32))
    ident = np.eye(128, dtype=np.float32)
    shared = dict(
        xT=xT, cos_t=cos2, sin_t=sin2,
        wq_a=np.ascontiguousarray(wq_a, np.float32),
        wq_b=np.ascontiguousarray(wq_b, np.float32),
        wkv_a=np.ascontiguousarray(wkv_a, np.float32),
        wkv_b=np.ascontiguousarray(wkv_b, np.float32),
        wo=np.ascontiguousarray(wo, np.float32),
        idx_wq_b=np.ascontiguousarray(idx_wq_b, np.float32),
        idx_wk=np.ascontiguousarray(idx_wk, np.float32),
        idx_gate=np.ascontiguousarray(idx_gate, np.float32),
        q_norm_w=np.ascontiguousarray(q_norm_w, np.float32),
        kv_norm_w=np.ascontiguousarray(kv_norm_w, np.float32),
        idx_knorm_w=np.ascontiguousarray(idx_knorm_w, np.float32),
        idx_knorm_b=np.ascontiguousarray(idx_knorm_b, np.float32),
        ident=ident,
    )
    maps = []
    for c in range(NCORES):
        r0, r1 = c * NB, (c + 1) * NB
        m = dict(shared)
        m["xTb"] = np.ascontiguousarray(xT[:, r0:r1])
        m["cosb"] = np.ascontiguousarray(cos2[r0:r1])
        m["sinb"] = np.ascontiguousarray(sin2[r0:r1])
        m["amask"] = np.ascontiguousarray(am[r0:r1])
        maps.append(m)
    return maps


def kernel(x, cos, sin, attn_mask, wq_a, q_norm_w, wq_b, wkv_a, kv_norm_w,
           wkv_b, wo, idx_wq_b, idx_wk, idx_knorm_w, idx_knorm_b, idx_gate):
    from concourse.bass_utils import run_bass_kernel_spmd
    nc = _get_nc()
    maps = make_core_inputs(x, cos, sin, attn_mask, wq_a, q_norm_w, wq_b,
                            wkv_a, kv_norm_w, wkv_b, wo, idx_wq_b, idx_wk,
                            idx_knorm_w, idx_knorm_b, idx_gate)
    res = run_bass_kernel_spmd(nc, maps, list(range(NCORES)))
    outs = [np.asarray(r["outT"]).T for r in res.results]      # [NB, HID] each
    out = np.concatenate(outs, axis=0)[None]                   # [1, S, HID]
    return out.astype(np.float32)



# revision 9
# speedup vs baseline: 6.5747x; 6.5747x over previous
"""DSA sparse MLA attention kernel for TRN2, 8 NeuronCores.

Transfer-optimized SPMD design. The host->device tunnel moves ~52 MB/s,
so every input byte is shipped exactly ONCE: each core receives a 1/8
slice of two packed blobs (bf16 + f32) and the cores AllGather them
on-device (HBM-to-HBM over on-chip links, ~GB/ms). Per-core query-block
slices are carved out of the gathered blobs at runtime with
partition_id()-based dynamic DMA offsets, so no per-core host tensors
are needed at all.

Precision plan (harness gate: rel_err < 2e-2; this lands ~6e-3):
 - Indexer path (x, wq_a, idx_*) is selection-critical: tensors are
   shipped as hi/lo bf16 pairs (same bytes as f32) and matmuls use a
   3-pass hi/lo bf16 emulation (~1e-5 rel, 4x faster than fp32r which
   is only ~1e-3 accurate).
 - Top-256 selection is EXACT: 32 rounds of vector.max + match_replace
   give the true 256th-largest index score per row.
 - Attention path (wq_b, wkv_a, wkv_b, wo, q/k/v, probs) is plain bf16.
 - Output is fp16 (halves the donated-zeros upload + fetch).

Sharding: sequence-parallel over query rows; core c owns rows
[256c, 256(c+1)). KV/indexer-key expansion over all 2048 keys is
replicated (compute is ~free vs transfer).
"""

import numpy as np
import ml_dtypes

import concourse.bass as bass
import concourse.bacc as bacc
import concourse.mybir as mybir
from concourse.tile import TileContext

F32 = mybir.dt.float32
BF16 = mybir.dt.bfloat16
FP16 = mybir.dt.float16

S, HID = 2048, 2048
H, DN, DR, DV = 16, 128, 64, 128
QLR, KVLR = 1024, 512
IH, IHD, TOPK = 8, 64, 256
NEG = -1e9
NB = 256            # query rows per core
NCORES = 8
NT = S // 128       # 16 token tiles
NQT = NB // 128     # 2 query tiles per core
SCALE_MLA = float((DN + DR) ** -0.5)
SCALE_IDX = float(IHD ** -0.5)
SCALE_GATE = float(IH ** -0.5)
ALIGN = 512         # element alignment for blob entries

BF_LAYOUT = [
    ("xT_hi", (HID, S)), ("xT_lo", (HID, S)),
    ("wqa_hi", (HID, QLR)), ("wqa_lo", (HID, QLR)),
    ("wq_b", (QLR, H * (DN + DR))),
    ("wkv_a", (HID, KVLR + DR)),
    ("wkv_b", (KVLR, H * (DN + DV))),
    ("wo", (H * DV, HID)),
    ("iwqb_hi", (QLR, IH * IHD)), ("iwqb_lo", (QLR, IH * IHD)),
    ("iwk_hi", (HID, IHD)), ("iwk_lo", (HID, IHD)),
    ("igate_hi", (HID, IH)), ("igate_lo", (HID, IH)),
    ("ident", (128, 128)),
]

F32_LAYOUT_BASE = [
    ("cos", (S, DR)), ("sin", (S, DR)),
    ("q_norm_w", (1, QLR)), ("kv_norm_w", (1, KVLR)),
    ("idx_knorm_w", (1, IHD)), ("idx_knorm_b", (1, IHD)),
    ("iota", (1, S)),
    ("rows", (NCORES * 128, 1)),
]


def _mk_layout(entries):
    offs, off = {}, 0
    for name, shape in entries:
        offs[name] = off
        n = int(np.prod(shape))
        off += (n + ALIGN - 1) // ALIGN * ALIGN
    tot = (off + NCORES * ALIGN - 1) // (NCORES * ALIGN) * (NCORES * ALIGN)
    return offs, tot


def _f32_layout(causal):
    ents = list(F32_LAYOUT_BASE)
    if not causal:
        ents.append(("amask", (S, S)))
    return _mk_layout(ents)


BF_OFF, BF_TOT = _mk_layout(BF_LAYOUT)
LB = BF_TOT // NCORES


def _v(blob1d, off, r, c):
    """[r, c] row-major view at element offset off of a 1-D DRAM AP."""
    return blob1d[off:off + r * c].rearrange("(r c) -> r c", c=c)


def _vb(blob1d, off, n, parts=128):
    """Partition-broadcast view [parts, n] of n elements at offset off."""
    return bass.AP(tensor=blob1d.tensor, offset=blob1d.offset + off,
                   ap=[[0, parts], [1, n]])


def _rmsnorm_from_psum(nc, pool, out_sb, psums, wb, d, eps=1e-6):
    """out_sb[p, d] = psum * rsqrt(mean(psum^2)+eps) * w."""
    ssq = pool.tile([128, len(psums)], F32)
    for i, ps in enumerate(psums):
        w = ps.shape[-1]
        scr = pool.tile([128, 512], F32, tag="rms_scr")
        nc.scalar.activation(out=scr[:, :w], in_=ps,
                             func=mybir.ActivationFunctionType.Square,
                             accum_out=ssq[:, i:i + 1])
    tot = pool.tile([128, 1], F32)
    if len(psums) == 1:
        nc.vector.tensor_scalar(out=tot, in0=ssq, scalar1=1.0 / d,
                                scalar2=eps, op0=mybir.AluOpType.mult,
                                op1=mybir.AluOpType.add)
    else:
        nc.vector.tensor_reduce(out=tot, in_=ssq, axis=mybir.AxisListType.X,
                                op=mybir.AluOpType.add)
        nc.vector.tensor_scalar(out=tot, in0=tot, scalar1=1.0 / d,
                                scalar2=eps, op0=mybir.AluOpType.mult,
                                op1=mybir.AluOpType.add)
    nc.scalar.activation(out=tot, in_=tot,
                         func=mybir.ActivationFunctionType.Sqrt)
    rinv = pool.tile([128, 1], F32)
    nc.vector.reciprocal(out=rinv, in_=tot)
    off = 0
    for ps in psums:
        w = ps.shape[-1]
        nc.vector.tensor_scalar(out=out_sb[:, off:off + w], in0=ps,
                                scalar1=rinv, scalar2=None,
                                op0=mybir.AluOpType.mult)
        off += w
    nc.vector.tensor_mul(out_sb[:, :d], out_sb[:, :d], wb[:, :d])


def _rope_int(nc, out, in_, cos, sin):
    """Interleaved (GPT-J) rope, token-major [128, 64] -> out[128, 64]."""
    xp = in_.rearrange("p (a b) -> p a b", b=2)
    op = out.rearrange("p (a b) -> p a b", b=2)
    c, s = cos[:, 0:32], sin[:, 0:32]
    x1, x2 = xp[:, :, 0], xp[:, :, 1]
    nc.vector.tensor_mul(op[:, :, 0], x1, c)
    nc.vector.tensor_mul(op[:, :, 1], x2, c)
    t = nc._rope_scr.tile([128, 32], F32, tag="rope_t")
    nc.vector.tensor_mul(t, x2, s)
    nc.vector.tensor_sub(op[:, :, 0], op[:, :, 0], t)
    nc.vector.tensor_mul(t, x1, s)
    nc.vector.tensor_add(op[:, :, 1], op[:, :, 1], t)


def _rope_ni(nc, out, in_, cos, sin):
    """Non-interleaved (rotate_half) rope, [128, 64]."""
    x1, x2 = in_[:, 0:32], in_[:, 32:64]
    c1, c2 = cos[:, 0:32], cos[:, 32:64]
    s1, s2 = sin[:, 0:32], sin[:, 32:64]
    nc.vector.tensor_mul(out[:, 0:32], x1, c1)
    nc.vector.tensor_mul(out[:, 32:64], x2, c2)
    t = nc._rope_scr.tile([128, 32], F32, tag="rope_t")
    nc.vector.tensor_mul(t, x2, s1)
    nc.vector.tensor_sub(out[:, 0:32], out[:, 0:32], t)
    nc.vector.tensor_mul(t, x1, s2)
    nc.vector.tensor_add(out[:, 32:64], out[:, 32:64], t)


def _split(nc, pool, src_f32, n, tag):
    """f32 [128, n] -> (hi bf16, lo bf16) with hi+lo ~= src."""
    hi = pool.tile([128, n], BF16, tag=tag + "_hi")
    nc.vector.tensor_copy(hi, src_f32)
    hi32 = pool.tile([128, n], F32, tag=tag + "_h32")
    nc.vector.tensor_copy(hi32, hi)
    lo32 = pool.tile([128, n], F32, tag=tag + "_l32")
    nc.vector.tensor_sub(lo32, src_f32, hi32)
    lo = pool.tile([128, n], BF16, tag=tag + "_lo")
    nc.vector.tensor_copy(lo, lo32)
    return hi, lo


def build_nc(causal=True, dbg=False):
    F32_OFF, F32_TOT = _f32_layout(causal)
    lf = F32_TOT // NCORES

    nc = bacc.Bacc("TRN2", target_bir_lowering=False, debug=False)
    shard_bf = nc.dram_tensor("shard_bf", [LB], BF16, kind="ExternalInput").ap()
    shard_f32 = nc.dram_tensor("shard_f32", [lf], F32,
                               kind="ExternalInput").ap()
    outT = nc.dram_tensor("outT", [HID, NB], FP16, kind="ExternalOutput").ap()
    if dbg:
        d_ckvT = nc.dram_tensor("d_ckvT", [128, 4 * S], BF16,
                                kind="ExternalOutput").ap()
        d_kpeT = nc.dram_tensor("d_kpeT", [64, S], BF16,
                                kind="ExternalOutput").ap()
        d_kiT = nc.dram_tensor("d_kiT", [64, 2 * S], BF16,
                               kind="ExternalOutput").ap()
        d_qrT = nc.dram_tensor("d_qrT", [128, 2 * 8 * NB], BF16,
                               kind="ExternalOutput").ap()
        d_gate = nc.dram_tensor("d_gate", [128, NQT * IH], F32,
                                kind="ExternalOutput").ap()
        d_mask = nc.dram_tensor("d_mask", [128, NQT * S], F32,
                                kind="ExternalOutput").ap()
        d_thr = nc.dram_tensor("d_thr", [128, NQT], F32,
                               kind="ExternalOutput").ap()
        d_qiT = nc.dram_tensor("d_qiT", [64, 2 * IH * NB], BF16,
                               kind="ExternalOutput").ap()
        d_ohT = nc.dram_tensor("d_ohT", [128, H * NB], BF16,
                               kind="ExternalOutput").ap()

    with TileContext(nc) as tc:
        pid = nc.partition_id()
        r0 = pid * NB

        dram = tc.alloc_tile_pool(name="dram", bufs=1, space="DRAM")
        bounce_bf = dram.tile([LB], BF16)
        bounce_f32 = dram.tile([lf], F32)
        gath_bf = dram.tile([NCORES, LB], BF16)
        gath_f32 = dram.tile([NCORES, lf], F32)
        nc.gpsimd.dma_start(out=bounce_bf, in_=shard_bf)
        nc.gpsimd.dma_start(out=bounce_f32, in_=shard_f32)
        nc.gpsimd.collective_compute(
            "AllGather", mybir.AluOpType.bypass,
            replica_groups=[list(range(NCORES))],
            ins=[bounce_bf[:].opt()], outs=[gath_bf[:].opt()])
        nc.gpsimd.collective_compute(
            "AllGather", mybir.AluOpType.bypass,
            replica_groups=[list(range(NCORES))],
            ins=[bounce_f32[:].opt()], outs=[gath_f32[:].opt()])
        gb = gath_bf.rearrange("a b -> (a b)")
        gf = gath_f32.rearrange("a b -> (a b)")

        xTh_v = _v(gb, BF_OFF["xT_hi"], HID, S)
        xTl_v = _v(gb, BF_OFF["xT_lo"], HID, S)
        wqah_v = _v(gb, BF_OFF["wqa_hi"], HID, QLR)
        wqal_v = _v(gb, BF_OFF["wqa_lo"], HID, QLR)
        wqb_v = _v(gb, BF_OFF["wq_b"], QLR, H * (DN + DR))
        wkva_v = _v(gb, BF_OFF["wkv_a"], HID, KVLR + DR)
        wkvb_v = _v(gb, BF_OFF["wkv_b"], KVLR, H * (DN + DV))
        wo_v = _v(gb, BF_OFF["wo"], H * DV, HID)
        iwqbh_v = _v(gb, BF_OFF["iwqb_hi"], QLR, IH * IHD)
        iwqbl_v = _v(gb, BF_OFF["iwqb_lo"], QLR, IH * IHD)
        iwkh_v = _v(gb, BF_OFF["iwk_hi"], HID, IHD)
        iwkl_v = _v(gb, BF_OFF["iwk_lo"], HID, IHD)
        igh_v = _v(gb, BF_OFF["igate_hi"], HID, IH)
        igl_v = _v(gb, BF_OFF["igate_lo"], HID, IH)
        ident_v = _v(gb, BF_OFF["ident"], 128, 128)
        cos_v = _v(gf, F32_OFF["cos"], S, DR)
        sin_v = _v(gf, F32_OFF["sin"], S, DR)
        rows_v = _v(gf, F32_OFF["rows"], NCORES * 128, 1)

        consts = tc.alloc_tile_pool(name="consts", bufs=1)
        nc._rope_scr = consts

        ident = consts.tile([128, 128], BF16)
        nc.gpsimd.dma_start(out=ident, in_=ident_v)
        kvnw = consts.tile([128, KVLR], F32)
        nc.gpsimd.dma_start(out=kvnw, in_=_vb(gf, F32_OFF["kv_norm_w"], KVLR))
        knw = consts.tile([128, IHD], F32)
        nc.gpsimd.dma_start(out=knw, in_=_vb(gf, F32_OFF["idx_knorm_w"], IHD))
        knb = consts.tile([128, IHD], F32)
        nc.gpsimd.dma_start(out=knb, in_=_vb(gf, F32_OFF["idx_knorm_b"], IHD))
        iota_sb = consts.tile([128, S], F32)
        nc.gpsimd.dma_start(out=iota_sb, in_=_vb(gf, F32_OFF["iota"], S))
        rowid = consts.tile([128, 1], F32)
        nc.gpsimd.dma_start(out=rowid, in_=rows_v[bass.ds(pid * 128, 128), :])

        ckvT = consts.tile([128, 4, S], BF16)      # [ckv_chunk, 4, tok]
        kpeT = consts.tile([64, S], BF16)
        kiT_hi = consts.tile([64, S], BF16)
        kiT_lo = consts.tile([64, S], BF16)

        # ---------------- P1: KV / indexer-key expansion ----------------
        with tc.tile_pool(name="p1w", bufs=1) as p1w, \
             tc.tile_pool(name="p1", bufs=3) as p1, \
             tc.tile_pool(name="p1ps", bufs=2, space="PSUM") as p1ps, \
             tc.tile_pool(name="p1tr", bufs=2, space="PSUM") as p1tr:
            cos_t = p1w.tile([128, NT, DR], F32)
            sin_t = p1w.tile([128, NT, DR], F32)
            cr = cos_v.rearrange("(t p) d -> p t d", p=128)
            sr = sin_v.rearrange("(t p) d -> p t d", p=128)
            wkva_sb = p1w.tile([128, NT, KVLR], BF16)
            wr = wkva_v.rearrange("(c p) n -> p c n", p=128)
            # wcat: [k_pe cols of wkv_a | iwk_hi | iwk_lo]
            wcat = p1w.tile([128, NT, DR + 2 * IHD], BF16)
            ikh = iwkh_v.rearrange("(c p) n -> p c n", p=128)
            ikl = iwkl_v.rearrange("(c p) n -> p c n", p=128)
            for c in range(NT):
                nc.gpsimd.dma_start(out=cos_t[:, c, :], in_=cr[:, c, :])
                nc.gpsimd.dma_start(out=sin_t[:, c, :], in_=sr[:, c, :])
                nc.gpsimd.dma_start(out=wkva_sb[:, c, :],
                                    in_=wr[:, c, 0:KVLR])
                nc.gpsimd.dma_start(out=wcat[:, c, 0:DR],
                                    in_=wr[:, c, KVLR:])
                nc.gpsimd.dma_start(out=wcat[:, c, DR:DR + IHD],
                                    in_=ikh[:, c, :])
                nc.gpsimd.dma_start(out=wcat[:, c, DR + IHD:],
                                    in_=ikl[:, c, :])

            xrh = xTh_v.rearrange("(c p) (u q) -> p c u q", p=128, q=128)
            xrl = xTl_v.rearrange("(c p) (u q) -> p c u q", p=128, q=128)
            for t in range(NT):
                xt_hi = p1.tile([128, NT, 128], BF16, tag="xt_hi")
                xt_lo = p1.tile([128, NT, 128], BF16, tag="xt_lo")
                for c in range(NT):
                    nc.gpsimd.dma_start(out=xt_hi[:, c, :], in_=xrh[:, c, t, :])
                    nc.gpsimd.dma_start(out=xt_lo[:, c, :], in_=xrl[:, c, t, :])
                ps_kv = p1ps.tile([128, KVLR], F32, tag="ps_kv")
                ps_x = p1ps.tile([128, DR + 2 * IHD], F32, tag="ps_x")
                ps_kl = p1ps.tile([128, IHD], F32, tag="ps_kl")
                for f in range(NT):
                    st, sp = (f == 0), (f == NT - 1)
                    nc.tensor.matmul(ps_kv, xt_hi[:, f, :],
                                     wkva_sb[:, f, :], start=st, stop=sp)
                    nc.tensor.matmul(ps_x, xt_hi[:, f, :],
                                     wcat[:, f, :], start=st, stop=sp)
                    nc.tensor.matmul(ps_kl, xt_lo[:, f, :],
                                     wcat[:, f, DR:DR + IHD],
                                     start=st, stop=sp)
                # ckv rmsnorm -> bf16 -> transpose into ckvT
                ckv_sb = p1.tile([128, KVLR], F32, tag="ckv_sb")
                _rmsnorm_from_psum(nc, p1, ckv_sb, [ps_kv], kvnw, KVLR)
                ckv_bf = p1.tile([128, KVLR], BF16, tag="ckv_bf")
                nc.vector.tensor_copy(ckv_bf, ckv_sb)
                for ch in range(4):
                    ptr = p1tr.tile([128, 128], BF16, tag="ptr")
                    nc.tensor.transpose(ptr, ckv_bf[:, ch * 128:(ch + 1) * 128],
                                        ident)
                    nc.scalar.copy(out=ckvT[:, ch, t * 128:(t + 1) * 128],
                                   in_=ptr)
                # k_pe rope -> bf16 -> transpose into kpeT
                pe_sb = p1.tile([128, DR], F32, tag="pe_sb")
                _rope_int(nc, pe_sb, ps_x[:, 0:DR],
                          cos_t[:, t, :], sin_t[:, t, :])
                pe_bf = p1.tile([128, DR], BF16, tag="pe_bf")
                nc.vector.tensor_copy(pe_bf, pe_sb)
                ptr = p1tr.tile([128, 128], BF16, tag="ptr")
                nc.tensor.transpose(ptr[:64, :], pe_bf, ident)
                nc.scalar.copy(out=kpeT[:, t * 128:(t + 1) * 128],
                               in_=ptr[:64, :])
                # ki = layernorm(3-pass sum) + rope -> split -> transpose
                ki32 = p1.tile([128, IHD], F32, tag="ki32")
                nc.scalar.copy(out=ki32, in_=ps_x[:, DR:DR + IHD])
                nc.vector.tensor_add(ki32, ki32, ps_x[:, DR + IHD:])
                nc.vector.tensor_add(ki32, ki32, ps_kl)
                s1 = p1.tile([128, 2], F32, tag="ki_s")
                scr = p1.tile([128, IHD], F32, tag="ki_scr")
                nc.scalar.activation(out=scr, in_=ki32,
                                     func=mybir.ActivationFunctionType.Copy,
                                     accum_out=s1[:, 0:1])
                nc.scalar.activation(out=scr, in_=ki32,
                                     func=mybir.ActivationFunctionType.Square,
                                     accum_out=s1[:, 1:2])
                mom = p1.tile([128, 4], F32, tag="ki_m")
                nc.vector.tensor_scalar(out=mom[:, 0:1], in0=s1[:, 0:1],
                                        scalar1=1.0 / IHD, scalar2=None,
                                        op0=mybir.AluOpType.mult)
                nc.vector.tensor_scalar(out=mom[:, 1:2], in0=s1[:, 1:2],
                                        scalar1=1.0 / IHD, scalar2=None,
                                        op0=mybir.AluOpType.mult)
                nc.vector.tensor_mul(mom[:, 2:3], mom[:, 0:1], mom[:, 0:1])
                nc.vector.tensor_sub(mom[:, 2:3], mom[:, 1:2], mom[:, 2:3])
                nc.vector.tensor_scalar(out=mom[:, 2:3], in0=mom[:, 2:3],
                                        scalar1=1e-5, scalar2=None,
                                        op0=mybir.AluOpType.add)
                nc.scalar.activation(out=mom[:, 2:3], in_=mom[:, 2:3],
                                     func=mybir.ActivationFunctionType.Sqrt)
                nc.vector.reciprocal(out=mom[:, 3:4], in_=mom[:, 2:3])
                ki_n = p1.tile([128, IHD], F32, tag="ki_n")
                nc.vector.tensor_scalar(out=ki_n, in0=ki32,
                                        scalar1=mom[:, 0:1],
                                        scalar2=mom[:, 3:4],
                                        op0=mybir.AluOpType.subtract,
                                        op1=mybir.AluOpType.mult)
                nc.vector.tensor_mul(ki_n, ki_n, knw)
                nc.vector.tensor_add(ki_n, ki_n, knb)
                ki_r = p1.tile([128, IHD], F32, tag="ki_r")
                _rope_ni(nc, ki_r, ki_n, cos_t[:, t, :], sin_t[:, t, :])
                ki_hi, ki_lo = _split(nc, p1, ki_r, IHD, "ki")
                ptr = p1tr.tile([128, 128], BF16, tag="ptr")
                nc.tensor.transpose(ptr[:64, :], ki_hi, ident)
                nc.scalar.copy(out=kiT_hi[:, t * 128:(t + 1) * 128],
                               in_=ptr[:64, :])
                ptr = p1tr.tile([128, 128], BF16, tag="ptr")
                nc.tensor.transpose(ptr[:64, :], ki_lo, ident)
                nc.scalar.copy(out=kiT_lo[:, t * 128:(t + 1) * 128],
                               in_=ptr[:64, :])

        if dbg:
            nc.gpsimd.dma_start(out=d_ckvT,
                                in_=ckvT.rearrange("p a b -> p (a b)"))
            nc.gpsimd.dma_start(out=d_kpeT, in_=kpeT)
            nc.gpsimd.dma_start(out=d_kiT[:, 0:S], in_=kiT_hi)
            nc.gpsimd.dma_start(out=d_kiT[:, S:], in_=kiT_lo)

        # ---------------- P2: query-block projections ----------------
        mid = tc.alloc_tile_pool(name="mid", bufs=1)
        qTn = mid.tile([128, H, NB], BF16)       # nope part, feature-major
        qTp = mid.tile([64, H, NB], BF16)        # rope part
        qiT_hi = mid.tile([64, IH, NB], BF16)
        qiT_lo = mid.tile([64, IH, NB], BF16)

        with tc.tile_pool(name="p2w", bufs=2) as p2w, \
             tc.tile_pool(name="p2", bufs=2) as p2, \
             tc.tile_pool(name="p2ps", bufs=1, space="PSUM") as p2ps, \
             tc.tile_pool(name="p2tr", bufs=1, space="PSUM") as p2tr:
            cosb = p2.tile([128, NQT, DR], F32, tag="cosb", bufs=1)
            sinb = p2.tile([128, NQT, DR], F32, tag="sinb", bufs=1)
            for q in range(NQT):
                nc.gpsimd.dma_start(
                    out=cosb[:, q, :],
                    in_=cos_v[bass.ds(r0 + q * 128, 128), :])
                nc.gpsimd.dma_start(
                    out=sinb[:, q, :],
                    in_=sin_v[bass.ds(r0 + q * 128, 128), :])
            qnw = p2.tile([128, QLR], F32, tag="qnw", bufs=1)
            nc.gpsimd.dma_start(out=qnw, in_=_vb(gf, F32_OFF["q_norm_w"], QLR))
            gcat_w = p2.tile([128, NT, 2 * IH], BF16, tag="gcat", bufs=1)
            igh_r = igh_v.rearrange("(c p) n -> p c n", p=128)
            igl_r = igl_v.rearrange("(c p) n -> p c n", p=128)
            for c in range(NT):
                nc.gpsimd.dma_start(out=gcat_w[:, c, 0:IH], in_=igh_r[:, c, :])
                nc.gpsimd.dma_start(out=gcat_w[:, c, IH:], in_=igl_r[:, c, :])
            ps_qr = [p2ps.tile([128, 512], F32, tag=f"ps_qr{q}{i}",
                               name=f"ps_qr{q}{i}")
                     for q in range(NQT) for i in range(2)]
            ps_g = [p2ps.tile([128, 2 * IH], F32, tag=f"ps_g{q}",
                              name=f"ps_g{q}") for q in range(NQT)]
            for f in range(NT):
                wqah_f = p2w.tile([128, QLR], BF16, tag="wqah_f")
                nc.gpsimd.dma_start(out=wqah_f,
                                    in_=wqah_v[f * 128:(f + 1) * 128, :])
                wqal_f = p2w.tile([128, QLR], BF16, tag="wqal_f")
                nc.gpsimd.dma_start(out=wqal_f,
                                    in_=wqal_v[f * 128:(f + 1) * 128, :])
                xq_hi = p2w.tile([128, NB], BF16, tag="xq_hi", bufs=3)
                nc.gpsimd.dma_start(
                    out=xq_hi,
                    in_=xTh_v[f * 128:(f + 1) * 128, bass.ds(r0, NB)])
                xq_lo = p2w.tile([128, NB], BF16, tag="xq_lo", bufs=3)
                nc.gpsimd.dma_start(
                    out=xq_lo,
                    in_=xTl_v[f * 128:(f + 1) * 128, bass.ds(r0, NB)])
                st, sp = (f == 0), (f == NT - 1)
                for q in range(NQT):
                    lhs_hi = xq_hi[:, q * 128:(q + 1) * 128]
                    lhs_lo = xq_lo[:, q * 128:(q + 1) * 128]
                    for i in range(2):
                        cols = slice(i * 512, (i + 1) * 512)
                        nc.tensor.matmul(ps_qr[2 * q + i], lhs_hi,
                                         wqah_f[:, cols], start=st, stop=False)
                        nc.tensor.matmul(ps_qr[2 * q + i], lhs_hi,
                                         wqal_f[:, cols], start=False,
                                         stop=False)
                        nc.tensor.matmul(ps_qr[2 * q + i], lhs_lo,
                                         wqah_f[:, cols], start=False, stop=sp)
                    nc.tensor.matmul(ps_g[q][:, 0:2 * IH], lhs_hi,
                                     gcat_w[:, f, :], start=st, stop=False)
                    nc.tensor.matmul(ps_g[q][:, 0:IH], lhs_lo,
                                     gcat_w[:, f, 0:IH], start=False, stop=sp)
            qrT_hi = p2.tile([128, 8, NB], BF16, tag="qrT_hi", bufs=1)
            qrT_lo = p2.tile([128, 8, NB], BF16, tag="qrT_lo", bufs=1)
            gate_sb = p2.tile([128, NQT, IH], F32, tag="gate_sb", bufs=1)
            for q in range(NQT):
                qr_sb = p2.tile([128, QLR], F32, tag="qr_sb")
                _rmsnorm_from_psum(nc, p2, qr_sb,
                                   [ps_qr[2 * q], ps_qr[2 * q + 1]], qnw, QLR)
                nc.scalar.copy(out=gate_sb[:, q, :], in_=ps_g[q][:, 0:IH])
                nc.vector.tensor_add(gate_sb[:, q, :], gate_sb[:, q, :],
                                     ps_g[q][:, IH:2 * IH])
                nc.vector.tensor_scalar(out=gate_sb[:, q, :],
                                        in0=gate_sb[:, q, :],
                                        scalar1=SCALE_GATE * SCALE_IDX,
                                        scalar2=None,
                                        op0=mybir.AluOpType.mult)
                qr_hi, qr_lo = _split(nc, p2, qr_sb, QLR, "qr")
                for ch in range(8):
                    cols = slice(ch * 128, (ch + 1) * 128)
                    ptr = p2tr.tile([128, 128], BF16, tag="ptr2")
                    nc.tensor.transpose(ptr, qr_hi[:, cols], ident)
                    nc.scalar.copy(out=qrT_hi[:, ch, q * 128:(q + 1) * 128],
                                   in_=ptr)
                    ptr = p2tr.tile([128, 128], BF16, tag="ptr2")
                    nc.tensor.transpose(ptr, qr_lo[:, cols], ident)
                    nc.scalar.copy(out=qrT_lo[:, ch, q * 128:(q + 1) * 128],
                                   in_=ptr)
            # q projection per MLA head (bf16)
            wqbr = wqb_v.rearrange("(c p) n -> p c n", p=128)
            for h in range(H):
                wqb_h = p2w.tile([128, 8, DN + DR], BF16, tag="wqb_h")
                for c in range(8):
                    nc.gpsimd.dma_start(
                        out=wqb_h[:, c, :],
                        in_=wqbr[:, c, h * (DN + DR):(h + 1) * (DN + DR)])
                for q in range(NQT):
                    ps_q = p2ps.tile([128, DN + DR], F32, tag="ps_q")
                    for ch in range(8):
                        nc.tensor.matmul(
                            ps_q, qrT_hi[:, ch, q * 128:(q + 1) * 128],
                            wqb_h[:, ch, :],
                            start=(ch == 0), stop=(ch == 7))
                    qn_bf = p2.tile([128, DN], BF16, tag="qn_bf")
                    nc.vector.tensor_scalar(out=qn_bf, in0=ps_q[:, 0:DN],
                                            scalar1=SCALE_MLA, scalar2=None,
                                            op0=mybir.AluOpType.mult)
                    qp32 = p2.tile([128, DR], F32, tag="qp32")
                    _rope_int(nc, qp32, ps_q[:, DN:],
                              cosb[:, q, :], sinb[:, q, :])
                    qp_bf = p2.tile([128, DR], BF16, tag="qp_bf")
                    nc.vector.tensor_scalar(out=qp_bf, in0=qp32,
                                            scalar1=SCALE_MLA, scalar2=None,
                                            op0=mybir.AluOpType.mult)
                    ptr = p2tr.tile([128, 128], BF16, tag="ptr2")
                    nc.tensor.transpose(ptr, qn_bf, ident)
                    nc.scalar.copy(out=qTn[:, h, q * 128:(q + 1) * 128],
                                   in_=ptr)
                    ptr = p2tr.tile([128, 128], BF16, tag="ptr2")
                    nc.tensor.transpose(ptr[:64, :], qp_bf, ident)
                    nc.scalar.copy(out=qTp[:, h, q * 128:(q + 1) * 128],
                                   in_=ptr[:64, :])
            # indexer q heads: 3-pass hi/lo, rope, * gate, split, transpose
            iwqbh_r = iwqbh_v.rearrange("(c p) n -> p c n", p=128)
            iwqbl_r = iwqbl_v.rearrange("(c p) n -> p c n", p=128)
            for ih in range(IH):
                wiq_cat = p2w.tile([128, 8, 2 * IHD], BF16, tag="wiq_cat")
                for c in range(8):
                    nc.gpsimd.dma_start(
                        out=wiq_cat[:, c, 0:IHD],
                        in_=iwqbh_r[:, c, ih * IHD:(ih + 1) * IHD])
                    nc.gpsimd.dma_start(
                        out=wiq_cat[:, c, IHD:],
                        in_=iwqbl_r[:, c, ih * IHD:(ih + 1) * IHD])
                for q in range(NQT):
                    ps_qc = p2ps.tile([128, 2 * IHD], F32, tag="ps_q")
                    for ch in range(8):
                        nc.tensor.matmul(
                            ps_qc[:, 0:2 * IHD],
                            qrT_hi[:, ch, q * 128:(q + 1) * 128],
                            wiq_cat[:, ch, :],
                            start=(ch == 0), stop=False)
                        nc.tensor.matmul(
                            ps_qc[:, 0:IHD],
                            qrT_lo[:, ch, q * 128:(q + 1) * 128],
                            wiq_cat[:, ch, 0:IHD],
                            start=False, stop=(ch == 7))
                    qi32 = p2.tile([128, IHD], F32, tag="qi32")
                    nc.scalar.copy(out=qi32, in_=ps_qc[:, 0:IHD])
                    nc.vector.tensor_add(qi32, qi32, ps_qc[:, IHD:2 * IHD])
                    qi_r = p2.tile([128, IHD], F32, tag="qi_r")
                    _rope_ni(nc, qi_r, qi32, cosb[:, q, :], sinb[:, q, :])
                    nc.vector.tensor_scalar(out=qi_r, in0=qi_r,
                                            scalar1=gate_sb[:, q, ih:ih + 1],
                                            scalar2=None,
                                            op0=mybir.AluOpType.mult)
                    qi_hi, qi_lo = _split(nc, p2, qi_r, IHD, "qi")
                    ptr = p2tr.tile([128, 128], BF16, tag="ptr2")
                    nc.tensor.transpose(ptr[:64, :], qi_hi, ident)
                    nc.scalar.copy(out=qiT_hi[:, ih, q * 128:(q + 1) * 128],
                                   in_=ptr[:64, :])
                    ptr = p2tr.tile([128, 128], BF16, tag="ptr2")
                    nc.tensor.transpose(ptr[:64, :], qi_lo, ident)
                    nc.scalar.copy(out=qiT_lo[:, ih, q * 128:(q + 1) * 128],
                                   in_=ptr[:64, :])
            if dbg:
                nc.gpsimd.dma_start(out=d_qrT[:, 0:8 * NB],
                                    in_=qrT_hi.rearrange("p a b -> p (a b)"))
                nc.gpsimd.dma_start(out=d_qrT[:, 8 * NB:],
                                    in_=qrT_lo.rearrange("p a b -> p (a b)"))
                nc.gpsimd.dma_start(out=d_gate,
                                    in_=gate_sb.rearrange("p a b -> p (a b)"))
                nc.gpsimd.dma_start(out=d_qiT[:, 0:IH * NB],
                                    in_=qiT_hi.rearrange("p a b -> p (a b)"))
                nc.gpsimd.dma_start(out=d_qiT[:, IH * NB:],
                                    in_=qiT_lo.rearrange("p a b -> p (a b)"))

        # ---------------- P3: index scores + EXACT top-k ----------------
        maskNEG = mid.tile([128, NQT, S], F32)
        with tc.tile_pool(name="p3", bufs=1) as p3, \
             tc.tile_pool(name="p3ps", bufs=4, space="PSUM") as p3ps:
            amask_v = None
            if not causal:
                amask_v = _v(gf, F32_OFF["amask"], S, S)
            for q in range(NQT):
                cm = p3.tile([128, S], F32, tag="cm")
                if causal:
                    # cmask = (col > row) * NEG
                    rq = p3.tile([128, 1], F32, tag="rq")
                    nc.vector.tensor_scalar(out=rq, in0=rowid,
                                            scalar1=float(q * 128),
                                            scalar2=None,
                                            op0=mybir.AluOpType.add)
                    nc.vector.tensor_scalar(out=cm, in0=iota_sb,
                                            scalar1=rq, scalar2=NEG,
                                            op0=mybir.AluOpType.is_gt,
                                            op1=mybir.AluOpType.mult)
                else:
                    nc.gpsimd.dma_start(
                        out=cm, in_=amask_v[bass.ds(r0 + q * 128, 128), :])
                isc = p3.tile([128, S], F32, tag="isc")
                for kc in range(4):
                    cols = slice(kc * 512, (kc + 1) * 512)
                    ps = p3ps.tile([128, 512], F32, tag="ps_isc")
                    for ih in range(IH):
                        qcols = slice(q * 128, (q + 1) * 128)
                        nc.tensor.matmul(ps, qiT_hi[:, ih, qcols],
                                         kiT_hi[:, cols],
                                         start=(ih == 0), stop=False)
                        nc.tensor.matmul(ps, qiT_hi[:, ih, qcols],
                                         kiT_lo[:, cols],
                                         start=False, stop=False)
                        nc.tensor.matmul(ps, qiT_lo[:, ih, qcols],
                                         kiT_hi[:, cols],
                                         start=False, stop=(ih == IH - 1))
                    nc.vector.tensor_add(isc[:, cols], ps, cm[:, cols])
                # clamp; masked cols sit at -200 (amask re-kills them later)
                nc.vector.tensor_scalar(out=isc, in0=isc, scalar1=-200.0,
                                        scalar2=None, op0=mybir.AluOpType.max)
                # exact top-256 threshold: 32 rounds of top-8 + replace
                scr = p3.tile([128, S], F32, tag="sel_scr")
                nc.vector.tensor_copy(scr, isc)
                mx = p3.tile([128, 8], F32, tag="mx")
                for r in range(TOPK // 8):
                    nc.vector.max(out=mx, in_=scr)
                    if r < TOPK // 8 - 1:
                        nc.vector.match_replace(out=scr, in_to_replace=mx,
                                                in_values=scr, imm_value=-3e9)
                nc.vector.tensor_scalar(out=maskNEG[:, q, :], in0=isc,
                                        scalar1=mx[:, 7:8], scalar2=NEG,
                                        op0=mybir.AluOpType.is_lt,
                                        op1=mybir.AluOpType.mult)
                nc.vector.tensor_add(maskNEG[:, q, :], maskNEG[:, q, :], cm)
                if dbg:
                    nc.gpsimd.dma_start(out=d_thr[:, q:q + 1], in_=mx[:, 7:8])

        if dbg:
            nc.gpsimd.dma_start(out=d_mask,
                                in_=maskNEG.rearrange("p a b -> p (a b)"))

        # ---------------- P4: sparse MLA attention per head ----------------
        out_hT = mid.tile([128, H, NB], BF16)
        with tc.tile_pool(name="p4w", bufs=2) as p4w, \
             tc.tile_pool(name="p4k", bufs=2) as p4k, \
             tc.tile_pool(name="p4p", bufs=2) as p4p, \
             tc.tile_pool(name="p4ps", bufs=2, space="PSUM") as p4ps, \
             tc.tile_pool(name="p4po", bufs=2, space="PSUM") as p4po:
            wbr = wkvb_v.rearrange("(c p) n -> p c n", p=128)
            for h in range(H):
                wb_k = p4w.tile([128, 4, DN], BF16, tag="wb_k")
                wb_v = p4w.tile([128, 4, DV], BF16, tag="wb_v")
                for c in range(4):
                    nc.gpsimd.dma_start(
                        out=wb_k[:, c, :],
                        in_=wbr[:, c, h * (DN + DV):h * (DN + DV) + DN])
                    nc.gpsimd.dma_start(
                        out=wb_v[:, c, :],
                        in_=wbr[:, c, h * (DN + DV) + DN:(h + 1) * (DN + DV)])
                knT = p4k.tile([128, S], BF16, tag="knT")
                for kc in range(4):
                    ps = p4ps.tile([128, 512], F32, tag="ps_kn")
                    for c in range(4):
                        nc.tensor.matmul(
                            ps, wb_k[:, c, :],
                            ckvT[:, c, kc * 512:(kc + 1) * 512],
                            start=(c == 0), stop=(c == 3))
                    nc.scalar.copy(out=knT[:, kc * 512:(kc + 1) * 512], in_=ps)
                v_sb = p4k.tile([128, NT, DV], BF16, tag="v_sb")
                for kt in range(NT):
                    ps = p4ps.tile([128, DV], F32, tag="ps_v")
                    for c in range(4):
                        nc.tensor.matmul(
                            ps,
                            ckvT[:, c, kt * 128:(kt + 1) * 128],
                            wb_v[:, c, :],
                            start=(c == 0), stop=(c == 3))
                    nc.scalar.copy(out=v_sb[:, kt, :], in_=ps)
                ps_o = p4po.tile([128, NB], F32, tag="ps_o")
                for q in range(NQT):
                    probs = p4p.tile([128, S], F32, tag="probs", bufs=1)
                    for kc in range(4):
                        cols = slice(kc * 512, (kc + 1) * 512)
                        ps = p4ps.tile([128, 512], F32, tag="ps_s")
                        nc.tensor.matmul(
                            ps, qTn[:, h, q * 128:(q + 1) * 128],
                            knT[:, cols], start=True, stop=False)
                        nc.tensor.matmul(
                            ps, qTp[:, h, q * 128:(q + 1) * 128],
                            kpeT[:, cols], start=False, stop=True)
                        nc.vector.tensor_add(probs[:, cols], ps,
                                             maskNEG[:, q, cols])
                    den = p4p.tile([128, 2], F32, tag="den")
                    nc.scalar.activation(out=probs, in_=probs,
                                         func=mybir.ActivationFunctionType.Exp,
                                         accum_out=den[:, 0:1])
                    nc.vector.reciprocal(out=den[:, 1:2], in_=den[:, 0:1])
                    pb = p4p.tile([128, S], BF16, tag="pb")
                    nc.vector.tensor_scalar(out=pb, in0=probs,
                                            scalar1=den[:, 1:2], scalar2=None,
                                            op0=mybir.AluOpType.mult)
                    pT = p4p.tile([128, NT, 128], BF16, tag="pT", bufs=1)
                    for kt in range(NT):
                        nc.scalar.dma_start_transpose(
                            out=pT[:, kt, :],
                            in_=pb[:, kt * 128:(kt + 1) * 128])
                    for kt in range(NT):
                        nc.tensor.matmul(
                            ps_o[:, q * 128:(q + 1) * 128],
                            v_sb[:, kt, :], pT[:, kt, :],
                            start=(kt == 0), stop=(kt == NT - 1))
                nc.scalar.copy(out=out_hT[:, h, :], in_=ps_o)

        if dbg:
            nc.gpsimd.dma_start(out=d_ohT,
                                in_=out_hT.rearrange("p a b -> p (a b)"))

        # ---------------- P5: output projection ----------------
        with tc.tile_pool(name="p5w", bufs=3) as p5w, \
             tc.tile_pool(name="p5", bufs=3) as p5, \
             tc.tile_pool(name="p5ps", bufs=4, space="PSUM") as p5ps:
            wor = wo_v.rearrange("(hh p) n -> p hh n", p=128)
            for g in range(NT):
                wo_g = p5w.tile([128, H, 128], BF16, tag="wo_g")
                for c in range(H):
                    nc.gpsimd.dma_start(
                        out=wo_g[:, c, :],
                        in_=wor[:, c, g * 128:(g + 1) * 128])
                ps = p5ps.tile([128, NB], F32, tag="ps_w")
                for h in range(H):
                    nc.tensor.matmul(ps, wo_g[:, h, :],
                                     out_hT[:, h, :],
                                     start=(h == 0), stop=(h == H - 1))
                ot = p5.tile([128, NB], FP16, tag="ot")
                nc.scalar.copy(out=ot, in_=ps)
                nc.gpsimd.dma_start(out=outT[g * 128:(g + 1) * 128, :], in_=ot)

        mid.release()
        consts.release()
        dram.release()
    nc.compile()
    return nc


_NC_CACHE = {}


def _get_nc(causal=True):
    if causal not in _NC_CACHE:
        _NC_CACHE[causal] = build_nc(causal)
    return _NC_CACHE[causal]


def _split_np(a):
    hi = a.astype(ml_dtypes.bfloat16)
    lo = (a - hi.astype(np.float32)).astype(ml_dtypes.bfloat16)
    return hi, lo


def _is_causal(am):
    s = am.shape[-1]
    r = np.arange(s, dtype=np.int64)
    causal = np.where(r[:, None] >= r[None, :], np.float32(0.0),
                      np.float32(NEG))
    return np.array_equal(am.reshape(s, s), causal)


def make_core_inputs(x, cos, sin, attn_mask, wq_a, q_norm_w, wq_b, wkv_a,
                     kv_norm_w, wkv_b, wo, idx_wq_b, idx_wk, idx_knorm_w,
                     idx_knorm_b, idx_gate):
    causal = _is_causal(np.asarray(attn_mask, np.float32))
    F32_OFF, F32_TOT = _f32_layout(causal)

    blob_bf = np.zeros(BF_TOT, ml_dtypes.bfloat16)

    def put_bf(name, arr):
        o = BF_OFF[name]
        blob_bf[o:o + arr.size] = np.ascontiguousarray(arr).reshape(-1)

    xT = np.ascontiguousarray(x[0].astype(np.float32).T)
    xh, xl = _split_np(xT)
    put_bf("xT_hi", xh)
    put_bf("xT_lo", xl)
    wh, wl = _split_np(np.asarray(wq_a, np.float32))
    put_bf("wqa_hi", wh)
    put_bf("wqa_lo", wl)
    put_bf("wq_b", np.asarray(wq_b, np.float32).astype(ml_dtypes.bfloat16))
    put_bf("wkv_a", np.asarray(wkv_a, np.float32).astype(ml_dtypes.bfloat16))
    put_bf("wkv_b", np.asarray(wkv_b, np.float32).astype(ml_dtypes.bfloat16))
    put_bf("wo", np.asarray(wo, np.float32).astype(ml_dtypes.bfloat16))
    ih_, il_ = _split_np(np.asarray(idx_wq_b, np.float32))
    put_bf("iwqb_hi", ih_)
    put_bf("iwqb_lo", il_)
    kh, kl = _split_np(np.asarray(idx_wk, np.float32))
    put_bf("iwk_hi", kh)
    put_bf("iwk_lo", kl)
    gh, gl = _split_np(np.asarray(idx_gate, np.float32))
    put_bf("igate_hi", gh)
    put_bf("igate_lo", gl)
    put_bf("ident", np.eye(128, dtype=np.float32))

    blob_f32 = np.zeros(F32_TOT, np.float32)

    def put_f(name, arr):
        o = F32_OFF[name]
        blob_f32[o:o + arr.size] = np.ascontiguousarray(
            arr, np.float32).reshape(-1)

    put_f("cos", cos[0])
    put_f("sin", sin[0])
    put_f("q_norm_w", q_norm_w)
    put_f("kv_norm_w", kv_norm_w)
    put_f("idx_knorm_w", idx_knorm_w)
    put_f("idx_knorm_b", idx_knorm_b)
    put_f("iota", np.arange(S, dtype=np.float32))
    rows = (np.arange(NCORES)[:, None] * NB
            + np.arange(128)[None, :]).astype(np.float32)
    put_f("rows", rows)
    if not causal:
        put_f("amask", attn_mask[0, 0])

    lb, lf = BF_TOT // NCORES, F32_TOT // NCORES
    maps = []
    for c in range(NCORES):
        maps.append({
            "shard_bf": np.ascontiguousarray(blob_bf[c * lb:(c + 1) * lb]),
            "shard_f32": np.ascontiguousarray(blob_f32[c * lf:(c + 1) * lf]),
        })
    return maps, causal


def kernel(x, cos, sin, attn_mask, wq_a, q_norm_w, wq_b, wkv_a, kv_norm_w,
           wkv_b, wo, idx_wq_b, idx_wk, idx_knorm_w, idx_knorm_b, idx_gate):
    from concourse.bass_utils import run_bass_kernel_spmd
    maps, causal = make_core_inputs(
        x, cos, sin, attn_mask, wq_a, q_norm_w, wq_b, wkv_a, kv_norm_w,
        wkv_b, wo, idx_wq_b, idx_wk, idx_knorm_w, idx_knorm_b, idx_gate)
    nc = _get_nc(causal)
    res = run_bass_kernel_spmd(nc, maps, list(range(NCORES)))
    outs = [np.asarray(r["outT"]).astype(np.float32).T for r in res.results]
    out = np.concatenate(outs, axis=0)[None]                   # [1, S, HID]
    return out.astype(np.float32)


# revision 10
# speedup vs baseline: 9.5471x; 1.4521x over previous
"""DSA sparse MLA attention kernel for TRN2, 8 NeuronCores.

Transfer-optimized SPMD design. The host->device tunnel moves ~52 MB/s,
so every input byte is shipped exactly ONCE: each core receives a 1/8
slice of two packed blobs (bf16 + f32) and the cores AllGather them
on-device (HBM-to-HBM over on-chip links, ~GB/ms). Per-core query-block
slices are carved out of the gathered blobs at runtime with
partition_id()-based dynamic DMA offsets, so no per-core host tensors
are needed at all.

Precision plan (harness gate: rel_err < 2e-2; this lands ~6e-3):
 - Indexer path (x, wq_a, idx_*) is selection-critical: tensors are
   shipped as hi/lo bf16 pairs (same bytes as f32) and matmuls use a
   3-pass hi/lo bf16 emulation (~1e-5 rel, 4x faster than fp32r which
   is only ~1e-3 accurate).
 - Top-256 selection is EXACT: 32 rounds of vector.max + match_replace
   give the true 256th-largest index score per row.
 - Attention path (wq_b, wkv_a, wkv_b, wo, q/k/v, probs) is plain bf16.
 - Output is fp16 (halves the donated-zeros upload + fetch).

Sharding: sequence-parallel over query rows; core c owns rows
[256c, 256(c+1)). KV/indexer-key expansion over all 2048 keys is
replicated (compute is ~free vs transfer).
"""

import numpy as np
import ml_dtypes

import jax

# Persistent XLA compilation cache: run_bass_kernel_spmd re-jits (and
# would re-run the walrus NEFF compile, ~1s) on every call; the disk
# cache turns that into a deserialize+load.
jax.config.update("jax_compilation_cache_dir", "/tmp/jax_cc_cache")
jax.config.update("jax_persistent_cache_min_compile_time_secs", 0.0)
jax.config.update("jax_persistent_cache_min_entry_size_bytes", 0)

import concourse.bass as bass
import concourse.bacc as bacc
import concourse.mybir as mybir
from concourse.tile import TileContext

F32 = mybir.dt.float32
BF16 = mybir.dt.bfloat16
FP16 = mybir.dt.float16

S, HID = 2048, 2048
H, DN, DR, DV = 16, 128, 64, 128
QLR, KVLR = 1024, 512
IH, IHD, TOPK = 8, 64, 256
NEG = -1e9
NB = 256            # query rows per core
NCORES = 8
NT = S // 128       # 16 token tiles
NQT = NB // 128     # 2 query tiles per core
SCALE_MLA = float((DN + DR) ** -0.5)
SCALE_IDX = float(IHD ** -0.5)
SCALE_GATE = float(IH ** -0.5)
ALIGN = 512         # element alignment for blob entries

BF_LAYOUT = [
    ("xT_hi", (HID, S)), ("xT_lo", (HID, S)),
    ("wqa_hi", (HID, QLR)), ("wqa_lo", (HID, QLR)),
    ("wq_b", (QLR, H * (DN + DR))),
    ("wkv_a", (HID, KVLR + DR)),
    ("wkv_b", (KVLR, H * (DN + DV))),
    ("wo", (H * DV, HID)),
    ("iwqb_hi", (QLR, IH * IHD)), ("iwqb_lo", (QLR, IH * IHD)),
    ("iwk_hi", (HID, IHD)), ("iwk_lo", (HID, IHD)),
    ("igate_hi", (HID, IH)), ("igate_lo", (HID, IH)),
    ("ident", (128, 128)),
]

F32_LAYOUT_BASE = [
    ("cos", (S, DR)), ("sin", (S, DR)),
    ("q_norm_w", (1, QLR)), ("kv_norm_w", (1, KVLR)),
    ("idx_knorm_w", (1, IHD)), ("idx_knorm_b", (1, IHD)),
    ("iota", (1, S)),
    ("rows", (NCORES * 128, 1)),
]


def _mk_layout(entries):
    offs, off = {}, 0
    for name, shape in entries:
        offs[name] = off
        n = int(np.prod(shape))
        off += (n + ALIGN - 1) // ALIGN * ALIGN
    tot = (off + NCORES * ALIGN - 1) // (NCORES * ALIGN) * (NCORES * ALIGN)
    return offs, tot


def _f32_layout(causal):
    ents = list(F32_LAYOUT_BASE)
    if not causal:
        ents.append(("amask", (S, S)))
    return _mk_layout(ents)


BF_OFF, BF_TOT = _mk_layout(BF_LAYOUT)
LB = BF_TOT // NCORES


def _v(blob1d, off, r, c):
    """[r, c] row-major view at element offset off of a 1-D DRAM AP."""
    return blob1d[off:off + r * c].rearrange("(r c) -> r c", c=c)


def _vb(blob1d, off, n, parts=128):
    """Partition-broadcast view [parts, n] of n elements at offset off."""
    return bass.AP(tensor=blob1d.tensor, offset=blob1d.offset + off,
                   ap=[[0, parts], [1, n]])


def _rmsnorm_from_psum(nc, pool, out_sb, psums, wb, d, eps=1e-6):
    """out_sb[p, d] = psum * rsqrt(mean(psum^2)+eps) * w."""
    ssq = pool.tile([128, len(psums)], F32)
    for i, ps in enumerate(psums):
        w = ps.shape[-1]
        scr = pool.tile([128, 512], F32, tag="rms_scr")
        nc.scalar.activation(out=scr[:, :w], in_=ps,
                             func=mybir.ActivationFunctionType.Square,
                             accum_out=ssq[:, i:i + 1])
    tot = pool.tile([128, 1], F32)
    if len(psums) == 1:
        nc.vector.tensor_scalar(out=tot, in0=ssq, scalar1=1.0 / d,
                                scalar2=eps, op0=mybir.AluOpType.mult,
                                op1=mybir.AluOpType.add)
    else:
        nc.vector.tensor_reduce(out=tot, in_=ssq, axis=mybir.AxisListType.X,
                                op=mybir.AluOpType.add)
        nc.vector.tensor_scalar(out=tot, in0=tot, scalar1=1.0 / d,
                                scalar2=eps, op0=mybir.AluOpType.mult,
                                op1=mybir.AluOpType.add)
    nc.scalar.activation(out=tot, in_=tot,
                         func=mybir.ActivationFunctionType.Sqrt)
    rinv = pool.tile([128, 1], F32)
    nc.vector.reciprocal(out=rinv, in_=tot)
    off = 0
    for ps in psums:
        w = ps.shape[-1]
        nc.vector.tensor_scalar(out=out_sb[:, off:off + w], in0=ps,
                                scalar1=rinv, scalar2=None,
                                op0=mybir.AluOpType.mult)
        off += w
    nc.vector.tensor_mul(out_sb[:, :d], out_sb[:, :d], wb[:, :d])


def _rope_int(nc, out, in_, cos, sin):
    """Interleaved (GPT-J) rope, token-major [128, 64] -> out[128, 64]."""
    xp = in_.rearrange("p (a b) -> p a b", b=2)
    op = out.rearrange("p (a b) -> p a b", b=2)
    c, s = cos[:, 0:32], sin[:, 0:32]
    x1, x2 = xp[:, :, 0], xp[:, :, 1]
    nc.vector.tensor_mul(op[:, :, 0], x1, c)
    nc.vector.tensor_mul(op[:, :, 1], x2, c)
    t = nc._rope_scr.tile([128, 32], F32, tag="rope_t")
    nc.vector.tensor_mul(t, x2, s)
    nc.vector.tensor_sub(op[:, :, 0], op[:, :, 0], t)
    nc.vector.tensor_mul(t, x1, s)
    nc.vector.tensor_add(op[:, :, 1], op[:, :, 1], t)


def _rope_ni(nc, out, in_, cos, sin):
    """Non-interleaved (rotate_half) rope, [128, 64]."""
    x1, x2 = in_[:, 0:32], in_[:, 32:64]
    c1, c2 = cos[:, 0:32], cos[:, 32:64]
    s1, s2 = sin[:, 0:32], sin[:, 32:64]
    nc.vector.tensor_mul(out[:, 0:32], x1, c1)
    nc.vector.tensor_mul(out[:, 32:64], x2, c2)
    t = nc._rope_scr.tile([128, 32], F32, tag="rope_t")
    nc.vector.tensor_mul(t, x2, s1)
    nc.vector.tensor_sub(out[:, 0:32], out[:, 0:32], t)
    nc.vector.tensor_mul(t, x1, s2)
    nc.vector.tensor_add(out[:, 32:64], out[:, 32:64], t)


def _split(nc, pool, src_f32, n, tag):
    """f32 [128, n] -> (hi bf16, lo bf16) with hi+lo ~= src."""
    hi = pool.tile([128, n], BF16, tag=tag + "_hi")
    nc.vector.tensor_copy(hi, src_f32)
    hi32 = pool.tile([128, n], F32, tag=tag + "_h32")
    nc.vector.tensor_copy(hi32, hi)
    lo32 = pool.tile([128, n], F32, tag=tag + "_l32")
    nc.vector.tensor_sub(lo32, src_f32, hi32)
    lo = pool.tile([128, n], BF16, tag=tag + "_lo")
    nc.vector.tensor_copy(lo, lo32)
    return hi, lo


def build_nc(causal=True, dbg=False):
    F32_OFF, F32_TOT = _f32_layout(causal)
    lf = F32_TOT // NCORES

    nc = bacc.Bacc("TRN2", target_bir_lowering=False, debug=False)
    shard_bf = nc.dram_tensor("shard_bf", [LB], BF16, kind="ExternalInput").ap()
    shard_f32 = nc.dram_tensor("shard_f32", [lf], F32,
                               kind="ExternalInput").ap()
    outT = nc.dram_tensor("outT", [HID, NB], FP16, kind="ExternalOutput").ap()
    if dbg:
        d_ckvT = nc.dram_tensor("d_ckvT", [128, 4 * S], BF16,
                                kind="ExternalOutput").ap()
        d_kpeT = nc.dram_tensor("d_kpeT", [64, S], BF16,
                                kind="ExternalOutput").ap()
        d_kiT = nc.dram_tensor("d_kiT", [64, 2 * S], BF16,
                               kind="ExternalOutput").ap()
        d_qrT = nc.dram_tensor("d_qrT", [128, 2 * 8 * NB], BF16,
                               kind="ExternalOutput").ap()
        d_gate = nc.dram_tensor("d_gate", [128, NQT * IH], F32,
                                kind="ExternalOutput").ap()
        d_mask = nc.dram_tensor("d_mask", [128, NQT * S], F32,
                                kind="ExternalOutput").ap()
        d_thr = nc.dram_tensor("d_thr", [128, NQT], F32,
                               kind="ExternalOutput").ap()
        d_qiT = nc.dram_tensor("d_qiT", [64, 2 * IH * NB], BF16,
                               kind="ExternalOutput").ap()
        d_ohT = nc.dram_tensor("d_ohT", [128, H * NB], BF16,
                               kind="ExternalOutput").ap()

    with TileContext(nc) as tc:
        pid = nc.partition_id()
        r0 = pid * NB

        dram = tc.alloc_tile_pool(name="dram", bufs=1, space="DRAM")
        bounce_bf = dram.tile([LB], BF16)
        bounce_f32 = dram.tile([lf], F32)
        gath_bf = dram.tile([NCORES, LB], BF16)
        gath_f32 = dram.tile([NCORES, lf], F32)
        nc.gpsimd.dma_start(out=bounce_bf, in_=shard_bf)
        nc.gpsimd.dma_start(out=bounce_f32, in_=shard_f32)
        nc.gpsimd.collective_compute(
            "AllGather", mybir.AluOpType.bypass,
            replica_groups=[list(range(NCORES))],
            ins=[bounce_bf[:].opt()], outs=[gath_bf[:].opt()])
        nc.gpsimd.collective_compute(
            "AllGather", mybir.AluOpType.bypass,
            replica_groups=[list(range(NCORES))],
            ins=[bounce_f32[:].opt()], outs=[gath_f32[:].opt()])
        gb = gath_bf.rearrange("a b -> (a b)")
        gf = gath_f32.rearrange("a b -> (a b)")

        xTh_v = _v(gb, BF_OFF["xT_hi"], HID, S)
        xTl_v = _v(gb, BF_OFF["xT_lo"], HID, S)
        wqah_v = _v(gb, BF_OFF["wqa_hi"], HID, QLR)
        wqal_v = _v(gb, BF_OFF["wqa_lo"], HID, QLR)
        wqb_v = _v(gb, BF_OFF["wq_b"], QLR, H * (DN + DR))
        wkva_v = _v(gb, BF_OFF["wkv_a"], HID, KVLR + DR)
        wkvb_v = _v(gb, BF_OFF["wkv_b"], KVLR, H * (DN + DV))
        wo_v = _v(gb, BF_OFF["wo"], H * DV, HID)
        iwqbh_v = _v(gb, BF_OFF["iwqb_hi"], QLR, IH * IHD)
        iwqbl_v = _v(gb, BF_OFF["iwqb_lo"], QLR, IH * IHD)
        iwkh_v = _v(gb, BF_OFF["iwk_hi"], HID, IHD)
        iwkl_v = _v(gb, BF_OFF["iwk_lo"], HID, IHD)
        igh_v = _v(gb, BF_OFF["igate_hi"], HID, IH)
        igl_v = _v(gb, BF_OFF["igate_lo"], HID, IH)
        ident_v = _v(gb, BF_OFF["ident"], 128, 128)
        cos_v = _v(gf, F32_OFF["cos"], S, DR)
        sin_v = _v(gf, F32_OFF["sin"], S, DR)
        rows_v = _v(gf, F32_OFF["rows"], NCORES * 128, 1)

        consts = tc.alloc_tile_pool(name="consts", bufs=1)
        nc._rope_scr = consts

        ident = consts.tile([128, 128], BF16)
        nc.gpsimd.dma_start(out=ident, in_=ident_v)
        kvnw = consts.tile([128, KVLR], F32)
        nc.gpsimd.dma_start(out=kvnw, in_=_vb(gf, F32_OFF["kv_norm_w"], KVLR))
        knw = consts.tile([128, IHD], F32)
        nc.gpsimd.dma_start(out=knw, in_=_vb(gf, F32_OFF["idx_knorm_w"], IHD))
        knb = consts.tile([128, IHD], F32)
        nc.gpsimd.dma_start(out=knb, in_=_vb(gf, F32_OFF["idx_knorm_b"], IHD))
        iota_sb = consts.tile([128, S], F32)
        nc.gpsimd.dma_start(out=iota_sb, in_=_vb(gf, F32_OFF["iota"], S))
        rowid = consts.tile([128, 1], F32)
        nc.gpsimd.dma_start(out=rowid, in_=rows_v[bass.ds(pid * 128, 128), :])

        ckvT = consts.tile([128, 4, S], BF16)      # [ckv_chunk, 4, tok]
        kpeT = consts.tile([64, S], BF16)
        kiT_hi = consts.tile([64, S], BF16)
        kiT_lo = consts.tile([64, S], BF16)

        # ---------------- P1: KV / indexer-key expansion ----------------
        with tc.tile_pool(name="p1w", bufs=1) as p1w, \
             tc.tile_pool(name="p1", bufs=3) as p1, \
             tc.tile_pool(name="p1ps", bufs=2, space="PSUM") as p1ps, \
             tc.tile_pool(name="p1tr", bufs=2, space="PSUM") as p1tr:
            cos_t = p1w.tile([128, NT, DR], F32)
            sin_t = p1w.tile([128, NT, DR], F32)
            cr = cos_v.rearrange("(t p) d -> p t d", p=128)
            sr = sin_v.rearrange("(t p) d -> p t d", p=128)
            wkva_sb = p1w.tile([128, NT, KVLR], BF16)
            wr = wkva_v.rearrange("(c p) n -> p c n", p=128)
            # wcat: [k_pe cols of wkv_a | iwk_hi | iwk_lo]
            wcat = p1w.tile([128, NT, DR + 2 * IHD], BF16)
            ikh = iwkh_v.rearrange("(c p) n -> p c n", p=128)
            ikl = iwkl_v.rearrange("(c p) n -> p c n", p=128)
            for c in range(NT):
                nc.gpsimd.dma_start(out=cos_t[:, c, :], in_=cr[:, c, :])
                nc.gpsimd.dma_start(out=sin_t[:, c, :], in_=sr[:, c, :])
                nc.gpsimd.dma_start(out=wkva_sb[:, c, :],
                                    in_=wr[:, c, 0:KVLR])
                nc.gpsimd.dma_start(out=wcat[:, c, 0:DR],
                                    in_=wr[:, c, KVLR:])
                nc.gpsimd.dma_start(out=wcat[:, c, DR:DR + IHD],
                                    in_=ikh[:, c, :])
                nc.gpsimd.dma_start(out=wcat[:, c, DR + IHD:],
                                    in_=ikl[:, c, :])

            xrh = xTh_v.rearrange("(c p) (u q) -> p c u q", p=128, q=128)
            xrl = xTl_v.rearrange("(c p) (u q) -> p c u q", p=128, q=128)
            for t in range(NT):
                xt_hi = p1.tile([128, NT, 128], BF16, tag="xt_hi")
                xt_lo = p1.tile([128, NT, 128], BF16, tag="xt_lo")
                for c in range(NT):
                    nc.gpsimd.dma_start(out=xt_hi[:, c, :], in_=xrh[:, c, t, :])
                    nc.gpsimd.dma_start(out=xt_lo[:, c, :], in_=xrl[:, c, t, :])
                ps_kv = p1ps.tile([128, KVLR], F32, tag="ps_kv")
                ps_x = p1ps.tile([128, DR + 2 * IHD], F32, tag="ps_x")
                ps_kl = p1ps.tile([128, IHD], F32, tag="ps_kl")
                for f in range(NT):
                    st, sp = (f == 0), (f == NT - 1)
                    nc.tensor.matmul(ps_kv, xt_hi[:, f, :],
                                     wkva_sb[:, f, :], start=st, stop=sp)
                    nc.tensor.matmul(ps_x, xt_hi[:, f, :],
                                     wcat[:, f, :], start=st, stop=sp)
                    nc.tensor.matmul(ps_kl, xt_lo[:, f, :],
                                     wcat[:, f, DR:DR + IHD],
                                     start=st, stop=sp)
                # ckv rmsnorm -> bf16 -> transpose into ckvT
                ckv_sb = p1.tile([128, KVLR], F32, tag="ckv_sb")
                _rmsnorm_from_psum(nc, p1, ckv_sb, [ps_kv], kvnw, KVLR)
                ckv_bf = p1.tile([128, KVLR], BF16, tag="ckv_bf")
                nc.vector.tensor_copy(ckv_bf, ckv_sb)
                for ch in range(4):
                    ptr = p1tr.tile([128, 128], BF16, tag="ptr")
                    nc.tensor.transpose(ptr, ckv_bf[:, ch * 128:(ch + 1) * 128],
                                        ident)
                    nc.scalar.copy(out=ckvT[:, ch, t * 128:(t + 1) * 128],
                                   in_=ptr)
                # k_pe rope -> bf16 -> transpose into kpeT
                pe_sb = p1.tile([128, DR], F32, tag="pe_sb")
                _rope_int(nc, pe_sb, ps_x[:, 0:DR],
                          cos_t[:, t, :], sin_t[:, t, :])
                pe_bf = p1.tile([128, DR], BF16, tag="pe_bf")
                nc.vector.tensor_copy(pe_bf, pe_sb)
                ptr = p1tr.tile([128, 128], BF16, tag="ptr")
                nc.tensor.transpose(ptr[:64, :], pe_bf, ident)
                nc.scalar.copy(out=kpeT[:, t * 128:(t + 1) * 128],
                               in_=ptr[:64, :])
                # ki = layernorm(3-pass sum) + rope -> split -> transpose
                ki32 = p1.tile([128, IHD], F32, tag="ki32")
                nc.scalar.copy(out=ki32, in_=ps_x[:, DR:DR + IHD])
                nc.vector.tensor_add(ki32, ki32, ps_x[:, DR + IHD:])
                nc.vector.tensor_add(ki32, ki32, ps_kl)
                s1 = p1.tile([128, 2], F32, tag="ki_s")
                scr = p1.tile([128, IHD], F32, tag="ki_scr")
                nc.scalar.activation(out=scr, in_=ki32,
                                     func=mybir.ActivationFunctionType.Copy,
                                     accum_out=s1[:, 0:1])
                nc.scalar.activation(out=scr, in_=ki32,
                                     func=mybir.ActivationFunctionType.Square,
                                     accum_out=s1[:, 1:2])
                mom = p1.tile([128, 4], F32, tag="ki_m")
                nc.vector.tensor_scalar(out=mom[:, 0:1], in0=s1[:, 0:1],
                                        scalar1=1.0 / IHD, scalar2=None,
                                        op0=mybir.AluOpType.mult)
                nc.vector.tensor_scalar(out=mom[:, 1:2], in0=s1[:, 1:2],
                                        scalar1=1.0 / IHD, scalar2=None,
                                        op0=mybir.AluOpType.mult)
                nc.vector.tensor_mul(mom[:, 2:3], mom[:, 0:1], mom[:, 0:1])
                nc.vector.tensor_sub(mom[:, 2:3], mom[:, 1:2], mom[:, 2:3])
                nc.vector.tensor_scalar(out=mom[:, 2:3], in0=mom[:, 2:3],
                                        scalar1=1e-5, scalar2=None,
                                        op0=mybir.AluOpType.add)
                nc.scalar.activation(out=mom[:, 2:3], in_=mom[:, 2:3],
                                     func=mybir.ActivationFunctionType.Sqrt)
                nc.vector.reciprocal(out=mom[:, 3:4], in_=mom[:, 2:3])
                ki_n = p1.tile([128, IHD], F32, tag="ki_n")
                nc.vector.tensor_scalar(out=ki_n, in0=ki32,
                                        scalar1=mom[:, 0:1],
                                        scalar2=mom[:, 3:4],
                                        op0=mybir.AluOpType.subtract,
                                        op1=mybir.AluOpType.mult)
                nc.vector.tensor_mul(ki_n, ki_n, knw)
                nc.vector.tensor_add(ki_n, ki_n, knb)
                ki_r = p1.tile([128, IHD], F32, tag="ki_r")
                _rope_ni(nc, ki_r, ki_n, cos_t[:, t, :], sin_t[:, t, :])
                ki_hi, ki_lo = _split(nc, p1, ki_r, IHD, "ki")
                ptr = p1tr.tile([128, 128], BF16, tag="ptr")
                nc.tensor.transpose(ptr[:64, :], ki_hi, ident)
                nc.scalar.copy(out=kiT_hi[:, t * 128:(t + 1) * 128],
                               in_=ptr[:64, :])
                ptr = p1tr.tile([128, 128], BF16, tag="ptr")
                nc.tensor.transpose(ptr[:64, :], ki_lo, ident)
                nc.scalar.copy(out=kiT_lo[:, t * 128:(t + 1) * 128],
                               in_=ptr[:64, :])

        if dbg:
            nc.gpsimd.dma_start(out=d_ckvT,
                                in_=ckvT.rearrange("p a b -> p (a b)"))
            nc.gpsimd.dma_start(out=d_kpeT, in_=kpeT)
            nc.gpsimd.dma_start(out=d_kiT[:, 0:S], in_=kiT_hi)
            nc.gpsimd.dma_start(out=d_kiT[:, S:], in_=kiT_lo)

        # ---------------- P2: query-block projections ----------------
        mid = tc.alloc_tile_pool(name="mid", bufs=1)
        qTn = mid.tile([128, H, NB], BF16)       # nope part, feature-major
        qTp = mid.tile([64, H, NB], BF16)        # rope part
        qiT_hi = mid.tile([64, IH, NB], BF16)
        qiT_lo = mid.tile([64, IH, NB], BF16)

        with tc.tile_pool(name="p2w", bufs=2) as p2w, \
             tc.tile_pool(name="p2", bufs=2) as p2, \
             tc.tile_pool(name="p2ps", bufs=1, space="PSUM") as p2ps, \
             tc.tile_pool(name="p2tr", bufs=1, space="PSUM") as p2tr:
            cosb = p2.tile([128, NQT, DR], F32, tag="cosb", bufs=1)
            sinb = p2.tile([128, NQT, DR], F32, tag="sinb", bufs=1)
            for q in range(NQT):
                nc.gpsimd.dma_start(
                    out=cosb[:, q, :],
                    in_=cos_v[bass.ds(r0 + q * 128, 128), :])
                nc.gpsimd.dma_start(
                    out=sinb[:, q, :],
                    in_=sin_v[bass.ds(r0 + q * 128, 128), :])
            qnw = p2.tile([128, QLR], F32, tag="qnw", bufs=1)
            nc.gpsimd.dma_start(out=qnw, in_=_vb(gf, F32_OFF["q_norm_w"], QLR))
            gcat_w = p2.tile([128, NT, 2 * IH], BF16, tag="gcat", bufs=1)
            igh_r = igh_v.rearrange("(c p) n -> p c n", p=128)
            igl_r = igl_v.rearrange("(c p) n -> p c n", p=128)
            for c in range(NT):
                nc.gpsimd.dma_start(out=gcat_w[:, c, 0:IH], in_=igh_r[:, c, :])
                nc.gpsimd.dma_start(out=gcat_w[:, c, IH:], in_=igl_r[:, c, :])
            ps_qr = [p2ps.tile([128, 512], F32, tag=f"ps_qr{q}{i}",
                               name=f"ps_qr{q}{i}")
                     for q in range(NQT) for i in range(2)]
            ps_g = [p2ps.tile([128, 2 * IH], F32, tag=f"ps_g{q}",
                              name=f"ps_g{q}") for q in range(NQT)]
            for f in range(NT):
                wqah_f = p2w.tile([128, QLR], BF16, tag="wqah_f")
                nc.gpsimd.dma_start(out=wqah_f,
                                    in_=wqah_v[f * 128:(f + 1) * 128, :])
                wqal_f = p2w.tile([128, QLR], BF16, tag="wqal_f")
                nc.gpsimd.dma_start(out=wqal_f,
                                    in_=wqal_v[f * 128:(f + 1) * 128, :])
                xq_hi = p2w.tile([128, NB], BF16, tag="xq_hi", bufs=3)
                nc.gpsimd.dma_start(
                    out=xq_hi,
                    in_=xTh_v[f * 128:(f + 1) * 128, bass.ds(r0, NB)])
                xq_lo = p2w.tile([128, NB], BF16, tag="xq_lo", bufs=3)
                nc.gpsimd.dma_start(
                    out=xq_lo,
                    in_=xTl_v[f * 128:(f + 1) * 128, bass.ds(r0, NB)])
                st, sp = (f == 0), (f == NT - 1)
                for q in range(NQT):
                    lhs_hi = xq_hi[:, q * 128:(q + 1) * 128]
                    lhs_lo = xq_lo[:, q * 128:(q + 1) * 128]
                    for i in range(2):
                        cols = slice(i * 512, (i + 1) * 512)
                        nc.tensor.matmul(ps_qr[2 * q + i], lhs_hi,
                                         wqah_f[:, cols], start=st, stop=False)
                        nc.tensor.matmul(ps_qr[2 * q + i], lhs_hi,
                                         wqal_f[:, cols], start=False,
                                         stop=False)
                        nc.tensor.matmul(ps_qr[2 * q + i], lhs_lo,
                                         wqah_f[:, cols], start=False, stop=sp)
                    nc.tensor.matmul(ps_g[q][:, 0:2 * IH], lhs_hi,
                                     gcat_w[:, f, :], start=st, stop=False)
                    nc.tensor.matmul(ps_g[q][:, 0:IH], lhs_lo,
                                     gcat_w[:, f, 0:IH], start=False, stop=sp)
            qrT_hi = p2.tile([128, 8, NB], BF16, tag="qrT_hi", bufs=1)
            qrT_lo = p2.tile([128, 8, NB], BF16, tag="qrT_lo", bufs=1)
            gate_sb = p2.tile([128, NQT, IH], F32, tag="gate_sb", bufs=1)
            for q in range(NQT):
                qr_sb = p2.tile([128, QLR], F32, tag="qr_sb")
                _rmsnorm_from_psum(nc, p2, qr_sb,
                                   [ps_qr[2 * q], ps_qr[2 * q + 1]], qnw, QLR)
                nc.scalar.copy(out=gate_sb[:, q, :], in_=ps_g[q][:, 0:IH])
                nc.vector.tensor_add(gate_sb[:, q, :], gate_sb[:, q, :],
                                     ps_g[q][:, IH:2 * IH])
                nc.vector.tensor_scalar(out=gate_sb[:, q, :],
                                        in0=gate_sb[:, q, :],
                                        scalar1=SCALE_GATE * SCALE_IDX,
                                        scalar2=None,
                                        op0=mybir.AluOpType.mult)
                qr_hi, qr_lo = _split(nc, p2, qr_sb, QLR, "qr")
                for ch in range(8):
                    cols = slice(ch * 128, (ch + 1) * 128)
                    ptr = p2tr.tile([128, 128], BF16, tag="ptr2")
                    nc.tensor.transpose(ptr, qr_hi[:, cols], ident)
                    nc.scalar.copy(out=qrT_hi[:, ch, q * 128:(q + 1) * 128],
                                   in_=ptr)
                    ptr = p2tr.tile([128, 128], BF16, tag="ptr2")
                    nc.tensor.transpose(ptr, qr_lo[:, cols], ident)
                    nc.scalar.copy(out=qrT_lo[:, ch, q * 128:(q + 1) * 128],
                                   in_=ptr)
            # q projection per MLA head (bf16)
            wqbr = wqb_v.rearrange("(c p) n -> p c n", p=128)
            for h in range(H):
                wqb_h = p2w.tile([128, 8, DN + DR], BF16, tag="wqb_h")
                for c in range(8):
                    nc.gpsimd.dma_start(
                        out=wqb_h[:, c, :],
                        in_=wqbr[:, c, h * (DN + DR):(h + 1) * (DN + DR)])
                for q in range(NQT):
                    ps_q = p2ps.tile([128, DN + DR], F32, tag="ps_q")
                    for ch in range(8):
                        nc.tensor.matmul(
                            ps_q, qrT_hi[:, ch, q * 128:(q + 1) * 128],
                            wqb_h[:, ch, :],
                            start=(ch == 0), stop=(ch == 7))
                    qn_bf = p2.tile([128, DN], BF16, tag="qn_bf")
                    nc.vector.tensor_scalar(out=qn_bf, in0=ps_q[:, 0:DN],
                                            scalar1=SCALE_MLA, scalar2=None,
                                            op0=mybir.AluOpType.mult)
                    qp32 = p2.tile([128, DR], F32, tag="qp32")
                    _rope_int(nc, qp32, ps_q[:, DN:],
                              cosb[:, q, :], sinb[:, q, :])
                    qp_bf = p2.tile([128, DR], BF16, tag="qp_bf")
                    nc.vector.tensor_scalar(out=qp_bf, in0=qp32,
                                            scalar1=SCALE_MLA, scalar2=None,
                                            op0=mybir.AluOpType.mult)
                    ptr = p2tr.tile([128, 128], BF16, tag="ptr2")
                    nc.tensor.transpose(ptr, qn_bf, ident)
                    nc.scalar.copy(out=qTn[:, h, q * 128:(q + 1) * 128],
                                   in_=ptr)
                    ptr = p2tr.tile([128, 128], BF16, tag="ptr2")
                    nc.tensor.transpose(ptr[:64, :], qp_bf, ident)
                    nc.scalar.copy(out=qTp[:, h, q * 128:(q + 1) * 128],
                                   in_=ptr[:64, :])
            # indexer q heads: 3-pass hi/lo, rope, * gate, split, transpose
            iwqbh_r = iwqbh_v.rearrange("(c p) n -> p c n", p=128)
            iwqbl_r = iwqbl_v.rearrange("(c p) n -> p c n", p=128)
            for ih in range(IH):
                wiq_cat = p2w.tile([128, 8, 2 * IHD], BF16, tag="wiq_cat")
                for c in range(8):
                    nc.gpsimd.dma_start(
                        out=wiq_cat[:, c, 0:IHD],
                        in_=iwqbh_r[:, c, ih * IHD:(ih + 1) * IHD])
                    nc.gpsimd.dma_start(
                        out=wiq_cat[:, c, IHD:],
                        in_=iwqbl_r[:, c, ih * IHD:(ih + 1) * IHD])
                for q in range(NQT):
                    ps_qc = p2ps.tile([128, 2 * IHD], F32, tag="ps_q")
                    for ch in range(8):
                        nc.tensor.matmul(
                            ps_qc[:, 0:2 * IHD],
                            qrT_hi[:, ch, q * 128:(q + 1) * 128],
                            wiq_cat[:, ch, :],
                            start=(ch == 0), stop=False)
                        nc.tensor.matmul(
                            ps_qc[:, 0:IHD],
                            qrT_lo[:, ch, q * 128:(q + 1) * 128],
                            wiq_cat[:, ch, 0:IHD],
                            start=False, stop=(ch == 7))
                    qi32 = p2.tile([128, IHD], F32, tag="qi32")
                    nc.scalar.copy(out=qi32, in_=ps_qc[:, 0:IHD])
                    nc.vector.tensor_add(qi32, qi32, ps_qc[:, IHD:2 * IHD])
                    qi_r = p2.tile([128, IHD], F32, tag="qi_r")
                    _rope_ni(nc, qi_r, qi32, cosb[:, q, :], sinb[:, q, :])
                    nc.vector.tensor_scalar(out=qi_r, in0=qi_r,
                                            scalar1=gate_sb[:, q, ih:ih + 1],
                                            scalar2=None,
                                            op0=mybir.AluOpType.mult)
                    qi_hi, qi_lo = _split(nc, p2, qi_r, IHD, "qi")
                    ptr = p2tr.tile([128, 128], BF16, tag="ptr2")
                    nc.tensor.transpose(ptr[:64, :], qi_hi, ident)
                    nc.scalar.copy(out=qiT_hi[:, ih, q * 128:(q + 1) * 128],
                                   in_=ptr[:64, :])
                    ptr = p2tr.tile([128, 128], BF16, tag="ptr2")
                    nc.tensor.transpose(ptr[:64, :], qi_lo, ident)
                    nc.scalar.copy(out=qiT_lo[:, ih, q * 128:(q + 1) * 128],
                                   in_=ptr[:64, :])
            if dbg:
                nc.gpsimd.dma_start(out=d_qrT[:, 0:8 * NB],
                                    in_=qrT_hi.rearrange("p a b -> p (a b)"))
                nc.gpsimd.dma_start(out=d_qrT[:, 8 * NB:],
                                    in_=qrT_lo.rearrange("p a b -> p (a b)"))
                nc.gpsimd.dma_start(out=d_gate,
                                    in_=gate_sb.rearrange("p a b -> p (a b)"))
                nc.gpsimd.dma_start(out=d_qiT[:, 0:IH * NB],
                                    in_=qiT_hi.rearrange("p a b -> p (a b)"))
                nc.gpsimd.dma_start(out=d_qiT[:, IH * NB:],
                                    in_=qiT_lo.rearrange("p a b -> p (a b)"))

        # ---------------- P3: index scores + EXACT top-k ----------------
        maskNEG = mid.tile([128, NQT, S], F32)
        with tc.tile_pool(name="p3", bufs=1) as p3, \
             tc.tile_pool(name="p3ps", bufs=4, space="PSUM") as p3ps:
            amask_v = None
            if not causal:
                amask_v = _v(gf, F32_OFF["amask"], S, S)
            for q in range(NQT):
                cm = p3.tile([128, S], F32, tag="cm")
                if causal:
                    # cmask = (col > row) * NEG
                    rq = p3.tile([128, 1], F32, tag="rq")
                    nc.vector.tensor_scalar(out=rq, in0=rowid,
                                            scalar1=float(q * 128),
                                            scalar2=None,
                                            op0=mybir.AluOpType.add)
                    nc.vector.tensor_scalar(out=cm, in0=iota_sb,
                                            scalar1=rq, scalar2=NEG,
                                            op0=mybir.AluOpType.is_gt,
                                            op1=mybir.AluOpType.mult)
                else:
                    nc.gpsimd.dma_start(
                        out=cm, in_=amask_v[bass.ds(r0 + q * 128, 128), :])
                isc = p3.tile([128, S], F32, tag="isc")
                for kc in range(4):
                    cols = slice(kc * 512, (kc + 1) * 512)
                    ps = p3ps.tile([128, 512], F32, tag="ps_isc")
                    for ih in range(IH):
                        qcols = slice(q * 128, (q + 1) * 128)
                        nc.tensor.matmul(ps, qiT_hi[:, ih, qcols],
                                         kiT_hi[:, cols],
                                         start=(ih == 0), stop=False)
                        nc.tensor.matmul(ps, qiT_hi[:, ih, qcols],
                                         kiT_lo[:, cols],
                                         start=False, stop=False)
                        nc.tensor.matmul(ps, qiT_lo[:, ih, qcols],
                                         kiT_hi[:, cols],
                                         start=False, stop=(ih == IH - 1))
                    nc.vector.tensor_add(isc[:, cols], ps, cm[:, cols])
                # clamp; masked cols sit at -200 (amask re-kills them later)
                nc.vector.tensor_scalar(out=isc, in0=isc, scalar1=-200.0,
                                        scalar2=None, op0=mybir.AluOpType.max)
                # exact top-256 threshold: 32 rounds of top-8 + replace
                scr = p3.tile([128, S], F32, tag="sel_scr")
                nc.vector.tensor_copy(scr, isc)
                mx = p3.tile([128, 8], F32, tag="mx")
                for r in range(TOPK // 8):
                    nc.vector.max(out=mx, in_=scr)
                    if r < TOPK // 8 - 1:
                        nc.vector.match_replace(out=scr, in_to_replace=mx,
                                                in_values=scr, imm_value=-3e9)
                nc.vector.tensor_scalar(out=maskNEG[:, q, :], in0=isc,
                                        scalar1=mx[:, 7:8], scalar2=NEG,
                                        op0=mybir.AluOpType.is_lt,
                                        op1=mybir.AluOpType.mult)
                nc.vector.tensor_add(maskNEG[:, q, :], maskNEG[:, q, :], cm)
                if dbg:
                    nc.gpsimd.dma_start(out=d_thr[:, q:q + 1], in_=mx[:, 7:8])

        if dbg:
            nc.gpsimd.dma_start(out=d_mask,
                                in_=maskNEG.rearrange("p a b -> p (a b)"))

        # ---------------- P4: sparse MLA attention per head ----------------
        out_hT = mid.tile([128, H, NB], BF16)
        with tc.tile_pool(name="p4w", bufs=2) as p4w, \
             tc.tile_pool(name="p4k", bufs=2) as p4k, \
             tc.tile_pool(name="p4p", bufs=2) as p4p, \
             tc.tile_pool(name="p4ps", bufs=2, space="PSUM") as p4ps, \
             tc.tile_pool(name="p4po", bufs=2, space="PSUM") as p4po:
            wbr = wkvb_v.rearrange("(c p) n -> p c n", p=128)
            for h in range(H):
                wb_k = p4w.tile([128, 4, DN], BF16, tag="wb_k")
                wb_v = p4w.tile([128, 4, DV], BF16, tag="wb_v")
                for c in range(4):
                    nc.gpsimd.dma_start(
                        out=wb_k[:, c, :],
                        in_=wbr[:, c, h * (DN + DV):h * (DN + DV) + DN])
                    nc.gpsimd.dma_start(
                        out=wb_v[:, c, :],
                        in_=wbr[:, c, h * (DN + DV) + DN:(h + 1) * (DN + DV)])
                knT = p4k.tile([128, S], BF16, tag="knT")
                for kc in range(4):
                    ps = p4ps.tile([128, 512], F32, tag="ps_kn")
                    for c in range(4):
                        nc.tensor.matmul(
                            ps, wb_k[:, c, :],
                            ckvT[:, c, kc * 512:(kc + 1) * 512],
                            start=(c == 0), stop=(c == 3))
                    nc.scalar.copy(out=knT[:, kc * 512:(kc + 1) * 512], in_=ps)
                v_sb = p4k.tile([128, NT, DV], BF16, tag="v_sb")
                for kt in range(NT):
                    ps = p4ps.tile([128, DV], F32, tag="ps_v")
                    for c in range(4):
                        nc.tensor.matmul(
                            ps,
                            ckvT[:, c, kt * 128:(kt + 1) * 128],
                            wb_v[:, c, :],
                            start=(c == 0), stop=(c == 3))
                    nc.scalar.copy(out=v_sb[:, kt, :], in_=ps)
                ps_o = p4po.tile([128, NB], F32, tag="ps_o")
                for q in range(NQT):
                    probs = p4p.tile([128, S], F32, tag="probs", bufs=1)
                    for kc in range(4):
                        cols = slice(kc * 512, (kc + 1) * 512)
                        ps = p4ps.tile([128, 512], F32, tag="ps_s")
                        nc.tensor.matmul(
                            ps, qTn[:, h, q * 128:(q + 1) * 128],
                            knT[:, cols], start=True, stop=False)
                        nc.tensor.matmul(
                            ps, qTp[:, h, q * 128:(q + 1) * 128],
                            kpeT[:, cols], start=False, stop=True)
                        nc.vector.tensor_add(probs[:, cols], ps,
                                             maskNEG[:, q, cols])
                    den = p4p.tile([128, 2], F32, tag="den")
                    nc.scalar.activation(out=probs, in_=probs,
                                         func=mybir.ActivationFunctionType.Exp,
                                         accum_out=den[:, 0:1])
                    nc.vector.reciprocal(out=den[:, 1:2], in_=den[:, 0:1])
                    pb = p4p.tile([128, S], BF16, tag="pb")
                    nc.vector.tensor_scalar(out=pb, in0=probs,
                                            scalar1=den[:, 1:2], scalar2=None,
                                            op0=mybir.AluOpType.mult)
                    pT = p4p.tile([128, NT, 128], BF16, tag="pT", bufs=1)
                    for kt in range(NT):
                        nc.scalar.dma_start_transpose(
                            out=pT[:, kt, :],
                            in_=pb[:, kt * 128:(kt + 1) * 128])
                    for kt in range(NT):
                        nc.tensor.matmul(
                            ps_o[:, q * 128:(q + 1) * 128],
                            v_sb[:, kt, :], pT[:, kt, :],
                            start=(kt == 0), stop=(kt == NT - 1))
                nc.scalar.copy(out=out_hT[:, h, :], in_=ps_o)

        if dbg:
            nc.gpsimd.dma_start(out=d_ohT,
                                in_=out_hT.rearrange("p a b -> p (a b)"))

        # ---------------- P5: output projection ----------------
        with tc.tile_pool(name="p5w", bufs=3) as p5w, \
             tc.tile_pool(name="p5", bufs=3) as p5, \
             tc.tile_pool(name="p5ps", bufs=4, space="PSUM") as p5ps:
            wor = wo_v.rearrange("(hh p) n -> p hh n", p=128)
            for g in range(NT):
                wo_g = p5w.tile([128, H, 128], BF16, tag="wo_g")
                for c in range(H):
                    nc.gpsimd.dma_start(
                        out=wo_g[:, c, :],
                        in_=wor[:, c, g * 128:(g + 1) * 128])
                ps = p5ps.tile([128, NB], F32, tag="ps_w")
                for h in range(H):
                    nc.tensor.matmul(ps, wo_g[:, h, :],
                                     out_hT[:, h, :],
                                     start=(h == 0), stop=(h == H - 1))
                ot = p5.tile([128, NB], FP16, tag="ot")
                nc.scalar.copy(out=ot, in_=ps)
                nc.gpsimd.dma_start(out=outT[g * 128:(g + 1) * 128, :], in_=ot)

        mid.release()
        consts.release()
        dram.release()
    nc.compile()
    return nc


_NC_CACHE = {}


def _get_nc(causal=True):
    if causal not in _NC_CACHE:
        _NC_CACHE[causal] = build_nc(causal)
    return _NC_CACHE[causal]


def _split_np(a):
    hi = a.astype(ml_dtypes.bfloat16)
    lo = (a - hi.astype(np.float32)).astype(ml_dtypes.bfloat16)
    return hi, lo


def _is_causal(am):
    s = am.shape[-1]
    r = np.arange(s, dtype=np.int64)
    causal = np.where(r[:, None] >= r[None, :], np.float32(0.0),
                      np.float32(NEG))
    return np.array_equal(am.reshape(s, s), causal)


def make_core_inputs(x, cos, sin, attn_mask, wq_a, q_norm_w, wq_b, wkv_a,
                     kv_norm_w, wkv_b, wo, idx_wq_b, idx_wk, idx_knorm_w,
                     idx_knorm_b, idx_gate):
    causal = _is_causal(np.asarray(attn_mask, np.float32))
    F32_OFF, F32_TOT = _f32_layout(causal)

    blob_bf = np.zeros(BF_TOT, ml_dtypes.bfloat16)

    def put_bf(name, arr):
        o = BF_OFF[name]
        blob_bf[o:o + arr.size] = np.ascontiguousarray(arr).reshape(-1)

    xT = np.ascontiguousarray(x[0].astype(np.float32).T)
    xh, xl = _split_np(xT)
    put_bf("xT_hi", xh)
    put_bf("xT_lo", xl)
    wh, wl = _split_np(np.asarray(wq_a, np.float32))
    put_bf("wqa_hi", wh)
    put_bf("wqa_lo", wl)
    put_bf("wq_b", np.asarray(wq_b, np.float32).astype(ml_dtypes.bfloat16))
    put_bf("wkv_a", np.asarray(wkv_a, np.float32).astype(ml_dtypes.bfloat16))
    put_bf("wkv_b", np.asarray(wkv_b, np.float32).astype(ml_dtypes.bfloat16))
    put_bf("wo", np.asarray(wo, np.float32).astype(ml_dtypes.bfloat16))
    ih_, il_ = _split_np(np.asarray(idx_wq_b, np.float32))
    put_bf("iwqb_hi", ih_)
    put_bf("iwqb_lo", il_)
    kh, kl = _split_np(np.asarray(idx_wk, np.float32))
    put_bf("iwk_hi", kh)
    put_bf("iwk_lo", kl)
    gh, gl = _split_np(np.asarray(idx_gate, np.float32))
    put_bf("igate_hi", gh)
    put_bf("igate_lo", gl)
    put_bf("ident", np.eye(128, dtype=np.float32))

    blob_f32 = np.zeros(F32_TOT, np.float32)

    def put_f(name, arr):
        o = F32_OFF[name]
        blob_f32[o:o + arr.size] = np.ascontiguousarray(
            arr, np.float32).reshape(-1)

    put_f("cos", cos[0])
    put_f("sin", sin[0])
    put_f("q_norm_w", q_norm_w)
    put_f("kv_norm_w", kv_norm_w)
    put_f("idx_knorm_w", idx_knorm_w)
    put_f("idx_knorm_b", idx_knorm_b)
    put_f("iota", np.arange(S, dtype=np.float32))
    rows = (np.arange(NCORES)[:, None] * NB
            + np.arange(128)[None, :]).astype(np.float32)
    put_f("rows", rows)
    if not causal:
        put_f("amask", attn_mask[0, 0])

    lb, lf = BF_TOT // NCORES, F32_TOT // NCORES
    maps = []
    for c in range(NCORES):
        maps.append({
            "shard_bf": np.ascontiguousarray(blob_bf[c * lb:(c + 1) * lb]),
            "shard_f32": np.ascontiguousarray(blob_f32[c * lf:(c + 1) * lf]),
        })
    return maps, causal


def kernel(x, cos, sin, attn_mask, wq_a, q_norm_w, wq_b, wkv_a, kv_norm_w,
           wkv_b, wo, idx_wq_b, idx_wk, idx_knorm_w, idx_knorm_b, idx_gate):
    from concourse.bass_utils import run_bass_kernel_spmd
    maps, causal = make_core_inputs(
        x, cos, sin, attn_mask, wq_a, q_norm_w, wq_b, wkv_a, kv_norm_w,
        wkv_b, wo, idx_wq_b, idx_wk, idx_knorm_w, idx_knorm_b, idx_gate)
    nc = _get_nc(causal)
    res = run_bass_kernel_spmd(nc, maps, list(range(NCORES)))
    outs = [np.asarray(r["outT"]).astype(np.float32).T for r in res.results]
    out = np.concatenate(outs, axis=0)[None]                   # [1, S, HID]
    return out.astype(np.float32)


# revision 12
# speedup vs baseline: 10.4456x; 1.0941x over previous
"""DSA sparse MLA attention kernel for TRN2, 8 NeuronCores.

Transfer-optimized SPMD design. The host->device tunnel moves ~52 MB/s,
so every input byte is shipped exactly ONCE: each core receives a 1/8
slice of two packed blobs (bf16 + f32) and the cores AllGather them
on-device (HBM-to-HBM over on-chip links, ~GB/ms). Per-core query-block
slices are carved out of the gathered blobs at runtime with
partition_id()-based dynamic DMA offsets, so no per-core host tensors
are needed at all.

Precision plan (harness gate: rel_err < 2e-2; this lands ~6e-3):
 - Indexer path (x, wq_a, idx_*) is selection-critical: tensors are
   shipped as hi/lo bf16 pairs (same bytes as f32) and matmuls use a
   3-pass hi/lo bf16 emulation (~1e-5 rel, 4x faster than fp32r which
   is only ~1e-3 accurate).
 - Top-256 selection is EXACT: 32 rounds of vector.max + match_replace
   give the true 256th-largest index score per row.
 - Attention path (wq_b, wkv_a, wkv_b, wo, q/k/v, probs) is plain bf16.
 - Output is fp16 (halves the donated-zeros upload + fetch).

Sharding: sequence-parallel over query rows; core c owns rows
[256c, 256(c+1)). KV/indexer-key expansion over all 2048 keys is
replicated (compute is ~free vs transfer).
"""

import numpy as np
import ml_dtypes

import jax

# Persistent XLA compilation cache: run_bass_kernel_spmd re-jits (and
# would re-run the walrus NEFF compile, ~1s) on every call; the disk
# cache turns that into a deserialize+load.
jax.config.update("jax_compilation_cache_dir", "/tmp/jax_cc_cache")
jax.config.update("jax_persistent_cache_min_compile_time_secs", 0.0)
jax.config.update("jax_persistent_cache_min_entry_size_bytes", 0)

import concourse.bass as bass
import concourse.bacc as bacc
import concourse.mybir as mybir
from concourse.tile import TileContext

F32 = mybir.dt.float32
BF16 = mybir.dt.bfloat16
FP16 = mybir.dt.float16

S, HID = 2048, 2048
H, DN, DR, DV = 16, 128, 64, 128
QLR, KVLR = 1024, 512
IH, IHD, TOPK = 8, 64, 256
NEG = -1e9
NB = 256            # query rows per core
NCORES = 8
NT = S // 128       # 16 token tiles
NQT = NB // 128     # 2 query tiles per core
SCALE_MLA = float((DN + DR) ** -0.5)
SCALE_IDX = float(IHD ** -0.5)
SCALE_GATE = float(IH ** -0.5)
ALIGN = 512         # element alignment for blob entries

BF_LAYOUT = [
    ("xT_hi", (HID, S)), ("xT_lo", (HID, S)),
    ("wqa_hi", (HID, QLR)), ("wqa_lo", (HID, QLR)),
    ("wq_b", (QLR, H * (DN + DR))),
    ("wkv_a", (HID, KVLR + DR)),
    ("wkv_b", (KVLR, H * (DN + DV))),
    ("wo", (H * DV, HID)),
    ("iwqb_hi", (QLR, IH * IHD)), ("iwqb_lo", (QLR, IH * IHD)),
    ("iwk_hi", (HID, IHD)), ("iwk_lo", (HID, IHD)),
    ("igate_hi", (HID, IH)), ("igate_lo", (HID, IH)),
    ("ident", (128, 128)),
]

F32_LAYOUT_BASE = [
    ("cos", (S, DR)), ("sin", (S, DR)),
    ("q_norm_w", (1, QLR)), ("kv_norm_w", (1, KVLR)),
    ("idx_knorm_w", (1, IHD)), ("idx_knorm_b", (1, IHD)),
    ("iota", (1, S)),
    ("rows", (NCORES * 128, 1)),
]


def _mk_layout(entries):
    offs, off = {}, 0
    for name, shape in entries:
        offs[name] = off
        n = int(np.prod(shape))
        off += (n + ALIGN - 1) // ALIGN * ALIGN
    tot = (off + NCORES * ALIGN - 1) // (NCORES * ALIGN) * (NCORES * ALIGN)
    return offs, tot


def _f32_layout(causal):
    return _mk_layout(list(F32_LAYOUT_BASE))


BF_OFF, BF_TOT = _mk_layout(BF_LAYOUT)
LB = BF_TOT // NCORES


def _v(blob1d, off, r, c):
    """[r, c] row-major view at element offset off of a 1-D DRAM AP."""
    return blob1d[off:off + r * c].rearrange("(r c) -> r c", c=c)


def _vb(blob1d, off, n, parts=128):
    """Partition-broadcast view [parts, n] of n elements at offset off."""
    return bass.AP(tensor=blob1d.tensor, offset=blob1d.offset + off,
                   ap=[[0, parts], [1, n]])


def _rmsnorm_from_psum(nc, pool, out_sb, psums, wb, d, eps=1e-6):
    """out_sb[p, d] = psum * rsqrt(mean(psum^2)+eps) * w."""
    ssq = pool.tile([128, len(psums)], F32)
    for i, ps in enumerate(psums):
        w = ps.shape[-1]
        scr = pool.tile([128, 512], F32, tag="rms_scr")
        nc.scalar.activation(out=scr[:, :w], in_=ps,
                             func=mybir.ActivationFunctionType.Square,
                             accum_out=ssq[:, i:i + 1])
    tot = pool.tile([128, 1], F32)
    if len(psums) == 1:
        nc.vector.tensor_scalar(out=tot, in0=ssq, scalar1=1.0 / d,
                                scalar2=eps, op0=mybir.AluOpType.mult,
                                op1=mybir.AluOpType.add)
    else:
        nc.vector.tensor_reduce(out=tot, in_=ssq, axis=mybir.AxisListType.X,
                                op=mybir.AluOpType.add)
        nc.vector.tensor_scalar(out=tot, in0=tot, scalar1=1.0 / d,
                                scalar2=eps, op0=mybir.AluOpType.mult,
                                op1=mybir.AluOpType.add)
    nc.scalar.activation(out=tot, in_=tot,
                         func=mybir.ActivationFunctionType.Sqrt)
    rinv = pool.tile([128, 1], F32)
    nc.vector.reciprocal(out=rinv, in_=tot)
    off = 0
    for ps in psums:
        w = ps.shape[-1]
        nc.vector.tensor_scalar(out=out_sb[:, off:off + w], in0=ps,
                                scalar1=rinv, scalar2=None,
                                op0=mybir.AluOpType.mult)
        off += w
    nc.vector.tensor_mul(out_sb[:, :d], out_sb[:, :d], wb[:, :d])


def _rope_int(nc, out, in_, cos, sin):
    """Interleaved (GPT-J) rope, token-major [128, 64] -> out[128, 64]."""
    xp = in_.rearrange("p (a b) -> p a b", b=2)
    op = out.rearrange("p (a b) -> p a b", b=2)
    c, s = cos[:, 0:32], sin[:, 0:32]
    x1, x2 = xp[:, :, 0], xp[:, :, 1]
    nc.vector.tensor_mul(op[:, :, 0], x1, c)
    nc.vector.tensor_mul(op[:, :, 1], x2, c)
    t = nc._rope_scr.tile([128, 32], F32, tag="rope_t")
    nc.vector.tensor_mul(t, x2, s)
    nc.vector.tensor_sub(op[:, :, 0], op[:, :, 0], t)
    nc.vector.tensor_mul(t, x1, s)
    nc.vector.tensor_add(op[:, :, 1], op[:, :, 1], t)


def _rope_ni(nc, out, in_, cos, sin):
    """Non-interleaved (rotate_half) rope, [128, 64]."""
    x1, x2 = in_[:, 0:32], in_[:, 32:64]
    c1, c2 = cos[:, 0:32], cos[:, 32:64]
    s1, s2 = sin[:, 0:32], sin[:, 32:64]
    nc.vector.tensor_mul(out[:, 0:32], x1, c1)
    nc.vector.tensor_mul(out[:, 32:64], x2, c2)
    t = nc._rope_scr.tile([128, 32], F32, tag="rope_t")
    nc.vector.tensor_mul(t, x2, s1)
    nc.vector.tensor_sub(out[:, 0:32], out[:, 0:32], t)
    nc.vector.tensor_mul(t, x1, s2)
    nc.vector.tensor_add(out[:, 32:64], out[:, 32:64], t)


def _split(nc, pool, src_f32, n, tag):
    """f32 [128, n] -> (hi bf16, lo bf16) with hi+lo ~= src."""
    hi = pool.tile([128, n], BF16, tag=tag + "_hi")
    nc.vector.tensor_copy(hi, src_f32)
    hi32 = pool.tile([128, n], F32, tag=tag + "_h32")
    nc.vector.tensor_copy(hi32, hi)
    lo32 = pool.tile([128, n], F32, tag=tag + "_l32")
    nc.vector.tensor_sub(lo32, src_f32, hi32)
    lo = pool.tile([128, n], BF16, tag=tag + "_lo")
    nc.vector.tensor_copy(lo, lo32)
    return hi, lo


class _Bacc(bacc.Bacc):
    """Bacc with memoized BIR serialization: run_bass_kernel_spmd re-lowers
    (and re-serializes the ~9 MB BIR) on every call; the module is frozen
    after compile(), so the bytes are reusable."""
    _json_cache = None

    def to_json_bytes(self):
        if self._json_cache is None:
            self._json_cache = super().to_json_bytes()
        return self._json_cache


def build_nc(causal=True, dbg=False):
    F32_OFF, F32_TOT = _f32_layout(causal)
    lf = F32_TOT // NCORES

    nc = _Bacc("TRN2", target_bir_lowering=False, debug=False)
    shard_bf = nc.dram_tensor("shard_bf", [LB], BF16, kind="ExternalInput").ap()
    shard_f32 = nc.dram_tensor("shard_f32", [lf], F32,
                               kind="ExternalInput").ap()
    outT = nc.dram_tensor("outT", [HID, NB], FP16, kind="ExternalOutput").ap()
    amask_d = None
    if not causal:
        amask_d = nc.dram_tensor("amask_rows", [NB, S], F32,
                                 kind="ExternalInput").ap()
    if dbg:
        d_ckvT = nc.dram_tensor("d_ckvT", [128, 4 * S], BF16,
                                kind="ExternalOutput").ap()
        d_kpeT = nc.dram_tensor("d_kpeT", [64, S], BF16,
                                kind="ExternalOutput").ap()
        d_kiT = nc.dram_tensor("d_kiT", [64, 2 * S], BF16,
                               kind="ExternalOutput").ap()
        d_qrT = nc.dram_tensor("d_qrT", [128, 2 * 8 * NB], BF16,
                               kind="ExternalOutput").ap()
        d_gate = nc.dram_tensor("d_gate", [128, NQT * IH], F32,
                                kind="ExternalOutput").ap()
        d_mask = nc.dram_tensor("d_mask", [128, NQT * S], F32,
                                kind="ExternalOutput").ap()
        d_thr = nc.dram_tensor("d_thr", [128, NQT], F32,
                               kind="ExternalOutput").ap()
        d_qiT = nc.dram_tensor("d_qiT", [64, 2 * IH * NB], BF16,
                               kind="ExternalOutput").ap()
        d_ohT = nc.dram_tensor("d_ohT", [128, H * NB], BF16,
                               kind="ExternalOutput").ap()

    with TileContext(nc) as tc:
        pid = nc.partition_id()
        r0 = pid * NB

        dram = tc.alloc_tile_pool(name="dram", bufs=1, space="DRAM")
        bounce_bf = dram.tile([LB], BF16)
        bounce_f32 = dram.tile([lf], F32)
        gath_bf = dram.tile([NCORES, LB], BF16)
        gath_f32 = dram.tile([NCORES, lf], F32)
        nc.gpsimd.dma_start(out=bounce_bf, in_=shard_bf)
        nc.gpsimd.dma_start(out=bounce_f32, in_=shard_f32)
        nc.gpsimd.collective_compute(
            "AllGather", mybir.AluOpType.bypass,
            replica_groups=[list(range(NCORES))],
            ins=[bounce_bf[:].opt()], outs=[gath_bf[:].opt()])
        nc.gpsimd.collective_compute(
            "AllGather", mybir.AluOpType.bypass,
            replica_groups=[list(range(NCORES))],
            ins=[bounce_f32[:].opt()], outs=[gath_f32[:].opt()])
        gb = gath_bf.rearrange("a b -> (a b)")
        gf = gath_f32.rearrange("a b -> (a b)")

        xTh_v = _v(gb, BF_OFF["xT_hi"], HID, S)
        xTl_v = _v(gb, BF_OFF["xT_lo"], HID, S)
        wqah_v = _v(gb, BF_OFF["wqa_hi"], HID, QLR)
        wqal_v = _v(gb, BF_OFF["wqa_lo"], HID, QLR)
        wqb_v = _v(gb, BF_OFF["wq_b"], QLR, H * (DN + DR))
        wkva_v = _v(gb, BF_OFF["wkv_a"], HID, KVLR + DR)
        wkvb_v = _v(gb, BF_OFF["wkv_b"], KVLR, H * (DN + DV))
        wo_v = _v(gb, BF_OFF["wo"], H * DV, HID)
        iwqbh_v = _v(gb, BF_OFF["iwqb_hi"], QLR, IH * IHD)
        iwqbl_v = _v(gb, BF_OFF["iwqb_lo"], QLR, IH * IHD)
        iwkh_v = _v(gb, BF_OFF["iwk_hi"], HID, IHD)
        iwkl_v = _v(gb, BF_OFF["iwk_lo"], HID, IHD)
        igh_v = _v(gb, BF_OFF["igate_hi"], HID, IH)
        igl_v = _v(gb, BF_OFF["igate_lo"], HID, IH)
        ident_v = _v(gb, BF_OFF["ident"], 128, 128)
        cos_v = _v(gf, F32_OFF["cos"], S, DR)
        sin_v = _v(gf, F32_OFF["sin"], S, DR)
        rows_v = _v(gf, F32_OFF["rows"], NCORES * 128, 1)

        consts = tc.alloc_tile_pool(name="consts", bufs=1)
        nc._rope_scr = consts

        ident = consts.tile([128, 128], BF16)
        nc.gpsimd.dma_start(out=ident, in_=ident_v)
        kvnw = consts.tile([128, KVLR], F32)
        nc.gpsimd.dma_start(out=kvnw, in_=_vb(gf, F32_OFF["kv_norm_w"], KVLR))
        knw = consts.tile([128, IHD], F32)
        nc.gpsimd.dma_start(out=knw, in_=_vb(gf, F32_OFF["idx_knorm_w"], IHD))
        knb = consts.tile([128, IHD], F32)
        nc.gpsimd.dma_start(out=knb, in_=_vb(gf, F32_OFF["idx_knorm_b"], IHD))
        iota_sb = consts.tile([128, S], F32)
        nc.gpsimd.dma_start(out=iota_sb, in_=_vb(gf, F32_OFF["iota"], S))
        rowid = consts.tile([128, 1], F32)
        nc.gpsimd.dma_start(out=rowid, in_=rows_v[bass.ds(pid * 128, 128), :])

        ckvT = consts.tile([128, 4, S], BF16)      # [ckv_chunk, 4, tok]
        kpeT = consts.tile([64, S], BF16)
        kiT_hi = consts.tile([64, S], BF16)
        kiT_lo = consts.tile([64, S], BF16)

        # ---------------- P1: KV / indexer-key expansion ----------------
        with tc.tile_pool(name="p1w", bufs=1) as p1w, \
             tc.tile_pool(name="p1", bufs=3) as p1, \
             tc.tile_pool(name="p1ps", bufs=2, space="PSUM") as p1ps, \
             tc.tile_pool(name="p1tr", bufs=2, space="PSUM") as p1tr:
            cos_t = p1w.tile([128, NT, DR], F32)
            sin_t = p1w.tile([128, NT, DR], F32)
            cr = cos_v.rearrange("(t p) d -> p t d", p=128)
            sr = sin_v.rearrange("(t p) d -> p t d", p=128)
            wkva_sb = p1w.tile([128, NT, KVLR], BF16)
            wr = wkva_v.rearrange("(c p) n -> p c n", p=128)
            # wcat: [k_pe cols of wkv_a | iwk_hi | iwk_lo]
            wcat = p1w.tile([128, NT, DR + 2 * IHD], BF16)
            ikh = iwkh_v.rearrange("(c p) n -> p c n", p=128)
            ikl = iwkl_v.rearrange("(c p) n -> p c n", p=128)
            for c in range(NT):
                nc.gpsimd.dma_start(out=cos_t[:, c, :], in_=cr[:, c, :])
                nc.gpsimd.dma_start(out=sin_t[:, c, :], in_=sr[:, c, :])
                nc.gpsimd.dma_start(out=wkva_sb[:, c, :],
                                    in_=wr[:, c, 0:KVLR])
                nc.gpsimd.dma_start(out=wcat[:, c, 0:DR],
                                    in_=wr[:, c, KVLR:])
                nc.gpsimd.dma_start(out=wcat[:, c, DR:DR + IHD],
                                    in_=ikh[:, c, :])
                nc.gpsimd.dma_start(out=wcat[:, c, DR + IHD:],
                                    in_=ikl[:, c, :])

            xrh = xTh_v.rearrange("(c p) (u q) -> p c u q", p=128, q=128)
            xrl = xTl_v.rearrange("(c p) (u q) -> p c u q", p=128, q=128)
            for t in range(NT):
                xt_hi = p1.tile([128, NT, 128], BF16, tag="xt_hi")
                xt_lo = p1.tile([128, NT, 128], BF16, tag="xt_lo")
                for c in range(NT):
                    nc.gpsimd.dma_start(out=xt_hi[:, c, :], in_=xrh[:, c, t, :])
                    nc.gpsimd.dma_start(out=xt_lo[:, c, :], in_=xrl[:, c, t, :])
                ps_kv = p1ps.tile([128, KVLR], F32, tag="ps_kv")
                ps_x = p1ps.tile([128, DR + 2 * IHD], F32, tag="ps_x")
                ps_kl = p1ps.tile([128, IHD], F32, tag="ps_kl")
                for f in range(NT):
                    st, sp = (f == 0), (f == NT - 1)
                    nc.tensor.matmul(ps_kv, xt_hi[:, f, :],
                                     wkva_sb[:, f, :], start=st, stop=sp)
                    nc.tensor.matmul(ps_x, xt_hi[:, f, :],
                                     wcat[:, f, :], start=st, stop=sp)
                    nc.tensor.matmul(ps_kl, xt_lo[:, f, :],
                                     wcat[:, f, DR:DR + IHD],
                                     start=st, stop=sp)
                # ckv rmsnorm -> bf16 -> transpose into ckvT
                ckv_sb = p1.tile([128, KVLR], F32, tag="ckv_sb")
                _rmsnorm_from_psum(nc, p1, ckv_sb, [ps_kv], kvnw, KVLR)
                ckv_bf = p1.tile([128, KVLR], BF16, tag="ckv_bf")
                nc.vector.tensor_copy(ckv_bf, ckv_sb)
                for ch in range(4):
                    ptr = p1tr.tile([128, 128], BF16, tag="ptr")
                    nc.tensor.transpose(ptr, ckv_bf[:, ch * 128:(ch + 1) * 128],
                                        ident)
                    nc.scalar.copy(out=ckvT[:, ch, t * 128:(t + 1) * 128],
                                   in_=ptr)
                # k_pe rope -> bf16 -> transpose into kpeT
                pe_sb = p1.tile([128, DR], F32, tag="pe_sb")
                _rope_int(nc, pe_sb, ps_x[:, 0:DR],
                          cos_t[:, t, :], sin_t[:, t, :])
                pe_bf = p1.tile([128, DR], BF16, tag="pe_bf")
                nc.vector.tensor_copy(pe_bf, pe_sb)
                ptr = p1tr.tile([128, 128], BF16, tag="ptr")
                nc.tensor.transpose(ptr[:64, :], pe_bf, ident)
                nc.scalar.copy(out=kpeT[:, t * 128:(t + 1) * 128],
                               in_=ptr[:64, :])
                # ki = layernorm(3-pass sum) + rope -> split -> transpose
                ki32 = p1.tile([128, IHD], F32, tag="ki32")
                nc.scalar.copy(out=ki32, in_=ps_x[:, DR:DR + IHD])
                nc.vector.tensor_add(ki32, ki32, ps_x[:, DR + IHD:])
                nc.vector.tensor_add(ki32, ki32, ps_kl)
                s1 = p1.tile([128, 2], F32, tag="ki_s")
                scr = p1.tile([128, IHD], F32, tag="ki_scr")
                nc.scalar.activation(out=scr, in_=ki32,
                                     func=mybir.ActivationFunctionType.Copy,
                                     accum_out=s1[:, 0:1])
                nc.scalar.activation(out=scr, in_=ki32,
                                     func=mybir.ActivationFunctionType.Square,
                                     accum_out=s1[:, 1:2])
                mom = p1.tile([128, 4], F32, tag="ki_m")
                nc.vector.tensor_scalar(out=mom[:, 0:1], in0=s1[:, 0:1],
                                        scalar1=1.0 / IHD, scalar2=None,
                                        op0=mybir.AluOpType.mult)
                nc.vector.tensor_scalar(out=mom[:, 1:2], in0=s1[:, 1:2],
                                        scalar1=1.0 / IHD, scalar2=None,
                                        op0=mybir.AluOpType.mult)
                nc.vector.tensor_mul(mom[:, 2:3], mom[:, 0:1], mom[:, 0:1])
                nc.vector.tensor_sub(mom[:, 2:3], mom[:, 1:2], mom[:, 2:3])
                nc.vector.tensor_scalar(out=mom[:, 2:3], in0=mom[:, 2:3],
                                        scalar1=1e-5, scalar2=None,
                                        op0=mybir.AluOpType.add)
                nc.scalar.activation(out=mom[:, 2:3], in_=mom[:, 2:3],
                                     func=mybir.ActivationFunctionType.Sqrt)
                nc.vector.reciprocal(out=mom[:, 3:4], in_=mom[:, 2:3])
                ki_n = p1.tile([128, IHD], F32, tag="ki_n")
                nc.vector.tensor_scalar(out=ki_n, in0=ki32,
                                        scalar1=mom[:, 0:1],
                                        scalar2=mom[:, 3:4],
                                        op0=mybir.AluOpType.subtract,
                                        op1=mybir.AluOpType.mult)
                nc.vector.tensor_mul(ki_n, ki_n, knw)
                nc.vector.tensor_add(ki_n, ki_n, knb)
                ki_r = p1.tile([128, IHD], F32, tag="ki_r")
                _rope_ni(nc, ki_r, ki_n, cos_t[:, t, :], sin_t[:, t, :])
                ki_hi, ki_lo = _split(nc, p1, ki_r, IHD, "ki")
                ptr = p1tr.tile([128, 128], BF16, tag="ptr")
                nc.tensor.transpose(ptr[:64, :], ki_hi, ident)
                nc.scalar.copy(out=kiT_hi[:, t * 128:(t + 1) * 128],
                               in_=ptr[:64, :])
                ptr = p1tr.tile([128, 128], BF16, tag="ptr")
                nc.tensor.transpose(ptr[:64, :], ki_lo, ident)
                nc.scalar.copy(out=kiT_lo[:, t * 128:(t + 1) * 128],
                               in_=ptr[:64, :])

        if dbg:
            nc.gpsimd.dma_start(out=d_ckvT,
                                in_=ckvT.rearrange("p a b -> p (a b)"))
            nc.gpsimd.dma_start(out=d_kpeT, in_=kpeT)
            nc.gpsimd.dma_start(out=d_kiT[:, 0:S], in_=kiT_hi)
            nc.gpsimd.dma_start(out=d_kiT[:, S:], in_=kiT_lo)

        # ---------------- P2: query-block projections ----------------
        mid = tc.alloc_tile_pool(name="mid", bufs=1)
        qTn = mid.tile([128, H, NB], BF16)       # nope part, feature-major
        qTp = mid.tile([64, H, NB], BF16)        # rope part
        qiT_hi = mid.tile([64, IH, NB], BF16)
        qiT_lo = mid.tile([64, IH, NB], BF16)

        with tc.tile_pool(name="p2w", bufs=2) as p2w, \
             tc.tile_pool(name="p2", bufs=2) as p2, \
             tc.tile_pool(name="p2ps", bufs=1, space="PSUM") as p2ps, \
             tc.tile_pool(name="p2tr", bufs=1, space="PSUM") as p2tr:
            cosb = p2.tile([128, NQT, DR], F32, tag="cosb", bufs=1)
            sinb = p2.tile([128, NQT, DR], F32, tag="sinb", bufs=1)
            for q in range(NQT):
                nc.gpsimd.dma_start(
                    out=cosb[:, q, :],
                    in_=cos_v[bass.ds(r0 + q * 128, 128), :])
                nc.gpsimd.dma_start(
                    out=sinb[:, q, :],
                    in_=sin_v[bass.ds(r0 + q * 128, 128), :])
            qnw = p2.tile([128, QLR], F32, tag="qnw", bufs=1)
            nc.gpsimd.dma_start(out=qnw, in_=_vb(gf, F32_OFF["q_norm_w"], QLR))
            gcat_w = p2.tile([128, NT, 2 * IH], BF16, tag="gcat", bufs=1)
            igh_r = igh_v.rearrange("(c p) n -> p c n", p=128)
            igl_r = igl_v.rearrange("(c p) n -> p c n", p=128)
            for c in range(NT):
                nc.gpsimd.dma_start(out=gcat_w[:, c, 0:IH], in_=igh_r[:, c, :])
                nc.gpsimd.dma_start(out=gcat_w[:, c, IH:], in_=igl_r[:, c, :])
            ps_qr = [p2ps.tile([128, 512], F32, tag=f"ps_qr{q}{i}",
                               name=f"ps_qr{q}{i}")
                     for q in range(NQT) for i in range(2)]
            ps_g = [p2ps.tile([128, 2 * IH], F32, tag=f"ps_g{q}",
                              name=f"ps_g{q}") for q in range(NQT)]
            for f in range(NT):
                wqah_f = p2w.tile([128, QLR], BF16, tag="wqah_f")
                nc.gpsimd.dma_start(out=wqah_f,
                                    in_=wqah_v[f * 128:(f + 1) * 128, :])
                wqal_f = p2w.tile([128, QLR], BF16, tag="wqal_f")
                nc.gpsimd.dma_start(out=wqal_f,
                                    in_=wqal_v[f * 128:(f + 1) * 128, :])
                xq_hi = p2w.tile([128, NB], BF16, tag="xq_hi", bufs=3)
                nc.gpsimd.dma_start(
                    out=xq_hi,
                    in_=xTh_v[f * 128:(f + 1) * 128, bass.ds(r0, NB)])
                xq_lo = p2w.tile([128, NB], BF16, tag="xq_lo", bufs=3)
                nc.gpsimd.dma_start(
                    out=xq_lo,
                    in_=xTl_v[f * 128:(f + 1) * 128, bass.ds(r0, NB)])
                st, sp = (f == 0), (f == NT - 1)
                for q in range(NQT):
                    lhs_hi = xq_hi[:, q * 128:(q + 1) * 128]
                    lhs_lo = xq_lo[:, q * 128:(q + 1) * 128]
                    for i in range(2):
                        cols = slice(i * 512, (i + 1) * 512)
                        nc.tensor.matmul(ps_qr[2 * q + i], lhs_hi,
                                         wqah_f[:, cols], start=st, stop=False)
                        nc.tensor.matmul(ps_qr[2 * q + i], lhs_hi,
                                         wqal_f[:, cols], start=False,
                                         stop=False)
                        nc.tensor.matmul(ps_qr[2 * q + i], lhs_lo,
                                         wqah_f[:, cols], start=False, stop=sp)
                    nc.tensor.matmul(ps_g[q][:, 0:2 * IH], lhs_hi,
                                     gcat_w[:, f, :], start=st, stop=False)
                    nc.tensor.matmul(ps_g[q][:, 0:IH], lhs_lo,
                                     gcat_w[:, f, 0:IH], start=False, stop=sp)
            qrT_hi = p2.tile([128, 8, NB], BF16, tag="qrT_hi", bufs=1)
            qrT_lo = p2.tile([128, 8, NB], BF16, tag="qrT_lo", bufs=1)
            gate_sb = p2.tile([128, NQT, IH], F32, tag="gate_sb", bufs=1)
            for q in range(NQT):
                qr_sb = p2.tile([128, QLR], F32, tag="qr_sb")
                _rmsnorm_from_psum(nc, p2, qr_sb,
                                   [ps_qr[2 * q], ps_qr[2 * q + 1]], qnw, QLR)
                nc.scalar.copy(out=gate_sb[:, q, :], in_=ps_g[q][:, 0:IH])
                nc.vector.tensor_add(gate_sb[:, q, :], gate_sb[:, q, :],
                                     ps_g[q][:, IH:2 * IH])
                nc.vector.tensor_scalar(out=gate_sb[:, q, :],
                                        in0=gate_sb[:, q, :],
                                        scalar1=SCALE_GATE * SCALE_IDX,
                                        scalar2=None,
                                        op0=mybir.AluOpType.mult)
                qr_hi, qr_lo = _split(nc, p2, qr_sb, QLR, "qr")
                for ch in range(8):
                    cols = slice(ch * 128, (ch + 1) * 128)
                    ptr = p2tr.tile([128, 128], BF16, tag="ptr2")
                    nc.tensor.transpose(ptr, qr_hi[:, cols], ident)
                    nc.scalar.copy(out=qrT_hi[:, ch, q * 128:(q + 1) * 128],
                                   in_=ptr)
                    ptr = p2tr.tile([128, 128], BF16, tag="ptr2")
                    nc.tensor.transpose(ptr, qr_lo[:, cols], ident)
                    nc.scalar.copy(out=qrT_lo[:, ch, q * 128:(q + 1) * 128],
                                   in_=ptr)
            # q projection per MLA head (bf16)
            wqbr = wqb_v.rearrange("(c p) n -> p c n", p=128)
            for h in range(H):
                wqb_h = p2w.tile([128, 8, DN + DR], BF16, tag="wqb_h")
                for c in range(8):
                    nc.gpsimd.dma_start(
                        out=wqb_h[:, c, :],
                        in_=wqbr[:, c, h * (DN + DR):(h + 1) * (DN + DR)])
                for q in range(NQT):
                    ps_q = p2ps.tile([128, DN + DR], F32, tag="ps_q")
                    for ch in range(8):
                        nc.tensor.matmul(
                            ps_q, qrT_hi[:, ch, q * 128:(q + 1) * 128],
                            wqb_h[:, ch, :],
                            start=(ch == 0), stop=(ch == 7))
                    qn_bf = p2.tile([128, DN], BF16, tag="qn_bf")
                    nc.vector.tensor_scalar(out=qn_bf, in0=ps_q[:, 0:DN],
                                            scalar1=SCALE_MLA, scalar2=None,
                                            op0=mybir.AluOpType.mult)
                    qp32 = p2.tile([128, DR], F32, tag="qp32")
                    _rope_int(nc, qp32, ps_q[:, DN:],
                              cosb[:, q, :], sinb[:, q, :])
                    qp_bf = p2.tile([128, DR], BF16, tag="qp_bf")
                    nc.vector.tensor_scalar(out=qp_bf, in0=qp32,
                                            scalar1=SCALE_MLA, scalar2=None,
                                            op0=mybir.AluOpType.mult)
                    ptr = p2tr.tile([128, 128], BF16, tag="ptr2")
                    nc.tensor.transpose(ptr, qn_bf, ident)
                    nc.scalar.copy(out=qTn[:, h, q * 128:(q + 1) * 128],
                                   in_=ptr)
                    ptr = p2tr.tile([128, 128], BF16, tag="ptr2")
                    nc.tensor.transpose(ptr[:64, :], qp_bf, ident)
                    nc.scalar.copy(out=qTp[:, h, q * 128:(q + 1) * 128],
                                   in_=ptr[:64, :])
            # indexer q heads: 3-pass hi/lo, rope, * gate, split, transpose
            iwqbh_r = iwqbh_v.rearrange("(c p) n -> p c n", p=128)
            iwqbl_r = iwqbl_v.rearrange("(c p) n -> p c n", p=128)
            for ih in range(IH):
                wiq_cat = p2w.tile([128, 8, 2 * IHD], BF16, tag="wiq_cat")
                for c in range(8):
                    nc.gpsimd.dma_start(
                        out=wiq_cat[:, c, 0:IHD],
                        in_=iwqbh_r[:, c, ih * IHD:(ih + 1) * IHD])
                    nc.gpsimd.dma_start(
                        out=wiq_cat[:, c, IHD:],
                        in_=iwqbl_r[:, c, ih * IHD:(ih + 1) * IHD])
                for q in range(NQT):
                    ps_qc = p2ps.tile([128, 2 * IHD], F32, tag="ps_q")
                    for ch in range(8):
                        nc.tensor.matmul(
                            ps_qc[:, 0:2 * IHD],
                            qrT_hi[:, ch, q * 128:(q + 1) * 128],
                            wiq_cat[:, ch, :],
                            start=(ch == 0), stop=False)
                        nc.tensor.matmul(
                            ps_qc[:, 0:IHD],
                            qrT_lo[:, ch, q * 128:(q + 1) * 128],
                            wiq_cat[:, ch, 0:IHD],
                            start=False, stop=(ch == 7))
                    qi32 = p2.tile([128, IHD], F32, tag="qi32")
                    nc.scalar.copy(out=qi32, in_=ps_qc[:, 0:IHD])
                    nc.vector.tensor_add(qi32, qi32, ps_qc[:, IHD:2 * IHD])
                    qi_r = p2.tile([128, IHD], F32, tag="qi_r")
                    _rope_ni(nc, qi_r, qi32, cosb[:, q, :], sinb[:, q, :])
                    nc.vector.tensor_scalar(out=qi_r, in0=qi_r,
                                            scalar1=gate_sb[:, q, ih:ih + 1],
                                            scalar2=None,
                                            op0=mybir.AluOpType.mult)
                    qi_hi, qi_lo = _split(nc, p2, qi_r, IHD, "qi")
                    ptr = p2tr.tile([128, 128], BF16, tag="ptr2")
                    nc.tensor.transpose(ptr[:64, :], qi_hi, ident)
                    nc.scalar.copy(out=qiT_hi[:, ih, q * 128:(q + 1) * 128],
                                   in_=ptr[:64, :])
                    ptr = p2tr.tile([128, 128], BF16, tag="ptr2")
                    nc.tensor.transpose(ptr[:64, :], qi_lo, ident)
                    nc.scalar.copy(out=qiT_lo[:, ih, q * 128:(q + 1) * 128],
                                   in_=ptr[:64, :])
            if dbg:
                nc.gpsimd.dma_start(out=d_qrT[:, 0:8 * NB],
                                    in_=qrT_hi.rearrange("p a b -> p (a b)"))
                nc.gpsimd.dma_start(out=d_qrT[:, 8 * NB:],
                                    in_=qrT_lo.rearrange("p a b -> p (a b)"))
                nc.gpsimd.dma_start(out=d_gate,
                                    in_=gate_sb.rearrange("p a b -> p (a b)"))
                nc.gpsimd.dma_start(out=d_qiT[:, 0:IH * NB],
                                    in_=qiT_hi.rearrange("p a b -> p (a b)"))
                nc.gpsimd.dma_start(out=d_qiT[:, IH * NB:],
                                    in_=qiT_lo.rearrange("p a b -> p (a b)"))

        # ---------------- P3: index scores + EXACT top-k ----------------
        maskNEG = mid.tile([128, NQT, S], F32)
        with tc.tile_pool(name="p3", bufs=1) as p3, \
             tc.tile_pool(name="p3ps", bufs=4, space="PSUM") as p3ps:
            for q in range(NQT):
                cm = p3.tile([128, S], F32, tag="cm")
                if causal:
                    # cmask = (col > row) * NEG
                    rq = p3.tile([128, 1], F32, tag="rq")
                    nc.vector.tensor_scalar(out=rq, in0=rowid,
                                            scalar1=float(q * 128),
                                            scalar2=None,
                                            op0=mybir.AluOpType.add)
                    nc.vector.tensor_scalar(out=cm, in0=iota_sb,
                                            scalar1=rq, scalar2=NEG,
                                            op0=mybir.AluOpType.is_gt,
                                            op1=mybir.AluOpType.mult)
                else:
                    nc.gpsimd.dma_start(
                        out=cm, in_=amask_d[q * 128:(q + 1) * 128, :])
                isc = p3.tile([128, S], F32, tag="isc")
                for kc in range(4):
                    cols = slice(kc * 512, (kc + 1) * 512)
                    ps = p3ps.tile([128, 512], F32, tag="ps_isc")
                    for ih in range(IH):
                        qcols = slice(q * 128, (q + 1) * 128)
                        nc.tensor.matmul(ps, qiT_hi[:, ih, qcols],
                                         kiT_hi[:, cols],
                                         start=(ih == 0), stop=False)
                        nc.tensor.matmul(ps, qiT_hi[:, ih, qcols],
                                         kiT_lo[:, cols],
                                         start=False, stop=False)
                        nc.tensor.matmul(ps, qiT_lo[:, ih, qcols],
                                         kiT_hi[:, cols],
                                         start=False, stop=(ih == IH - 1))
                    nc.vector.tensor_add(isc[:, cols], ps, cm[:, cols])
                # clamp; masked cols sit at -200 (amask re-kills them later)
                nc.vector.tensor_scalar(out=isc, in0=isc, scalar1=-200.0,
                                        scalar2=None, op0=mybir.AluOpType.max)
                # exact top-256 threshold: 32 rounds of top-8 + replace
                scr = p3.tile([128, S], F32, tag="sel_scr")
                nc.vector.tensor_copy(scr, isc)
                mx = p3.tile([128, 8], F32, tag="mx")
                for r in range(TOPK // 8):
                    nc.vector.max(out=mx, in_=scr)
                    if r < TOPK // 8 - 1:
                        nc.vector.match_replace(out=scr, in_to_replace=mx,
                                                in_values=scr, imm_value=-3e9)
                nc.vector.tensor_scalar(out=maskNEG[:, q, :], in0=isc,
                                        scalar1=mx[:, 7:8], scalar2=NEG,
                                        op0=mybir.AluOpType.is_lt,
                                        op1=mybir.AluOpType.mult)
                nc.vector.tensor_add(maskNEG[:, q, :], maskNEG[:, q, :], cm)
                if dbg:
                    nc.gpsimd.dma_start(out=d_thr[:, q:q + 1], in_=mx[:, 7:8])

        if dbg:
            nc.gpsimd.dma_start(out=d_mask,
                                in_=maskNEG.rearrange("p a b -> p (a b)"))

        # ---------------- P4: sparse MLA attention per head ----------------
        out_hT = mid.tile([128, H, NB], BF16)
        with tc.tile_pool(name="p4w", bufs=2) as p4w, \
             tc.tile_pool(name="p4k", bufs=2) as p4k, \
             tc.tile_pool(name="p4p", bufs=2) as p4p, \
             tc.tile_pool(name="p4ps", bufs=2, space="PSUM") as p4ps, \
             tc.tile_pool(name="p4po", bufs=2, space="PSUM") as p4po:
            wbr = wkvb_v.rearrange("(c p) n -> p c n", p=128)
            for h in range(H):
                wb_k = p4w.tile([128, 4, DN], BF16, tag="wb_k")
                wb_v = p4w.tile([128, 4, DV], BF16, tag="wb_v")
                for c in range(4):
                    nc.gpsimd.dma_start(
                        out=wb_k[:, c, :],
                        in_=wbr[:, c, h * (DN + DV):h * (DN + DV) + DN])
                    nc.gpsimd.dma_start(
                        out=wb_v[:, c, :],
                        in_=wbr[:, c, h * (DN + DV) + DN:(h + 1) * (DN + DV)])
                knT = p4k.tile([128, S], BF16, tag="knT")
                for kc in range(4):
                    ps = p4ps.tile([128, 512], F32, tag="ps_kn")
                    for c in range(4):
                        nc.tensor.matmul(
                            ps, wb_k[:, c, :],
                            ckvT[:, c, kc * 512:(kc + 1) * 512],
                            start=(c == 0), stop=(c == 3))
                    nc.scalar.copy(out=knT[:, kc * 512:(kc + 1) * 512], in_=ps)
                v_sb = p4k.tile([128, NT, DV], BF16, tag="v_sb")
                for kt in range(NT):
                    ps = p4ps.tile([128, DV], F32, tag="ps_v")
                    for c in range(4):
                        nc.tensor.matmul(
                            ps,
                            ckvT[:, c, kt * 128:(kt + 1) * 128],
                            wb_v[:, c, :],
                            start=(c == 0), stop=(c == 3))
                    nc.scalar.copy(out=v_sb[:, kt, :], in_=ps)
                ps_o = p4po.tile([128, NB], F32, tag="ps_o")
                for q in range(NQT):
                    probs = p4p.tile([128, S], F32, tag="probs", bufs=1)
                    for kc in range(4):
                        cols = slice(kc * 512, (kc + 1) * 512)
                        ps = p4ps.tile([128, 512], F32, tag="ps_s")
                        nc.tensor.matmul(
                            ps, qTn[:, h, q * 128:(q + 1) * 128],
                            knT[:, cols], start=True, stop=False)
                        nc.tensor.matmul(
                            ps, qTp[:, h, q * 128:(q + 1) * 128],
                            kpeT[:, cols], start=False, stop=True)
                        nc.vector.tensor_add(probs[:, cols], ps,
                                             maskNEG[:, q, cols])
                    den = p4p.tile([128, 2], F32, tag="den")
                    nc.scalar.activation(out=probs, in_=probs,
                                         func=mybir.ActivationFunctionType.Exp,
                                         accum_out=den[:, 0:1])
                    nc.vector.reciprocal(out=den[:, 1:2], in_=den[:, 0:1])
                    pb = p4p.tile([128, S], BF16, tag="pb")
                    nc.vector.tensor_scalar(out=pb, in0=probs,
                                            scalar1=den[:, 1:2], scalar2=None,
                                            op0=mybir.AluOpType.mult)
                    pT = p4p.tile([128, NT, 128], BF16, tag="pT", bufs=1)
                    for kt in range(NT):
                        nc.scalar.dma_start_transpose(
                            out=pT[:, kt, :],
                            in_=pb[:, kt * 128:(kt + 1) * 128])
                    for kt in range(NT):
                        nc.tensor.matmul(
                            ps_o[:, q * 128:(q + 1) * 128],
                            v_sb[:, kt, :], pT[:, kt, :],
                            start=(kt == 0), stop=(kt == NT - 1))
                nc.scalar.copy(out=out_hT[:, h, :], in_=ps_o)

        if dbg:
            nc.gpsimd.dma_start(out=d_ohT,
                                in_=out_hT.rearrange("p a b -> p (a b)"))

        # ---------------- P5: output projection ----------------
        with tc.tile_pool(name="p5w", bufs=3) as p5w, \
             tc.tile_pool(name="p5", bufs=3) as p5, \
             tc.tile_pool(name="p5ps", bufs=4, space="PSUM") as p5ps:
            wor = wo_v.rearrange("(hh p) n -> p hh n", p=128)
            for g in range(NT):
                wo_g = p5w.tile([128, H, 128], BF16, tag="wo_g")
                for c in range(H):
                    nc.gpsimd.dma_start(
                        out=wo_g[:, c, :],
                        in_=wor[:, c, g * 128:(g + 1) * 128])
                ps = p5ps.tile([128, NB], F32, tag="ps_w")
                for h in range(H):
                    nc.tensor.matmul(ps, wo_g[:, h, :],
                                     out_hT[:, h, :],
                                     start=(h == 0), stop=(h == H - 1))
                ot = p5.tile([128, NB], FP16, tag="ot")
                nc.scalar.copy(out=ot, in_=ps)
                nc.gpsimd.dma_start(out=outT[g * 128:(g + 1) * 128, :], in_=ot)

        mid.release()
        consts.release()
        dram.release()
    nc.compile()
    return nc


_NC_CACHE = {}


def _get_nc(causal=True):
    if causal not in _NC_CACHE:
        _NC_CACHE[causal] = build_nc(causal)
    return _NC_CACHE[causal]


def _split_np(a):
    hi = a.astype(ml_dtypes.bfloat16)
    lo = (a - hi.astype(np.float32)).astype(ml_dtypes.bfloat16)
    return hi, lo


def _is_causal(am):
    s = am.shape[-1]
    r = np.arange(s, dtype=np.int64)
    causal = np.where(r[:, None] >= r[None, :], np.float32(0.0),
                      np.float32(NEG))
    return np.array_equal(am.reshape(s, s), causal)


def make_core_inputs(x, cos, sin, attn_mask, wq_a, q_norm_w, wq_b, wkv_a,
                     kv_norm_w, wkv_b, wo, idx_wq_b, idx_wk, idx_knorm_w,
                     idx_knorm_b, idx_gate):
    causal = _is_causal(np.asarray(attn_mask, np.float32))
    F32_OFF, F32_TOT = _f32_layout(causal)

    blob_bf = np.zeros(BF_TOT, ml_dtypes.bfloat16)

    def put_bf(name, arr):
        o = BF_OFF[name]
        blob_bf[o:o + arr.size] = np.ascontiguousarray(arr).reshape(-1)

    xT = np.ascontiguousarray(x[0].astype(np.float32).T)
    xh, xl = _split_np(xT)
    put_bf("xT_hi", xh)
    put_bf("xT_lo", xl)
    wh, wl = _split_np(np.asarray(wq_a, np.float32))
    put_bf("wqa_hi", wh)
    put_bf("wqa_lo", wl)
    put_bf("wq_b", np.asarray(wq_b, np.float32).astype(ml_dtypes.bfloat16))
    put_bf("wkv_a", np.asarray(wkv_a, np.float32).astype(ml_dtypes.bfloat16))
    put_bf("wkv_b", np.asarray(wkv_b, np.float32).astype(ml_dtypes.bfloat16))
    put_bf("wo", np.asarray(wo, np.float32).astype(ml_dtypes.bfloat16))
    ih_, il_ = _split_np(np.asarray(idx_wq_b, np.float32))
    put_bf("iwqb_hi", ih_)
    put_bf("iwqb_lo", il_)
    kh, kl = _split_np(np.asarray(idx_wk, np.float32))
    put_bf("iwk_hi", kh)
    put_bf("iwk_lo", kl)
    gh, gl = _split_np(np.asarray(idx_gate, np.float32))
    put_bf("igate_hi", gh)
    put_bf("igate_lo", gl)
    put_bf("ident", np.eye(128, dtype=np.float32))

    blob_f32 = np.zeros(F32_TOT, np.float32)

    def put_f(name, arr):
        o = F32_OFF[name]
        blob_f32[o:o + arr.size] = np.ascontiguousarray(
            arr, np.float32).reshape(-1)

    put_f("cos", cos[0])
    put_f("sin", sin[0])
    put_f("q_norm_w", q_norm_w)
    put_f("kv_norm_w", kv_norm_w)
    put_f("idx_knorm_w", idx_knorm_w)
    put_f("idx_knorm_b", idx_knorm_b)
    put_f("iota", np.arange(S, dtype=np.float32))
    rows = (np.arange(NCORES)[:, None] * NB
            + np.arange(128)[None, :]).astype(np.float32)
    put_f("rows", rows)

    lb, lf = BF_TOT // NCORES, F32_TOT // NCORES
    maps = []
    am = np.ascontiguousarray(attn_mask[0, 0], np.float32)
    for c in range(NCORES):
        m = {
            "shard_bf": np.ascontiguousarray(blob_bf[c * lb:(c + 1) * lb]),
            "shard_f32": np.ascontiguousarray(blob_f32[c * lf:(c + 1) * lf]),
        }
        if not causal:
            m["amask_rows"] = np.ascontiguousarray(
                am[c * NB:(c + 1) * NB])
        maps.append(m)
    return maps, causal


def kernel(x, cos, sin, attn_mask, wq_a, q_norm_w, wq_b, wkv_a, kv_norm_w,
           wkv_b, wo, idx_wq_b, idx_wk, idx_knorm_w, idx_knorm_b, idx_gate):
    from concourse.bass_utils import run_bass_kernel_spmd
    maps, causal = make_core_inputs(
        x, cos, sin, attn_mask, wq_a, q_norm_w, wq_b, wkv_a, kv_norm_w,
        wkv_b, wo, idx_wq_b, idx_wk, idx_knorm_w, idx_knorm_b, idx_gate)
    nc = _get_nc(causal)
    res = run_bass_kernel_spmd(nc, maps, list(range(NCORES)))
    outs = [np.asarray(r["outT"]).astype(np.float32).T for r in res.results]
    out = np.concatenate(outs, axis=0)[None]                   # [1, S, HID]
    return out.astype(np.float32)


# revision 13
# speedup vs baseline: 10.6481x; 1.0194x over previous
"""DSA sparse MLA attention kernel for TRN2, 8 NeuronCores.

Transfer-optimized SPMD design. The host->device tunnel moves ~52 MB/s,
so every input byte is shipped exactly ONCE: each core receives a 1/8
slice of two packed blobs (bf16 + f32) and the cores AllGather them
on-device (HBM-to-HBM over on-chip links, ~GB/ms). Per-core query-block
slices are carved out of the gathered blobs at runtime with
partition_id()-based dynamic DMA offsets, so no per-core host tensors
are needed at all.

Precision plan (harness gate: rel_err < 2e-2; this lands ~6e-3):
 - Indexer path (x, wq_a, idx_*) is selection-critical: tensors are
   shipped as hi/lo bf16 pairs (same bytes as f32) and matmuls use a
   3-pass hi/lo bf16 emulation (~1e-5 rel, 4x faster than fp32r which
   is only ~1e-3 accurate).
 - Top-256 selection is EXACT: 32 rounds of vector.max + match_replace
   give the true 256th-largest index score per row.
 - Attention path (wq_b, wkv_a, wkv_b, wo, q/k/v, probs) is plain bf16.
 - Output is fp16 (halves the donated-zeros upload + fetch).

Sharding: sequence-parallel over query rows; core c owns rows
[256c, 256(c+1)). KV/indexer-key expansion over all 2048 keys is
replicated (compute is ~free vs transfer).
"""

import numpy as np
import ml_dtypes

import jax

# Persistent XLA compilation cache: run_bass_kernel_spmd re-jits (and
# would re-run the walrus NEFF compile, ~1s) on every call; the disk
# cache turns that into a deserialize+load.
jax.config.update("jax_compilation_cache_dir", "/tmp/jax_cc_cache")
jax.config.update("jax_persistent_cache_min_compile_time_secs", 0.0)
jax.config.update("jax_persistent_cache_min_entry_size_bytes", 0)

import concourse.bass as bass
import concourse.bacc as bacc
import concourse.mybir as mybir
from concourse.tile import TileContext

F32 = mybir.dt.float32
BF16 = mybir.dt.bfloat16
FP16 = mybir.dt.float16

S, HID = 2048, 2048
H, DN, DR, DV = 16, 128, 64, 128
QLR, KVLR = 1024, 512
IH, IHD, TOPK = 8, 64, 256
NEG = -1e9
NB = 256            # query rows per core
NCORES = 8
NT = S // 128       # 16 token tiles
NQT = NB // 128     # 2 query tiles per core
SCALE_MLA = float((DN + DR) ** -0.5)
SCALE_IDX = float(IHD ** -0.5)
SCALE_GATE = float(IH ** -0.5)
ALIGN = 512         # element alignment for blob entries

BF_LAYOUT = [
    ("xT_hi", (HID, S)), ("xT_lo", (HID, S)),
    ("wqa_hi", (HID, QLR)), ("wqa_lo", (HID, QLR)),
    ("wq_b", (QLR, H * (DN + DR))),
    ("wkv_a", (HID, KVLR + DR)),
    ("wkv_b", (KVLR, H * (DN + DV))),
    ("wo", (H * DV, HID)),
    ("iwqb_hi", (QLR, IH * IHD)), ("iwqb_lo", (QLR, IH * IHD)),
    ("iwk_hi", (HID, IHD)), ("iwk_lo", (HID, IHD)),
    ("igate_hi", (HID, IH)), ("igate_lo", (HID, IH)),
    ("ident", (128, 128)),
]

F32_LAYOUT_BASE = [
    ("cos", (S, DR)), ("sin", (S, DR)),
    ("q_norm_w", (1, QLR)), ("kv_norm_w", (1, KVLR)),
    ("idx_knorm_w", (1, IHD)), ("idx_knorm_b", (1, IHD)),
    ("iota", (1, S)),
    ("rows", (NCORES * 128, 1)),
]


def _mk_layout(entries):
    offs, off = {}, 0
    for name, shape in entries:
        offs[name] = off
        n = int(np.prod(shape))
        off += (n + ALIGN - 1) // ALIGN * ALIGN
    tot = (off + NCORES * ALIGN - 1) // (NCORES * ALIGN) * (NCORES * ALIGN)
    return offs, tot


def _f32_layout(causal):
    return _mk_layout(list(F32_LAYOUT_BASE))


BF_OFF, BF_TOT = _mk_layout(BF_LAYOUT)
LB = BF_TOT // NCORES


def _v(blob1d, off, r, c):
    """[r, c] row-major view at element offset off of a 1-D DRAM AP."""
    return blob1d[off:off + r * c].rearrange("(r c) -> r c", c=c)


def _vb(blob1d, off, n, parts=128):
    """Partition-broadcast view [parts, n] of n elements at offset off."""
    return bass.AP(tensor=blob1d.tensor, offset=blob1d.offset + off,
                   ap=[[0, parts], [1, n]])


def _rmsnorm_from_psum(nc, pool, out_sb, psums, wb, d, eps=1e-6):
    """out_sb[p, d] = psum * rsqrt(mean(psum^2)+eps) * w."""
    ssq = pool.tile([128, len(psums)], F32)
    for i, ps in enumerate(psums):
        w = ps.shape[-1]
        scr = pool.tile([128, 512], F32, tag="rms_scr")
        nc.scalar.activation(out=scr[:, :w], in_=ps,
                             func=mybir.ActivationFunctionType.Square,
                             accum_out=ssq[:, i:i + 1])
    tot = pool.tile([128, 1], F32)
    if len(psums) == 1:
        nc.vector.tensor_scalar(out=tot, in0=ssq, scalar1=1.0 / d,
                                scalar2=eps, op0=mybir.AluOpType.mult,
                                op1=mybir.AluOpType.add)
    else:
        nc.vector.tensor_reduce(out=tot, in_=ssq, axis=mybir.AxisListType.X,
                                op=mybir.AluOpType.add)
        nc.vector.tensor_scalar(out=tot, in0=tot, scalar1=1.0 / d,
                                scalar2=eps, op0=mybir.AluOpType.mult,
                                op1=mybir.AluOpType.add)
    nc.scalar.activation(out=tot, in_=tot,
                         func=mybir.ActivationFunctionType.Sqrt)
    rinv = pool.tile([128, 1], F32)
    nc.vector.reciprocal(out=rinv, in_=tot)
    off = 0
    for ps in psums:
        w = ps.shape[-1]
        nc.vector.tensor_scalar(out=out_sb[:, off:off + w], in0=ps,
                                scalar1=rinv, scalar2=None,
                                op0=mybir.AluOpType.mult)
        off += w
    nc.vector.tensor_mul(out_sb[:, :d], out_sb[:, :d], wb[:, :d])


def _rope_int(nc, out, in_, cos, sin):
    """Interleaved (GPT-J) rope, token-major [128, 64] -> out[128, 64]."""
    xp = in_.rearrange("p (a b) -> p a b", b=2)
    op = out.rearrange("p (a b) -> p a b", b=2)
    c, s = cos[:, 0:32], sin[:, 0:32]
    x1, x2 = xp[:, :, 0], xp[:, :, 1]
    nc.vector.tensor_mul(op[:, :, 0], x1, c)
    nc.vector.tensor_mul(op[:, :, 1], x2, c)
    t = nc._rope_scr.tile([128, 32], F32, tag="rope_t")
    nc.vector.tensor_mul(t, x2, s)
    nc.vector.tensor_sub(op[:, :, 0], op[:, :, 0], t)
    nc.vector.tensor_mul(t, x1, s)
    nc.vector.tensor_add(op[:, :, 1], op[:, :, 1], t)


def _rope_ni(nc, out, in_, cos, sin):
    """Non-interleaved (rotate_half) rope, [128, 64]."""
    x1, x2 = in_[:, 0:32], in_[:, 32:64]
    c1, c2 = cos[:, 0:32], cos[:, 32:64]
    s1, s2 = sin[:, 0:32], sin[:, 32:64]
    nc.vector.tensor_mul(out[:, 0:32], x1, c1)
    nc.vector.tensor_mul(out[:, 32:64], x2, c2)
    t = nc._rope_scr.tile([128, 32], F32, tag="rope_t")
    nc.vector.tensor_mul(t, x2, s1)
    nc.vector.tensor_sub(out[:, 0:32], out[:, 0:32], t)
    nc.vector.tensor_mul(t, x1, s2)
    nc.vector.tensor_add(out[:, 32:64], out[:, 32:64], t)


def _split(nc, pool, src_f32, n, tag):
    """f32 [128, n] -> (hi bf16, lo bf16) with hi+lo ~= src."""
    hi = pool.tile([128, n], BF16, tag=tag + "_hi")
    nc.vector.tensor_copy(hi, src_f32)
    hi32 = pool.tile([128, n], F32, tag=tag + "_h32")
    nc.vector.tensor_copy(hi32, hi)
    lo32 = pool.tile([128, n], F32, tag=tag + "_l32")
    nc.vector.tensor_sub(lo32, src_f32, hi32)
    lo = pool.tile([128, n], BF16, tag=tag + "_lo")
    nc.vector.tensor_copy(lo, lo32)
    return hi, lo


class _Bacc(bacc.Bacc):
    """Bacc with memoized BIR serialization: run_bass_kernel_spmd re-lowers
    (and re-serializes the ~9 MB BIR) on every call; the module is frozen
    after compile(), so the bytes are reusable."""
    _json_cache = None

    def to_json_bytes(self):
        if self._json_cache is None:
            self._json_cache = super().to_json_bytes()
        return self._json_cache


def build_nc(causal=True, dbg=False):
    F32_OFF, F32_TOT = _f32_layout(causal)
    lf = F32_TOT // NCORES

    nc = _Bacc("TRN2", target_bir_lowering=False, debug=False)
    shard_bf = nc.dram_tensor("shard_bf", [LB], BF16, kind="ExternalInput").ap()
    shard_f32 = nc.dram_tensor("shard_f32", [lf], F32,
                               kind="ExternalInput").ap()
    outT = nc.dram_tensor("outT", [HID, NB], FP16, kind="ExternalOutput").ap()
    amask_d = None
    if not causal:
        amask_d = nc.dram_tensor("amask_rows", [NB, S], F32,
                                 kind="ExternalInput").ap()
    if dbg:
        d_ckvT = nc.dram_tensor("d_ckvT", [128, 4 * S], BF16,
                                kind="ExternalOutput").ap()
        d_kpeT = nc.dram_tensor("d_kpeT", [64, S], BF16,
                                kind="ExternalOutput").ap()
        d_kiT = nc.dram_tensor("d_kiT", [64, 2 * S], BF16,
                               kind="ExternalOutput").ap()
        d_qrT = nc.dram_tensor("d_qrT", [128, 2 * 8 * NB], BF16,
                               kind="ExternalOutput").ap()
        d_gate = nc.dram_tensor("d_gate", [128, NQT * IH], F32,
                                kind="ExternalOutput").ap()
        d_mask = nc.dram_tensor("d_mask", [128, NQT * S], F32,
                                kind="ExternalOutput").ap()
        d_thr = nc.dram_tensor("d_thr", [128, NQT], F32,
                               kind="ExternalOutput").ap()
        d_qiT = nc.dram_tensor("d_qiT", [64, 2 * IH * NB], BF16,
                               kind="ExternalOutput").ap()
        d_ohT = nc.dram_tensor("d_ohT", [128, H * NB], BF16,
                               kind="ExternalOutput").ap()

    with TileContext(nc) as tc:
        pid = nc.partition_id()
        r0 = pid * NB

        dram = tc.alloc_tile_pool(name="dram", bufs=1, space="DRAM")
        bounce_bf = dram.tile([LB], BF16)
        bounce_f32 = dram.tile([lf], F32)
        gath_bf = dram.tile([NCORES, LB], BF16)
        gath_f32 = dram.tile([NCORES, lf], F32)
        nc.gpsimd.dma_start(out=bounce_bf, in_=shard_bf)
        nc.gpsimd.dma_start(out=bounce_f32, in_=shard_f32)
        nc.gpsimd.collective_compute(
            "AllGather", mybir.AluOpType.bypass,
            replica_groups=[list(range(NCORES))],
            ins=[bounce_bf[:].opt()], outs=[gath_bf[:].opt()])
        nc.gpsimd.collective_compute(
            "AllGather", mybir.AluOpType.bypass,
            replica_groups=[list(range(NCORES))],
            ins=[bounce_f32[:].opt()], outs=[gath_f32[:].opt()])
        gb = gath_bf.rearrange("a b -> (a b)")
        gf = gath_f32.rearrange("a b -> (a b)")

        xTh_v = _v(gb, BF_OFF["xT_hi"], HID, S)
        xTl_v = _v(gb, BF_OFF["xT_lo"], HID, S)
        wqah_v = _v(gb, BF_OFF["wqa_hi"], HID, QLR)
        wqal_v = _v(gb, BF_OFF["wqa_lo"], HID, QLR)
        wqb_v = _v(gb, BF_OFF["wq_b"], QLR, H * (DN + DR))
        wkva_v = _v(gb, BF_OFF["wkv_a"], HID, KVLR + DR)
        wkvb_v = _v(gb, BF_OFF["wkv_b"], KVLR, H * (DN + DV))
        wo_v = _v(gb, BF_OFF["wo"], H * DV, HID)
        iwqbh_v = _v(gb, BF_OFF["iwqb_hi"], QLR, IH * IHD)
        iwqbl_v = _v(gb, BF_OFF["iwqb_lo"], QLR, IH * IHD)
        iwkh_v = _v(gb, BF_OFF["iwk_hi"], HID, IHD)
        iwkl_v = _v(gb, BF_OFF["iwk_lo"], HID, IHD)
        igh_v = _v(gb, BF_OFF["igate_hi"], HID, IH)
        igl_v = _v(gb, BF_OFF["igate_lo"], HID, IH)
        ident_v = _v(gb, BF_OFF["ident"], 128, 128)
        cos_v = _v(gf, F32_OFF["cos"], S, DR)
        sin_v = _v(gf, F32_OFF["sin"], S, DR)
        rows_v = _v(gf, F32_OFF["rows"], NCORES * 128, 1)

        consts = tc.alloc_tile_pool(name="consts", bufs=1)
        nc._rope_scr = consts

        ident = consts.tile([128, 128], BF16)
        nc.gpsimd.dma_start(out=ident, in_=ident_v)
        kvnw = consts.tile([128, KVLR], F32)
        nc.gpsimd.dma_start(out=kvnw, in_=_vb(gf, F32_OFF["kv_norm_w"], KVLR))
        knw = consts.tile([128, IHD], F32)
        nc.gpsimd.dma_start(out=knw, in_=_vb(gf, F32_OFF["idx_knorm_w"], IHD))
        knb = consts.tile([128, IHD], F32)
        nc.gpsimd.dma_start(out=knb, in_=_vb(gf, F32_OFF["idx_knorm_b"], IHD))
        iota_sb = consts.tile([128, S], F32)
        nc.gpsimd.dma_start(out=iota_sb, in_=_vb(gf, F32_OFF["iota"], S))
        rowid = consts.tile([128, 1], F32)
        nc.gpsimd.dma_start(out=rowid, in_=rows_v[bass.ds(pid * 128, 128), :])

        ckvT = consts.tile([128, 4, S], BF16)      # [ckv_chunk, 4, tok]
        kpeT = consts.tile([64, S], BF16)
        kiT_hi = consts.tile([64, S], BF16)
        kiT_lo = consts.tile([64, S], BF16)

        # ---------------- P1: KV / indexer-key expansion ----------------
        with tc.tile_pool(name="p1w", bufs=1) as p1w, \
             tc.tile_pool(name="p1", bufs=3) as p1, \
             tc.tile_pool(name="p1ps", bufs=2, space="PSUM") as p1ps, \
             tc.tile_pool(name="p1tr", bufs=2, space="PSUM") as p1tr:
            cos_t = p1w.tile([128, NT, DR], F32)
            sin_t = p1w.tile([128, NT, DR], F32)
            cr = cos_v.rearrange("(t p) d -> p t d", p=128)
            sr = sin_v.rearrange("(t p) d -> p t d", p=128)
            wkva_sb = p1w.tile([128, NT, KVLR], BF16)
            wr = wkva_v.rearrange("(c p) n -> p c n", p=128)
            # wcat: [k_pe cols of wkv_a | iwk_hi | iwk_lo]
            wcat = p1w.tile([128, NT, DR + 2 * IHD], BF16)
            ikh = iwkh_v.rearrange("(c p) n -> p c n", p=128)
            ikl = iwkl_v.rearrange("(c p) n -> p c n", p=128)
            for c in range(NT):
                nc.gpsimd.dma_start(out=cos_t[:, c, :], in_=cr[:, c, :])
                nc.gpsimd.dma_start(out=sin_t[:, c, :], in_=sr[:, c, :])
                nc.gpsimd.dma_start(out=wkva_sb[:, c, :],
                                    in_=wr[:, c, 0:KVLR])
                nc.gpsimd.dma_start(out=wcat[:, c, 0:DR],
                                    in_=wr[:, c, KVLR:])
                nc.gpsimd.dma_start(out=wcat[:, c, DR:DR + IHD],
                                    in_=ikh[:, c, :])
                nc.gpsimd.dma_start(out=wcat[:, c, DR + IHD:],
                                    in_=ikl[:, c, :])

            xrh = xTh_v.rearrange("(c p) (u q) -> p c u q", p=128, q=128)
            xrl = xTl_v.rearrange("(c p) (u q) -> p c u q", p=128, q=128)
            for t in range(NT):
                xt_hi = p1.tile([128, NT, 128], BF16, tag="xt_hi")
                xt_lo = p1.tile([128, NT, 128], BF16, tag="xt_lo")
                for c in range(NT):
                    nc.gpsimd.dma_start(out=xt_hi[:, c, :], in_=xrh[:, c, t, :])
                    nc.gpsimd.dma_start(out=xt_lo[:, c, :], in_=xrl[:, c, t, :])
                ps_kv = p1ps.tile([128, KVLR], F32, tag="ps_kv")
                ps_x = p1ps.tile([128, DR + 2 * IHD], F32, tag="ps_x")
                ps_kl = p1ps.tile([128, IHD], F32, tag="ps_kl")
                for f in range(NT):
                    st, sp = (f == 0), (f == NT - 1)
                    nc.tensor.matmul(ps_kv, xt_hi[:, f, :],
                                     wkva_sb[:, f, :], start=st, stop=sp)
                    nc.tensor.matmul(ps_x, xt_hi[:, f, :],
                                     wcat[:, f, :], start=st, stop=sp)
                    nc.tensor.matmul(ps_kl, xt_lo[:, f, :],
                                     wcat[:, f, DR:DR + IHD],
                                     start=st, stop=sp)
                # ckv rmsnorm -> bf16 -> transpose into ckvT
                ckv_sb = p1.tile([128, KVLR], F32, tag="ckv_sb")
                _rmsnorm_from_psum(nc, p1, ckv_sb, [ps_kv], kvnw, KVLR)
                ckv_bf = p1.tile([128, KVLR], BF16, tag="ckv_bf")
                nc.vector.tensor_copy(ckv_bf, ckv_sb)
                for ch in range(4):
                    ptr = p1tr.tile([128, 128], BF16, tag="ptr")
                    nc.tensor.transpose(ptr, ckv_bf[:, ch * 128:(ch + 1) * 128],
                                        ident)
                    nc.scalar.copy(out=ckvT[:, ch, t * 128:(t + 1) * 128],
                                   in_=ptr)
                # k_pe rope -> bf16 -> transpose into kpeT
                pe_sb = p1.tile([128, DR], F32, tag="pe_sb")
                _rope_int(nc, pe_sb, ps_x[:, 0:DR],
                          cos_t[:, t, :], sin_t[:, t, :])
                pe_bf = p1.tile([128, DR], BF16, tag="pe_bf")
                nc.vector.tensor_copy(pe_bf, pe_sb)
                ptr = p1tr.tile([128, 128], BF16, tag="ptr")
                nc.tensor.transpose(ptr[:64, :], pe_bf, ident)
                nc.scalar.copy(out=kpeT[:, t * 128:(t + 1) * 128],
                               in_=ptr[:64, :])
                # ki = layernorm(3-pass sum) + rope -> split -> transpose
                ki32 = p1.tile([128, IHD], F32, tag="ki32")
                nc.scalar.copy(out=ki32, in_=ps_x[:, DR:DR + IHD])
                nc.vector.tensor_add(ki32, ki32, ps_x[:, DR + IHD:])
                nc.vector.tensor_add(ki32, ki32, ps_kl)
                s1 = p1.tile([128, 2], F32, tag="ki_s")
                scr = p1.tile([128, IHD], F32, tag="ki_scr")
                nc.scalar.activation(out=scr, in_=ki32,
                                     func=mybir.ActivationFunctionType.Copy,
                                     accum_out=s1[:, 0:1])
                nc.scalar.activation(out=scr, in_=ki32,
                                     func=mybir.ActivationFunctionType.Square,
                                     accum_out=s1[:, 1:2])
                mom = p1.tile([128, 4], F32, tag="ki_m")
                nc.vector.tensor_scalar(out=mom[:, 0:1], in0=s1[:, 0:1],
                                        scalar1=1.0 / IHD, scalar2=None,
                                        op0=mybir.AluOpType.mult)
                nc.vector.tensor_scalar(out=mom[:, 1:2], in0=s1[:, 1:2],
                                        scalar1=1.0 / IHD, scalar2=None,
                                        op0=mybir.AluOpType.mult)
                nc.vector.tensor_mul(mom[:, 2:3], mom[:, 0:1], mom[:, 0:1])
                nc.vector.tensor_sub(mom[:, 2:3], mom[:, 1:2], mom[:, 2:3])
                nc.vector.tensor_scalar(out=mom[:, 2:3], in0=mom[:, 2:3],
                                        scalar1=1e-5, scalar2=None,
                                        op0=mybir.AluOpType.add)
                nc.scalar.activation(out=mom[:, 2:3], in_=mom[:, 2:3],
                                     func=mybir.ActivationFunctionType.Sqrt)
                nc.vector.reciprocal(out=mom[:, 3:4], in_=mom[:, 2:3])
                ki_n = p1.tile([128, IHD], F32, tag="ki_n")
                nc.vector.tensor_scalar(out=ki_n, in0=ki32,
                                        scalar1=mom[:, 0:1],
                                        scalar2=mom[:, 3:4],
                                        op0=mybir.AluOpType.subtract,
                                        op1=mybir.AluOpType.mult)
                nc.vector.tensor_mul(ki_n, ki_n, knw)
                nc.vector.tensor_add(ki_n, ki_n, knb)
                ki_r = p1.tile([128, IHD], F32, tag="ki_r")
                _rope_ni(nc, ki_r, ki_n, cos_t[:, t, :], sin_t[:, t, :])
                ki_hi, ki_lo = _split(nc, p1, ki_r, IHD, "ki")
                ptr = p1tr.tile([128, 128], BF16, tag="ptr")
                nc.tensor.transpose(ptr[:64, :], ki_hi, ident)
                nc.scalar.copy(out=kiT_hi[:, t * 128:(t + 1) * 128],
                               in_=ptr[:64, :])
                ptr = p1tr.tile([128, 128], BF16, tag="ptr")
                nc.tensor.transpose(ptr[:64, :], ki_lo, ident)
                nc.scalar.copy(out=kiT_lo[:, t * 128:(t + 1) * 128],
                               in_=ptr[:64, :])

        if dbg:
            nc.gpsimd.dma_start(out=d_ckvT,
                                in_=ckvT.rearrange("p a b -> p (a b)"))
            nc.gpsimd.dma_start(out=d_kpeT, in_=kpeT)
            nc.gpsimd.dma_start(out=d_kiT[:, 0:S], in_=kiT_hi)
            nc.gpsimd.dma_start(out=d_kiT[:, S:], in_=kiT_lo)

        # ---------------- P2: query-block projections ----------------
        mid = tc.alloc_tile_pool(name="mid", bufs=1)
        qTn = mid.tile([128, H, NB], BF16)       # nope part, feature-major
        qTp = mid.tile([64, H, NB], BF16)        # rope part
        qiT_hi = mid.tile([64, IH, NB], BF16)
        qiT_lo = mid.tile([64, IH, NB], BF16)

        with tc.tile_pool(name="p2w", bufs=2) as p2w, \
             tc.tile_pool(name="p2", bufs=2) as p2, \
             tc.tile_pool(name="p2ps", bufs=1, space="PSUM") as p2ps, \
             tc.tile_pool(name="p2tr", bufs=1, space="PSUM") as p2tr:
            cosb = p2.tile([128, NQT, DR], F32, tag="cosb", bufs=1)
            sinb = p2.tile([128, NQT, DR], F32, tag="sinb", bufs=1)
            for q in range(NQT):
                nc.gpsimd.dma_start(
                    out=cosb[:, q, :],
                    in_=cos_v[bass.ds(r0 + q * 128, 128), :])
                nc.gpsimd.dma_start(
                    out=sinb[:, q, :],
                    in_=sin_v[bass.ds(r0 + q * 128, 128), :])
            qnw = p2.tile([128, QLR], F32, tag="qnw", bufs=1)
            nc.gpsimd.dma_start(out=qnw, in_=_vb(gf, F32_OFF["q_norm_w"], QLR))
            gcat_w = p2.tile([128, NT, 2 * IH], BF16, tag="gcat", bufs=1)
            igh_r = igh_v.rearrange("(c p) n -> p c n", p=128)
            igl_r = igl_v.rearrange("(c p) n -> p c n", p=128)
            for c in range(NT):
                nc.gpsimd.dma_start(out=gcat_w[:, c, 0:IH], in_=igh_r[:, c, :])
                nc.gpsimd.dma_start(out=gcat_w[:, c, IH:], in_=igl_r[:, c, :])
            ps_qr = [p2ps.tile([128, 512], F32, tag=f"ps_qr{q}{i}",
                               name=f"ps_qr{q}{i}")
                     for q in range(NQT) for i in range(2)]
            ps_g = [p2ps.tile([128, 2 * IH], F32, tag=f"ps_g{q}",
                              name=f"ps_g{q}") for q in range(NQT)]
            for f in range(NT):
                wqah_f = p2w.tile([128, QLR], BF16, tag="wqah_f")
                nc.gpsimd.dma_start(out=wqah_f,
                                    in_=wqah_v[f * 128:(f + 1) * 128, :])
                wqal_f = p2w.tile([128, QLR], BF16, tag="wqal_f")
                nc.gpsimd.dma_start(out=wqal_f,
                                    in_=wqal_v[f * 128:(f + 1) * 128, :])
                xq_hi = p2w.tile([128, NB], BF16, tag="xq_hi", bufs=3)
                nc.gpsimd.dma_start(
                    out=xq_hi,
                    in_=xTh_v[f * 128:(f + 1) * 128, bass.ds(r0, NB)])
                xq_lo = p2w.tile([128, NB], BF16, tag="xq_lo", bufs=3)
                nc.gpsimd.dma_start(
                    out=xq_lo,
                    in_=xTl_v[f * 128:(f + 1) * 128, bass.ds(r0, NB)])
                st, sp = (f == 0), (f == NT - 1)
                for q in range(NQT):
                    lhs_hi = xq_hi[:, q * 128:(q + 1) * 128]
                    lhs_lo = xq_lo[:, q * 128:(q + 1) * 128]
                    for i in range(2):
                        cols = slice(i * 512, (i + 1) * 512)
                        nc.tensor.matmul(ps_qr[2 * q + i], lhs_hi,
                                         wqah_f[:, cols], start=st, stop=False)
                        nc.tensor.matmul(ps_qr[2 * q + i], lhs_hi,
                                         wqal_f[:, cols], start=False,
                                         stop=False)
                        nc.tensor.matmul(ps_qr[2 * q + i], lhs_lo,
                                         wqah_f[:, cols], start=False, stop=sp)
                    nc.tensor.matmul(ps_g[q][:, 0:2 * IH], lhs_hi,
                                     gcat_w[:, f, :], start=st, stop=False)
                    nc.tensor.matmul(ps_g[q][:, 0:IH], lhs_lo,
                                     gcat_w[:, f, 0:IH], start=False, stop=sp)
            qrT_hi = p2.tile([128, 8, NB], BF16, tag="qrT_hi", bufs=1)
            qrT_lo = p2.tile([128, 8, NB], BF16, tag="qrT_lo", bufs=1)
            gate_sb = p2.tile([128, NQT, IH], F32, tag="gate_sb", bufs=1)
            for q in range(NQT):
                qr_sb = p2.tile([128, QLR], F32, tag="qr_sb")
                _rmsnorm_from_psum(nc, p2, qr_sb,
                                   [ps_qr[2 * q], ps_qr[2 * q + 1]], qnw, QLR)
                nc.scalar.copy(out=gate_sb[:, q, :], in_=ps_g[q][:, 0:IH])
                nc.vector.tensor_add(gate_sb[:, q, :], gate_sb[:, q, :],
                                     ps_g[q][:, IH:2 * IH])
                nc.vector.tensor_scalar(out=gate_sb[:, q, :],
                                        in0=gate_sb[:, q, :],
                                        scalar1=SCALE_GATE * SCALE_IDX,
                                        scalar2=None,
                                        op0=mybir.AluOpType.mult)
                qr_hi, qr_lo = _split(nc, p2, qr_sb, QLR, "qr")
                for ch in range(8):
                    cols = slice(ch * 128, (ch + 1) * 128)
                    ptr = p2tr.tile([128, 128], BF16, tag="ptr2")
                    nc.tensor.transpose(ptr, qr_hi[:, cols], ident)
                    nc.scalar.copy(out=qrT_hi[:, ch, q * 128:(q + 1) * 128],
                                   in_=ptr)
                    ptr = p2tr.tile([128, 128], BF16, tag="ptr2")
                    nc.tensor.transpose(ptr, qr_lo[:, cols], ident)
                    nc.scalar.copy(out=qrT_lo[:, ch, q * 128:(q + 1) * 128],
                                   in_=ptr)
            # q projection per MLA head (bf16)
            wqbr = wqb_v.rearrange("(c p) n -> p c n", p=128)
            for h in range(H):
                wqb_h = p2w.tile([128, 8, DN + DR], BF16, tag="wqb_h")
                for c in range(8):
                    nc.gpsimd.dma_start(
                        out=wqb_h[:, c, :],
                        in_=wqbr[:, c, h * (DN + DR):(h + 1) * (DN + DR)])
                for q in range(NQT):
                    ps_q = p2ps.tile([128, DN + DR], F32, tag="ps_q")
                    for ch in range(8):
                        nc.tensor.matmul(
                            ps_q, qrT_hi[:, ch, q * 128:(q + 1) * 128],
                            wqb_h[:, ch, :],
                            start=(ch == 0), stop=(ch == 7))
                    qn_bf = p2.tile([128, DN], BF16, tag="qn_bf")
                    nc.vector.tensor_scalar(out=qn_bf, in0=ps_q[:, 0:DN],
                                            scalar1=SCALE_MLA, scalar2=None,
                                            op0=mybir.AluOpType.mult)
                    qp32 = p2.tile([128, DR], F32, tag="qp32")
                    _rope_int(nc, qp32, ps_q[:, DN:],
                              cosb[:, q, :], sinb[:, q, :])
                    qp_bf = p2.tile([128, DR], BF16, tag="qp_bf")
                    nc.vector.tensor_scalar(out=qp_bf, in0=qp32,
                                            scalar1=SCALE_MLA, scalar2=None,
                                            op0=mybir.AluOpType.mult)
                    ptr = p2tr.tile([128, 128], BF16, tag="ptr2")
                    nc.tensor.transpose(ptr, qn_bf, ident)
                    nc.scalar.copy(out=qTn[:, h, q * 128:(q + 1) * 128],
                                   in_=ptr)
                    ptr = p2tr.tile([128, 128], BF16, tag="ptr2")
                    nc.tensor.transpose(ptr[:64, :], qp_bf, ident)
                    nc.scalar.copy(out=qTp[:, h, q * 128:(q + 1) * 128],
                                   in_=ptr[:64, :])
            # indexer q heads: 3-pass hi/lo, rope, * gate, split, transpose
            iwqbh_r = iwqbh_v.rearrange("(c p) n -> p c n", p=128)
            iwqbl_r = iwqbl_v.rearrange("(c p) n -> p c n", p=128)
            for ih in range(IH):
                wiq_cat = p2w.tile([128, 8, 2 * IHD], BF16, tag="wiq_cat")
                for c in range(8):
                    nc.gpsimd.dma_start(
                        out=wiq_cat[:, c, 0:IHD],
                        in_=iwqbh_r[:, c, ih * IHD:(ih + 1) * IHD])
                    nc.gpsimd.dma_start(
                        out=wiq_cat[:, c, IHD:],
                        in_=iwqbl_r[:, c, ih * IHD:(ih + 1) * IHD])
                for q in range(NQT):
                    ps_qc = p2ps.tile([128, 2 * IHD], F32, tag="ps_q")
                    for ch in range(8):
                        nc.tensor.matmul(
                            ps_qc[:, 0:2 * IHD],
                            qrT_hi[:, ch, q * 128:(q + 1) * 128],
                            wiq_cat[:, ch, :],
                            start=(ch == 0), stop=False)
                        nc.tensor.matmul(
                            ps_qc[:, 0:IHD],
                            qrT_lo[:, ch, q * 128:(q + 1) * 128],
                            wiq_cat[:, ch, 0:IHD],
                            start=False, stop=(ch == 7))
                    qi32 = p2.tile([128, IHD], F32, tag="qi32")
                    nc.scalar.copy(out=qi32, in_=ps_qc[:, 0:IHD])
                    nc.vector.tensor_add(qi32, qi32, ps_qc[:, IHD:2 * IHD])
                    qi_r = p2.tile([128, IHD], F32, tag="qi_r")
                    _rope_ni(nc, qi_r, qi32, cosb[:, q, :], sinb[:, q, :])
                    nc.vector.tensor_scalar(out=qi_r, in0=qi_r,
                                            scalar1=gate_sb[:, q, ih:ih + 1],
                                            scalar2=None,
                                            op0=mybir.AluOpType.mult)
                    qi_hi, qi_lo = _split(nc, p2, qi_r, IHD, "qi")
                    ptr = p2tr.tile([128, 128], BF16, tag="ptr2")
                    nc.tensor.transpose(ptr[:64, :], qi_hi, ident)
                    nc.scalar.copy(out=qiT_hi[:, ih, q * 128:(q + 1) * 128],
                                   in_=ptr[:64, :])
                    ptr = p2tr.tile([128, 128], BF16, tag="ptr2")
                    nc.tensor.transpose(ptr[:64, :], qi_lo, ident)
                    nc.scalar.copy(out=qiT_lo[:, ih, q * 128:(q + 1) * 128],
                                   in_=ptr[:64, :])
            if dbg:
                nc.gpsimd.dma_start(out=d_qrT[:, 0:8 * NB],
                                    in_=qrT_hi.rearrange("p a b -> p (a b)"))
                nc.gpsimd.dma_start(out=d_qrT[:, 8 * NB:],
                                    in_=qrT_lo.rearrange("p a b -> p (a b)"))
                nc.gpsimd.dma_start(out=d_gate,
                                    in_=gate_sb.rearrange("p a b -> p (a b)"))
                nc.gpsimd.dma_start(out=d_qiT[:, 0:IH * NB],
                                    in_=qiT_hi.rearrange("p a b -> p (a b)"))
                nc.gpsimd.dma_start(out=d_qiT[:, IH * NB:],
                                    in_=qiT_lo.rearrange("p a b -> p (a b)"))

        # ---------------- P3: index scores + EXACT top-k ----------------
        maskNEG = mid.tile([128, NQT, S], F32)
        with tc.tile_pool(name="p3", bufs=1) as p3, \
             tc.tile_pool(name="p3ps", bufs=4, space="PSUM") as p3ps:
            for q in range(NQT):
                cm = p3.tile([128, S], F32, tag="cm")
                if causal:
                    # cmask = (col > row) * NEG
                    rq = p3.tile([128, 1], F32, tag="rq")
                    nc.vector.tensor_scalar(out=rq, in0=rowid,
                                            scalar1=float(q * 128),
                                            scalar2=None,
                                            op0=mybir.AluOpType.add)
                    nc.vector.tensor_scalar(out=cm, in0=iota_sb,
                                            scalar1=rq, scalar2=NEG,
                                            op0=mybir.AluOpType.is_gt,
                                            op1=mybir.AluOpType.mult)
                else:
                    nc.gpsimd.dma_start(
                        out=cm, in_=amask_d[q * 128:(q + 1) * 128, :])
                isc = p3.tile([128, S], F32, tag="isc")
                for kc in range(4):
                    cols = slice(kc * 512, (kc + 1) * 512)
                    ps = p3ps.tile([128, 512], F32, tag="ps_isc")
                    for ih in range(IH):
                        qcols = slice(q * 128, (q + 1) * 128)
                        nc.tensor.matmul(ps, qiT_hi[:, ih, qcols],
                                         kiT_hi[:, cols],
                                         start=(ih == 0), stop=False)
                        nc.tensor.matmul(ps, qiT_hi[:, ih, qcols],
                                         kiT_lo[:, cols],
                                         start=False, stop=False)
                        nc.tensor.matmul(ps, qiT_lo[:, ih, qcols],
                                         kiT_hi[:, cols],
                                         start=False, stop=(ih == IH - 1))
                    nc.vector.tensor_add(isc[:, cols], ps, cm[:, cols])
                # clamp; masked cols sit at -200 (amask re-kills them later)
                nc.vector.tensor_scalar(out=isc, in0=isc, scalar1=-200.0,
                                        scalar2=None, op0=mybir.AluOpType.max)
                # exact top-256 threshold: 32 rounds of top-8 + replace
                scr = p3.tile([128, S], F32, tag="sel_scr")
                nc.vector.tensor_copy(scr, isc)
                mx = p3.tile([128, 8], F32, tag="mx")
                for r in range(TOPK // 8):
                    nc.vector.max(out=mx, in_=scr)
                    if r < TOPK // 8 - 1:
                        nc.vector.match_replace(out=scr, in_to_replace=mx,
                                                in_values=scr, imm_value=-3e9)
                nc.vector.tensor_scalar(out=maskNEG[:, q, :], in0=isc,
                                        scalar1=mx[:, 7:8], scalar2=NEG,
                                        op0=mybir.AluOpType.is_lt,
                                        op1=mybir.AluOpType.mult)
                nc.vector.tensor_add(maskNEG[:, q, :], maskNEG[:, q, :], cm)
                if dbg:
                    nc.gpsimd.dma_start(out=d_thr[:, q:q + 1], in_=mx[:, 7:8])

        if dbg:
            nc.gpsimd.dma_start(out=d_mask,
                                in_=maskNEG.rearrange("p a b -> p (a b)"))

        # ---------------- P4: sparse MLA attention per head ----------------
        out_hT = mid.tile([128, H, NB], BF16)
        with tc.tile_pool(name="p4w", bufs=2) as p4w, \
             tc.tile_pool(name="p4k", bufs=2) as p4k, \
             tc.tile_pool(name="p4p", bufs=2) as p4p, \
             tc.tile_pool(name="p4ps", bufs=2, space="PSUM") as p4ps, \
             tc.tile_pool(name="p4po", bufs=2, space="PSUM") as p4po:
            wbr = wkvb_v.rearrange("(c p) n -> p c n", p=128)
            for h in range(H):
                wb_k = p4w.tile([128, 4, DN], BF16, tag="wb_k")
                wb_v = p4w.tile([128, 4, DV], BF16, tag="wb_v")
                for c in range(4):
                    nc.gpsimd.dma_start(
                        out=wb_k[:, c, :],
                        in_=wbr[:, c, h * (DN + DV):h * (DN + DV) + DN])
                    nc.gpsimd.dma_start(
                        out=wb_v[:, c, :],
                        in_=wbr[:, c, h * (DN + DV) + DN:(h + 1) * (DN + DV)])
                knT = p4k.tile([128, S], BF16, tag="knT")
                for kc in range(4):
                    ps = p4ps.tile([128, 512], F32, tag="ps_kn")
                    for c in range(4):
                        nc.tensor.matmul(
                            ps, wb_k[:, c, :],
                            ckvT[:, c, kc * 512:(kc + 1) * 512],
                            start=(c == 0), stop=(c == 3))
                    nc.scalar.copy(out=knT[:, kc * 512:(kc + 1) * 512], in_=ps)
                v_sb = p4k.tile([128, NT, DV], BF16, tag="v_sb")
                for kt in range(NT):
                    ps = p4ps.tile([128, DV], F32, tag="ps_v")
                    for c in range(4):
                        nc.tensor.matmul(
                            ps,
                            ckvT[:, c, kt * 128:(kt + 1) * 128],
                            wb_v[:, c, :],
                            start=(c == 0), stop=(c == 3))
                    nc.scalar.copy(out=v_sb[:, kt, :], in_=ps)
                ps_o = p4po.tile([128, NB], F32, tag="ps_o")
                for q in range(NQT):
                    probs = p4p.tile([128, S], F32, tag="probs", bufs=1)
                    for kc in range(4):
                        cols = slice(kc * 512, (kc + 1) * 512)
                        ps = p4ps.tile([128, 512], F32, tag="ps_s")
                        nc.tensor.matmul(
                            ps, qTn[:, h, q * 128:(q + 1) * 128],
                            knT[:, cols], start=True, stop=False)
                        nc.tensor.matmul(
                            ps, qTp[:, h, q * 128:(q + 1) * 128],
                            kpeT[:, cols], start=False, stop=True)
                        nc.vector.tensor_add(probs[:, cols], ps,
                                             maskNEG[:, q, cols])
                    den = p4p.tile([128, 2], F32, tag="den")
                    nc.scalar.activation(out=probs, in_=probs,
                                         func=mybir.ActivationFunctionType.Exp,
                                         accum_out=den[:, 0:1])
                    nc.vector.reciprocal(out=den[:, 1:2], in_=den[:, 0:1])
                    pb = p4p.tile([128, S], BF16, tag="pb")
                    nc.vector.tensor_scalar(out=pb, in0=probs,
                                            scalar1=den[:, 1:2], scalar2=None,
                                            op0=mybir.AluOpType.mult)
                    pT = p4p.tile([128, NT, 128], BF16, tag="pT", bufs=1)
                    for kt in range(NT):
                        nc.scalar.dma_start_transpose(
                            out=pT[:, kt, :],
                            in_=pb[:, kt * 128:(kt + 1) * 128])
                    for kt in range(NT):
                        nc.tensor.matmul(
                            ps_o[:, q * 128:(q + 1) * 128],
                            v_sb[:, kt, :], pT[:, kt, :],
                            start=(kt == 0), stop=(kt == NT - 1))
                nc.scalar.copy(out=out_hT[:, h, :], in_=ps_o)

        if dbg:
            nc.gpsimd.dma_start(out=d_ohT,
                                in_=out_hT.rearrange("p a b -> p (a b)"))

        # ---------------- P5: output projection ----------------
        with tc.tile_pool(name="p5w", bufs=3) as p5w, \
             tc.tile_pool(name="p5", bufs=3) as p5, \
             tc.tile_pool(name="p5ps", bufs=4, space="PSUM") as p5ps:
            wor = wo_v.rearrange("(hh p) n -> p hh n", p=128)
            for g in range(NT):
                wo_g = p5w.tile([128, H, 128], BF16, tag="wo_g")
                for c in range(H):
                    nc.gpsimd.dma_start(
                        out=wo_g[:, c, :],
                        in_=wor[:, c, g * 128:(g + 1) * 128])
                ps = p5ps.tile([128, NB], F32, tag="ps_w")
                for h in range(H):
                    nc.tensor.matmul(ps, wo_g[:, h, :],
                                     out_hT[:, h, :],
                                     start=(h == 0), stop=(h == H - 1))
                ot = p5.tile([128, NB], FP16, tag="ot")
                nc.scalar.copy(out=ot, in_=ps)
                nc.gpsimd.dma_start(out=outT[g * 128:(g + 1) * 128, :], in_=ot)

        mid.release()
        consts.release()
        dram.release()
    nc.compile()
    return nc


_NC_CACHE = {}


def _get_nc(causal=True):
    if causal not in _NC_CACHE:
        _NC_CACHE[causal] = build_nc(causal)
    return _NC_CACHE[causal]


def _split_np(a):
    hi = a.astype(ml_dtypes.bfloat16)
    lo = (a - hi.astype(np.float32)).astype(ml_dtypes.bfloat16)
    return hi, lo


def _is_causal(am):
    s = am.shape[-1]
    r = np.arange(s, dtype=np.int64)
    causal = np.where(r[:, None] >= r[None, :], np.float32(0.0),
                      np.float32(NEG))
    return np.array_equal(am.reshape(s, s), causal)


def make_core_inputs(x, cos, sin, attn_mask, wq_a, q_norm_w, wq_b, wkv_a,
                     kv_norm_w, wkv_b, wo, idx_wq_b, idx_wk, idx_knorm_w,
                     idx_knorm_b, idx_gate):
    causal = _is_causal(np.asarray(attn_mask, np.float32))
    F32_OFF, F32_TOT = _f32_layout(causal)

    blob_bf = np.zeros(BF_TOT, ml_dtypes.bfloat16)

    def put_bf(name, arr):
        o = BF_OFF[name]
        blob_bf[o:o + arr.size] = np.ascontiguousarray(arr).reshape(-1)

    xT = np.ascontiguousarray(x[0].astype(np.float32).T)
    xh, xl = _split_np(xT)
    put_bf("xT_hi", xh)
    put_bf("xT_lo", xl)
    wh, wl = _split_np(np.asarray(wq_a, np.float32))
    put_bf("wqa_hi", wh)
    put_bf("wqa_lo", wl)
    put_bf("wq_b", np.asarray(wq_b, np.float32).astype(ml_dtypes.bfloat16))
    put_bf("wkv_a", np.asarray(wkv_a, np.float32).astype(ml_dtypes.bfloat16))
    put_bf("wkv_b", np.asarray(wkv_b, np.float32).astype(ml_dtypes.bfloat16))
    put_bf("wo", np.asarray(wo, np.float32).astype(ml_dtypes.bfloat16))
    ih_, il_ = _split_np(np.asarray(idx_wq_b, np.float32))
    put_bf("iwqb_hi", ih_)
    put_bf("iwqb_lo", il_)
    kh, kl = _split_np(np.asarray(idx_wk, np.float32))
    put_bf("iwk_hi", kh)
    put_bf("iwk_lo", kl)
    gh, gl = _split_np(np.asarray(idx_gate, np.float32))
    put_bf("igate_hi", gh)
    put_bf("igate_lo", gl)
    put_bf("ident", np.eye(128, dtype=np.float32))

    blob_f32 = np.zeros(F32_TOT, np.float32)

    def put_f(name, arr):
        o = F32_OFF[name]
        blob_f32[o:o + arr.size] = np.ascontiguousarray(
            arr, np.float32).reshape(-1)

    put_f("cos", cos[0])
    put_f("sin", sin[0])
    put_f("q_norm_w", q_norm_w)
    put_f("kv_norm_w", kv_norm_w)
    put_f("idx_knorm_w", idx_knorm_w)
    put_f("idx_knorm_b", idx_knorm_b)
    put_f("iota", np.arange(S, dtype=np.float32))
    rows = (np.arange(NCORES)[:, None] * NB
            + np.arange(128)[None, :]).astype(np.float32)
    put_f("rows", rows)

    lb, lf = BF_TOT // NCORES, F32_TOT // NCORES
    maps = []
    am = np.ascontiguousarray(attn_mask[0, 0], np.float32)
    for c in range(NCORES):
        m = {
            "shard_bf": np.ascontiguousarray(blob_bf[c * lb:(c + 1) * lb]),
            "shard_f32": np.ascontiguousarray(blob_f32[c * lf:(c + 1) * lf]),
        }
        if not causal:
            m["amask_rows"] = np.ascontiguousarray(
                am[c * NB:(c + 1) * NB])
        maps.append(m)
    return maps, causal


def kernel(x, cos, sin, attn_mask, wq_a, q_norm_w, wq_b, wkv_a, kv_norm_w,
           wkv_b, wo, idx_wq_b, idx_wk, idx_knorm_w, idx_knorm_b, idx_gate):
    from concourse.bass_utils import run_bass_kernel_spmd
    args = [np.asarray(a, np.float32) for a in (
        x, cos, sin, attn_mask, wq_a, q_norm_w, wq_b, wkv_a, kv_norm_w,
        wkv_b, wo, idx_wq_b, idx_wk, idx_knorm_w, idx_knorm_b, idx_gate)]
    maps, causal = make_core_inputs(*args)
    nc = _get_nc(causal)
    res = run_bass_kernel_spmd(nc, maps, list(range(NCORES)))
    outs = [np.asarray(r["outT"]).astype(np.float32).T for r in res.results]
    out = np.concatenate(outs, axis=0)[None]                   # [1, S, HID]
    return out.astype(np.float32)


# revision 14
# speedup vs baseline: 10.7746x; 1.0119x over previous
"""DSA sparse MLA attention kernel for TRN2, 8 NeuronCores.

Transfer-optimized SPMD design. The host->device tunnel moves ~52 MB/s,
so every input byte is shipped exactly ONCE: each core receives a 1/8
slice of two packed blobs (bf16 + f32) and the cores AllGather them
on-device (HBM-to-HBM over on-chip links, ~GB/ms). Per-core query-block
slices are carved out of the gathered blobs at runtime with
partition_id()-based dynamic DMA offsets, so no per-core host tensors
are needed at all.

Precision plan (harness gate: rel_err < 2e-2; this lands ~6e-3):
 - Indexer path (x, wq_a, idx_*) is selection-critical: tensors are
   shipped as hi/lo bf16 pairs (same bytes as f32) and matmuls use a
   3-pass hi/lo bf16 emulation (~1e-5 rel, 4x faster than fp32r which
   is only ~1e-3 accurate).
 - Top-256 selection is EXACT: 32 rounds of vector.max + match_replace
   give the true 256th-largest index score per row.
 - Attention path (wq_b, wkv_a, wkv_b, wo, q/k/v, probs) is plain bf16.
 - Output is fp16 (halves the donated-zeros upload + fetch).

Sharding: sequence-parallel over query rows; core c owns rows
[256c, 256(c+1)). KV/indexer-key expansion over all 2048 keys is
replicated (compute is ~free vs transfer).
"""

import numpy as np
import ml_dtypes

import jax

# Persistent XLA compilation cache: run_bass_kernel_spmd re-jits (and
# would re-run the walrus NEFF compile, ~1s) on every call; the disk
# cache turns that into a deserialize+load.
jax.config.update("jax_compilation_cache_dir", "/tmp/jax_cc_cache")
jax.config.update("jax_persistent_cache_min_compile_time_secs", 0.0)
jax.config.update("jax_persistent_cache_min_entry_size_bytes", 0)

import concourse.bass as bass
import concourse.bacc as bacc
import concourse.mybir as mybir
from concourse.tile import TileContext

F32 = mybir.dt.float32
BF16 = mybir.dt.bfloat16
FP16 = mybir.dt.float16

S, HID = 2048, 2048
H, DN, DR, DV = 16, 128, 64, 128
QLR, KVLR = 1024, 512
IH, IHD, TOPK = 8, 64, 256
NEG = -1e9
NB = 256            # query rows per core
NCORES = 8
NT = S // 128       # 16 token tiles
NQT = NB // 128     # 2 query tiles per core
SCALE_MLA = float((DN + DR) ** -0.5)
SCALE_IDX = float(IHD ** -0.5)
SCALE_GATE = float(IH ** -0.5)
ALIGN = 512         # element alignment for blob entries

BF_LAYOUT = [
    ("xT_hi", (HID, S)), ("xT_lo", (HID, S)),
    ("wqa_hi", (HID, QLR)), ("wqa_lo", (HID, QLR)),
    ("wq_b", (QLR, H * (DN + DR))),
    ("wkv_a", (HID, KVLR + DR)),
    ("wkv_b", (KVLR, H * (DN + DV))),
    ("wo", (H * DV, HID)),
    ("iwqb_hi", (QLR, IH * IHD)), ("iwqb_lo", (QLR, IH * IHD)),
    ("iwk_hi", (HID, IHD)), ("iwk_lo", (HID, IHD)),
    ("igate_hi", (HID, IH)), ("igate_lo", (HID, IH)),
    ("ident", (128, 128)),
]

F32_LAYOUT_BASE = [
    ("cos", (S, DR)), ("sin", (S, DR)),
    ("q_norm_w", (1, QLR)), ("kv_norm_w", (1, KVLR)),
    ("idx_knorm_w", (1, IHD)), ("idx_knorm_b", (1, IHD)),
    ("iota", (1, S)),
    ("rows", (NCORES * 128, 1)),
]


def _mk_layout(entries):
    offs, off = {}, 0
    for name, shape in entries:
        offs[name] = off
        n = int(np.prod(shape))
        off += (n + ALIGN - 1) // ALIGN * ALIGN
    tot = (off + NCORES * ALIGN - 1) // (NCORES * ALIGN) * (NCORES * ALIGN)
    return offs, tot


def _f32_layout(causal):
    return _mk_layout(list(F32_LAYOUT_BASE))


BF_OFF, BF_TOT = _mk_layout(BF_LAYOUT)
LB = BF_TOT // NCORES


def _v(blob1d, off, r, c):
    """[r, c] row-major view at element offset off of a 1-D DRAM AP."""
    return blob1d[off:off + r * c].rearrange("(r c) -> r c", c=c)


def _vb(blob1d, off, n, parts=128):
    """Partition-broadcast view [parts, n] of n elements at offset off."""
    return bass.AP(tensor=blob1d.tensor, offset=blob1d.offset + off,
                   ap=[[0, parts], [1, n]])


def _rmsnorm_from_psum(nc, pool, out_sb, psums, wb, d, eps=1e-6):
    """out_sb[p, d] = psum * rsqrt(mean(psum^2)+eps) * w."""
    ssq = pool.tile([128, len(psums)], F32)
    for i, ps in enumerate(psums):
        w = ps.shape[-1]
        scr = pool.tile([128, 512], F32, tag="rms_scr")
        nc.scalar.activation(out=scr[:, :w], in_=ps,
                             func=mybir.ActivationFunctionType.Square,
                             accum_out=ssq[:, i:i + 1])
    tot = pool.tile([128, 1], F32)
    if len(psums) == 1:
        nc.vector.tensor_scalar(out=tot, in0=ssq, scalar1=1.0 / d,
                                scalar2=eps, op0=mybir.AluOpType.mult,
                                op1=mybir.AluOpType.add)
    else:
        nc.vector.tensor_reduce(out=tot, in_=ssq, axis=mybir.AxisListType.X,
                                op=mybir.AluOpType.add)
        nc.vector.tensor_scalar(out=tot, in0=tot, scalar1=1.0 / d,
                                scalar2=eps, op0=mybir.AluOpType.mult,
                                op1=mybir.AluOpType.add)
    nc.scalar.activation(out=tot, in_=tot,
                         func=mybir.ActivationFunctionType.Sqrt)
    rinv = pool.tile([128, 1], F32)
    nc.vector.reciprocal(out=rinv, in_=tot)
    off = 0
    for ps in psums:
        w = ps.shape[-1]
        nc.vector.tensor_scalar(out=out_sb[:, off:off + w], in0=ps,
                                scalar1=rinv, scalar2=None,
                                op0=mybir.AluOpType.mult)
        off += w
    nc.vector.tensor_mul(out_sb[:, :d], out_sb[:, :d], wb[:, :d])


def _rope_int(nc, out, in_, cos, sin):
    """Interleaved (GPT-J) rope, token-major [128, 64] -> out[128, 64]."""
    xp = in_.rearrange("p (a b) -> p a b", b=2)
    op = out.rearrange("p (a b) -> p a b", b=2)
    c, s = cos[:, 0:32], sin[:, 0:32]
    x1, x2 = xp[:, :, 0], xp[:, :, 1]
    nc.vector.tensor_mul(op[:, :, 0], x1, c)
    nc.vector.tensor_mul(op[:, :, 1], x2, c)
    t = nc._rope_scr.tile([128, 32], F32, tag="rope_t")
    nc.vector.tensor_mul(t, x2, s)
    nc.vector.tensor_sub(op[:, :, 0], op[:, :, 0], t)
    nc.vector.tensor_mul(t, x1, s)
    nc.vector.tensor_add(op[:, :, 1], op[:, :, 1], t)


def _rope_ni(nc, out, in_, cos, sin):
    """Non-interleaved (rotate_half) rope, [128, 64]."""
    x1, x2 = in_[:, 0:32], in_[:, 32:64]
    c1, c2 = cos[:, 0:32], cos[:, 32:64]
    s1, s2 = sin[:, 0:32], sin[:, 32:64]
    nc.vector.tensor_mul(out[:, 0:32], x1, c1)
    nc.vector.tensor_mul(out[:, 32:64], x2, c2)
    t = nc._rope_scr.tile([128, 32], F32, tag="rope_t")
    nc.vector.tensor_mul(t, x2, s1)
    nc.vector.tensor_sub(out[:, 0:32], out[:, 0:32], t)
    nc.vector.tensor_mul(t, x1, s2)
    nc.vector.tensor_add(out[:, 32:64], out[:, 32:64], t)


def _split(nc, pool, src_f32, n, tag):
    """f32 [128, n] -> (hi bf16, lo bf16) with hi+lo ~= src."""
    hi = pool.tile([128, n], BF16, tag=tag + "_hi")
    nc.vector.tensor_copy(hi, src_f32)
    hi32 = pool.tile([128, n], F32, tag=tag + "_h32")
    nc.vector.tensor_copy(hi32, hi)
    lo32 = pool.tile([128, n], F32, tag=tag + "_l32")
    nc.vector.tensor_sub(lo32, src_f32, hi32)
    lo = pool.tile([128, n], BF16, tag=tag + "_lo")
    nc.vector.tensor_copy(lo, lo32)
    return hi, lo


class _Bacc(bacc.Bacc):
    """Bacc with memoized BIR serialization: run_bass_kernel_spmd re-lowers
    (and re-serializes the ~9 MB BIR) on every call; the module is frozen
    after compile(), so the bytes are reusable."""
    _json_cache = None

    def to_json_bytes(self):
        if self._json_cache is None:
            self._json_cache = super().to_json_bytes()
        return self._json_cache


def build_nc(causal=True, dbg=False):
    F32_OFF, F32_TOT = _f32_layout(causal)
    lf = F32_TOT // NCORES

    nc = _Bacc("TRN2", target_bir_lowering=False, debug=False)
    shard_bf = nc.dram_tensor("shard_bf", [LB], BF16, kind="ExternalInput").ap()
    shard_f32 = nc.dram_tensor("shard_f32", [lf], F32,
                               kind="ExternalInput").ap()
    outT = nc.dram_tensor("outT", [HID, NB], FP16, kind="ExternalOutput").ap()
    amask_d = None
    if not causal:
        amask_d = nc.dram_tensor("amask_rows", [NB, S], F32,
                                 kind="ExternalInput").ap()
    if dbg:
        d_ckvT = nc.dram_tensor("d_ckvT", [128, 4 * S], BF16,
                                kind="ExternalOutput").ap()
        d_kpeT = nc.dram_tensor("d_kpeT", [64, S], BF16,
                                kind="ExternalOutput").ap()
        d_kiT = nc.dram_tensor("d_kiT", [64, 2 * S], BF16,
                               kind="ExternalOutput").ap()
        d_qrT = nc.dram_tensor("d_qrT", [128, 2 * 8 * NB], BF16,
                               kind="ExternalOutput").ap()
        d_gate = nc.dram_tensor("d_gate", [128, NQT * IH], F32,
                                kind="ExternalOutput").ap()
        d_mask = nc.dram_tensor("d_mask", [128, NQT * S], F32,
                                kind="ExternalOutput").ap()
        d_thr = nc.dram_tensor("d_thr", [128, NQT], F32,
                               kind="ExternalOutput").ap()
        d_qiT = nc.dram_tensor("d_qiT", [64, 2 * IH * NB], BF16,
                               kind="ExternalOutput").ap()
        d_ohT = nc.dram_tensor("d_ohT", [128, H * NB], BF16,
                               kind="ExternalOutput").ap()

    with TileContext(nc) as tc:
        pid = nc.partition_id()
        r0 = pid * NB

        dram = tc.alloc_tile_pool(name="dram", bufs=1, space="DRAM")
        bounce_bf = dram.tile([LB], BF16)
        bounce_f32 = dram.tile([lf], F32)
        gath_bf = nc.dram_tensor("gath_bf", [NCORES, LB], BF16,
                                 kind="Internal", addr_space="Shared").ap()
        gath_f32 = nc.dram_tensor("gath_f32", [NCORES, lf], F32,
                                  kind="Internal", addr_space="Shared").ap()
        nc.gpsimd.dma_start(out=bounce_bf, in_=shard_bf)
        nc.gpsimd.dma_start(out=bounce_f32, in_=shard_f32)
        nc.gpsimd.collective_compute(
            "AllGather", mybir.AluOpType.bypass,
            replica_groups=[list(range(NCORES))],
            ins=[bounce_bf[:].opt()], outs=[gath_bf[:].opt()])
        nc.gpsimd.collective_compute(
            "AllGather", mybir.AluOpType.bypass,
            replica_groups=[list(range(NCORES))],
            ins=[bounce_f32[:].opt()], outs=[gath_f32[:].opt()])
        gb = gath_bf.rearrange("a b -> (a b)")
        gf = gath_f32.rearrange("a b -> (a b)")

        xTh_v = _v(gb, BF_OFF["xT_hi"], HID, S)
        xTl_v = _v(gb, BF_OFF["xT_lo"], HID, S)
        wqah_v = _v(gb, BF_OFF["wqa_hi"], HID, QLR)
        wqal_v = _v(gb, BF_OFF["wqa_lo"], HID, QLR)
        wqb_v = _v(gb, BF_OFF["wq_b"], QLR, H * (DN + DR))
        wkva_v = _v(gb, BF_OFF["wkv_a"], HID, KVLR + DR)
        wkvb_v = _v(gb, BF_OFF["wkv_b"], KVLR, H * (DN + DV))
        wo_v = _v(gb, BF_OFF["wo"], H * DV, HID)
        iwqbh_v = _v(gb, BF_OFF["iwqb_hi"], QLR, IH * IHD)
        iwqbl_v = _v(gb, BF_OFF["iwqb_lo"], QLR, IH * IHD)
        iwkh_v = _v(gb, BF_OFF["iwk_hi"], HID, IHD)
        iwkl_v = _v(gb, BF_OFF["iwk_lo"], HID, IHD)
        igh_v = _v(gb, BF_OFF["igate_hi"], HID, IH)
        igl_v = _v(gb, BF_OFF["igate_lo"], HID, IH)
        ident_v = _v(gb, BF_OFF["ident"], 128, 128)
        cos_v = _v(gf, F32_OFF["cos"], S, DR)
        sin_v = _v(gf, F32_OFF["sin"], S, DR)
        rows_v = _v(gf, F32_OFF["rows"], NCORES * 128, 1)

        consts = tc.alloc_tile_pool(name="consts", bufs=1)
        nc._rope_scr = consts

        ident = consts.tile([128, 128], BF16)
        nc.gpsimd.dma_start(out=ident, in_=ident_v)
        kvnw = consts.tile([128, KVLR], F32)
        nc.gpsimd.dma_start(out=kvnw, in_=_vb(gf, F32_OFF["kv_norm_w"], KVLR))
        knw = consts.tile([128, IHD], F32)
        nc.gpsimd.dma_start(out=knw, in_=_vb(gf, F32_OFF["idx_knorm_w"], IHD))
        knb = consts.tile([128, IHD], F32)
        nc.gpsimd.dma_start(out=knb, in_=_vb(gf, F32_OFF["idx_knorm_b"], IHD))
        iota_sb = consts.tile([128, S], F32)
        nc.gpsimd.dma_start(out=iota_sb, in_=_vb(gf, F32_OFF["iota"], S))
        rowid = consts.tile([128, 1], F32)
        nc.gpsimd.dma_start(out=rowid, in_=rows_v[bass.ds(pid * 128, 128), :])

        ckvT = consts.tile([128, 4, S], BF16)      # [ckv_chunk, 4, tok]
        kpeT = consts.tile([64, S], BF16)
        kiT_hi = consts.tile([64, S], BF16)
        kiT_lo = consts.tile([64, S], BF16)

        # ---------------- P1: KV / indexer-key expansion ----------------
        with tc.tile_pool(name="p1w", bufs=1) as p1w, \
             tc.tile_pool(name="p1", bufs=3) as p1, \
             tc.tile_pool(name="p1ps", bufs=2, space="PSUM") as p1ps, \
             tc.tile_pool(name="p1tr", bufs=2, space="PSUM") as p1tr:
            cos_t = p1w.tile([128, NT, DR], F32)
            sin_t = p1w.tile([128, NT, DR], F32)
            cr = cos_v.rearrange("(t p) d -> p t d", p=128)
            sr = sin_v.rearrange("(t p) d -> p t d", p=128)
            wkva_sb = p1w.tile([128, NT, KVLR], BF16)
            wr = wkva_v.rearrange("(c p) n -> p c n", p=128)
            # wcat: [k_pe cols of wkv_a | iwk_hi | iwk_lo]
            wcat = p1w.tile([128, NT, DR + 2 * IHD], BF16)
            ikh = iwkh_v.rearrange("(c p) n -> p c n", p=128)
            ikl = iwkl_v.rearrange("(c p) n -> p c n", p=128)
            for c in range(NT):
                nc.gpsimd.dma_start(out=cos_t[:, c, :], in_=cr[:, c, :])
                nc.gpsimd.dma_start(out=sin_t[:, c, :], in_=sr[:, c, :])
                nc.gpsimd.dma_start(out=wkva_sb[:, c, :],
                                    in_=wr[:, c, 0:KVLR])
                nc.gpsimd.dma_start(out=wcat[:, c, 0:DR],
                                    in_=wr[:, c, KVLR:])
                nc.gpsimd.dma_start(out=wcat[:, c, DR:DR + IHD],
                                    in_=ikh[:, c, :])
                nc.gpsimd.dma_start(out=wcat[:, c, DR + IHD:],
                                    in_=ikl[:, c, :])

            xrh = xTh_v.rearrange("(c p) (u q) -> p c u q", p=128, q=128)
            xrl = xTl_v.rearrange("(c p) (u q) -> p c u q", p=128, q=128)
            for t in range(NT):
                xt_hi = p1.tile([128, NT, 128], BF16, tag="xt_hi")
                xt_lo = p1.tile([128, NT, 128], BF16, tag="xt_lo")
                nc.gpsimd.dma_start(out=xt_hi, in_=xrh[:, :, t, :])
                nc.gpsimd.dma_start(out=xt_lo, in_=xrl[:, :, t, :])
                ps_kv = p1ps.tile([128, KVLR], F32, tag="ps_kv")
                ps_x = p1ps.tile([128, DR + 2 * IHD], F32, tag="ps_x")
                ps_kl = p1ps.tile([128, IHD], F32, tag="ps_kl")
                for f in range(NT):
                    st, sp = (f == 0), (f == NT - 1)
                    nc.tensor.matmul(ps_kv, xt_hi[:, f, :],
                                     wkva_sb[:, f, :], start=st, stop=sp)
                    nc.tensor.matmul(ps_x, xt_hi[:, f, :],
                                     wcat[:, f, :], start=st, stop=sp)
                    nc.tensor.matmul(ps_kl, xt_lo[:, f, :],
                                     wcat[:, f, DR:DR + IHD],
                                     start=st, stop=sp)
                # ckv rmsnorm -> bf16 -> transpose into ckvT
                ckv_sb = p1.tile([128, KVLR], F32, tag="ckv_sb")
                _rmsnorm_from_psum(nc, p1, ckv_sb, [ps_kv], kvnw, KVLR)
                ckv_bf = p1.tile([128, KVLR], BF16, tag="ckv_bf")
                nc.vector.tensor_copy(ckv_bf, ckv_sb)
                for ch in range(4):
                    ptr = p1tr.tile([128, 128], BF16, tag="ptr")
                    nc.tensor.transpose(ptr, ckv_bf[:, ch * 128:(ch + 1) * 128],
                                        ident)
                    nc.scalar.copy(out=ckvT[:, ch, t * 128:(t + 1) * 128],
                                   in_=ptr)
                # k_pe rope -> bf16 -> transpose into kpeT
                pe_sb = p1.tile([128, DR], F32, tag="pe_sb")
                _rope_int(nc, pe_sb, ps_x[:, 0:DR],
                          cos_t[:, t, :], sin_t[:, t, :])
                pe_bf = p1.tile([128, DR], BF16, tag="pe_bf")
                nc.vector.tensor_copy(pe_bf, pe_sb)
                ptr = p1tr.tile([128, 128], BF16, tag="ptr")
                nc.tensor.transpose(ptr[:64, :], pe_bf, ident)
                nc.scalar.copy(out=kpeT[:, t * 128:(t + 1) * 128],
                               in_=ptr[:64, :])
                # ki = layernorm(3-pass sum) + rope -> split -> transpose
                ki32 = p1.tile([128, IHD], F32, tag="ki32")
                nc.scalar.copy(out=ki32, in_=ps_x[:, DR:DR + IHD])
                nc.vector.tensor_add(ki32, ki32, ps_x[:, DR + IHD:])
                nc.vector.tensor_add(ki32, ki32, ps_kl)
                s1 = p1.tile([128, 2], F32, tag="ki_s")
                scr = p1.tile([128, IHD], F32, tag="ki_scr")
                nc.scalar.activation(out=scr, in_=ki32,
                                     func=mybir.ActivationFunctionType.Copy,
                                     accum_out=s1[:, 0:1])
                nc.scalar.activation(out=scr, in_=ki32,
                                     func=mybir.ActivationFunctionType.Square,
                                     accum_out=s1[:, 1:2])
                mom = p1.tile([128, 4], F32, tag="ki_m")
                nc.vector.tensor_scalar(out=mom[:, 0:1], in0=s1[:, 0:1],
                                        scalar1=1.0 / IHD, scalar2=None,
                                        op0=mybir.AluOpType.mult)
                nc.vector.tensor_scalar(out=mom[:, 1:2], in0=s1[:, 1:2],
                                        scalar1=1.0 / IHD, scalar2=None,
                                        op0=mybir.AluOpType.mult)
                nc.vector.tensor_mul(mom[:, 2:3], mom[:, 0:1], mom[:, 0:1])
                nc.vector.tensor_sub(mom[:, 2:3], mom[:, 1:2], mom[:, 2:3])
                nc.vector.tensor_scalar(out=mom[:, 2:3], in0=mom[:, 2:3],
                                        scalar1=1e-5, scalar2=None,
                                        op0=mybir.AluOpType.add)
                nc.scalar.activation(out=mom[:, 2:3], in_=mom[:, 2:3],
                                     func=mybir.ActivationFunctionType.Sqrt)
                nc.vector.reciprocal(out=mom[:, 3:4], in_=mom[:, 2:3])
                ki_n = p1.tile([128, IHD], F32, tag="ki_n")
                nc.vector.tensor_scalar(out=ki_n, in0=ki32,
                                        scalar1=mom[:, 0:1],
                                        scalar2=mom[:, 3:4],
                                        op0=mybir.AluOpType.subtract,
                                        op1=mybir.AluOpType.mult)
                nc.vector.tensor_mul(ki_n, ki_n, knw)
                nc.vector.tensor_add(ki_n, ki_n, knb)
                ki_r = p1.tile([128, IHD], F32, tag="ki_r")
                _rope_ni(nc, ki_r, ki_n, cos_t[:, t, :], sin_t[:, t, :])
                ki_hi, ki_lo = _split(nc, p1, ki_r, IHD, "ki")
                ptr = p1tr.tile([128, 128], BF16, tag="ptr")
                nc.tensor.transpose(ptr[:64, :], ki_hi, ident)
                nc.scalar.copy(out=kiT_hi[:, t * 128:(t + 1) * 128],
                               in_=ptr[:64, :])
                ptr = p1tr.tile([128, 128], BF16, tag="ptr")
                nc.tensor.transpose(ptr[:64, :], ki_lo, ident)
                nc.scalar.copy(out=kiT_lo[:, t * 128:(t + 1) * 128],
                               in_=ptr[:64, :])

        if dbg:
            nc.gpsimd.dma_start(out=d_ckvT,
                                in_=ckvT.rearrange("p a b -> p (a b)"))
            nc.gpsimd.dma_start(out=d_kpeT, in_=kpeT)
            nc.gpsimd.dma_start(out=d_kiT[:, 0:S], in_=kiT_hi)
            nc.gpsimd.dma_start(out=d_kiT[:, S:], in_=kiT_lo)

        # ---------------- P2: query-block projections ----------------
        mid = tc.alloc_tile_pool(name="mid", bufs=1)
        qTn = mid.tile([128, H, NB], BF16)       # nope part, feature-major
        qTp = mid.tile([64, H, NB], BF16)        # rope part
        qiT_hi = mid.tile([64, IH, NB], BF16)
        qiT_lo = mid.tile([64, IH, NB], BF16)

        with tc.tile_pool(name="p2w", bufs=2) as p2w, \
             tc.tile_pool(name="p2", bufs=2) as p2, \
             tc.tile_pool(name="p2ps", bufs=1, space="PSUM") as p2ps, \
             tc.tile_pool(name="p2tr", bufs=1, space="PSUM") as p2tr:
            cosb = p2.tile([128, NQT, DR], F32, tag="cosb", bufs=1)
            sinb = p2.tile([128, NQT, DR], F32, tag="sinb", bufs=1)
            for q in range(NQT):
                nc.gpsimd.dma_start(
                    out=cosb[:, q, :],
                    in_=cos_v[bass.ds(r0 + q * 128, 128), :])
                nc.gpsimd.dma_start(
                    out=sinb[:, q, :],
                    in_=sin_v[bass.ds(r0 + q * 128, 128), :])
            qnw = p2.tile([128, QLR], F32, tag="qnw", bufs=1)
            nc.gpsimd.dma_start(out=qnw, in_=_vb(gf, F32_OFF["q_norm_w"], QLR))
            gcat_w = p2.tile([128, NT, 2 * IH], BF16, tag="gcat", bufs=1)
            igh_r = igh_v.rearrange("(c p) n -> p c n", p=128)
            igl_r = igl_v.rearrange("(c p) n -> p c n", p=128)
            for c in range(NT):
                nc.gpsimd.dma_start(out=gcat_w[:, c, 0:IH], in_=igh_r[:, c, :])
                nc.gpsimd.dma_start(out=gcat_w[:, c, IH:], in_=igl_r[:, c, :])
            ps_qr = [p2ps.tile([128, 512], F32, tag=f"ps_qr{q}{i}",
                               name=f"ps_qr{q}{i}")
                     for q in range(NQT) for i in range(2)]
            ps_g = [p2ps.tile([128, 2 * IH], F32, tag=f"ps_g{q}",
                              name=f"ps_g{q}") for q in range(NQT)]
            for f in range(NT):
                wqah_f = p2w.tile([128, QLR], BF16, tag="wqah_f")
                nc.gpsimd.dma_start(out=wqah_f,
                                    in_=wqah_v[f * 128:(f + 1) * 128, :])
                wqal_f = p2w.tile([128, QLR], BF16, tag="wqal_f")
                nc.gpsimd.dma_start(out=wqal_f,
                                    in_=wqal_v[f * 128:(f + 1) * 128, :])
                xq_hi = p2w.tile([128, NB], BF16, tag="xq_hi", bufs=3)
                nc.gpsimd.dma_start(
                    out=xq_hi,
                    in_=xTh_v[f * 128:(f + 1) * 128, bass.ds(r0, NB)])
                xq_lo = p2w.tile([128, NB], BF16, tag="xq_lo", bufs=3)
                nc.gpsimd.dma_start(
                    out=xq_lo,
                    in_=xTl_v[f * 128:(f + 1) * 128, bass.ds(r0, NB)])
                st, sp = (f == 0), (f == NT - 1)
                for q in range(NQT):
                    lhs_hi = xq_hi[:, q * 128:(q + 1) * 128]
                    lhs_lo = xq_lo[:, q * 128:(q + 1) * 128]
                    for i in range(2):
                        cols = slice(i * 512, (i + 1) * 512)
                        nc.tensor.matmul(ps_qr[2 * q + i], lhs_hi,
                                         wqah_f[:, cols], start=st, stop=False)
                        nc.tensor.matmul(ps_qr[2 * q + i], lhs_hi,
                                         wqal_f[:, cols], start=False,
                                         stop=False)
                        nc.tensor.matmul(ps_qr[2 * q + i], lhs_lo,
                                         wqah_f[:, cols], start=False, stop=sp)
                    nc.tensor.matmul(ps_g[q][:, 0:2 * IH], lhs_hi,
                                     gcat_w[:, f, :], start=st, stop=False)
                    nc.tensor.matmul(ps_g[q][:, 0:IH], lhs_lo,
                                     gcat_w[:, f, 0:IH], start=False, stop=sp)
            qrT_hi = p2.tile([128, 8, NB], BF16, tag="qrT_hi", bufs=1)
            qrT_lo = p2.tile([128, 8, NB], BF16, tag="qrT_lo", bufs=1)
            gate_sb = p2.tile([128, NQT, IH], F32, tag="gate_sb", bufs=1)
            for q in range(NQT):
                qr_sb = p2.tile([128, QLR], F32, tag="qr_sb")
                _rmsnorm_from_psum(nc, p2, qr_sb,
                                   [ps_qr[2 * q], ps_qr[2 * q + 1]], qnw, QLR)
                nc.scalar.copy(out=gate_sb[:, q, :], in_=ps_g[q][:, 0:IH])
                nc.vector.tensor_add(gate_sb[:, q, :], gate_sb[:, q, :],
                                     ps_g[q][:, IH:2 * IH])
                nc.vector.tensor_scalar(out=gate_sb[:, q, :],
                                        in0=gate_sb[:, q, :],
                                        scalar1=SCALE_GATE * SCALE_IDX,
                                        scalar2=None,
                                        op0=mybir.AluOpType.mult)
                qr_hi, qr_lo = _split(nc, p2, qr_sb, QLR, "qr")
                for ch in range(8):
                    cols = slice(ch * 128, (ch + 1) * 128)
                    ptr = p2tr.tile([128, 128], BF16, tag="ptr2")
                    nc.tensor.transpose(ptr, qr_hi[:, cols], ident)
                    nc.scalar.copy(out=qrT_hi[:, ch, q * 128:(q + 1) * 128],
                                   in_=ptr)
                    ptr = p2tr.tile([128, 128], BF16, tag="ptr2")
                    nc.tensor.transpose(ptr, qr_lo[:, cols], ident)
                    nc.scalar.copy(out=qrT_lo[:, ch, q * 128:(q + 1) * 128],
                                   in_=ptr)
            # q projection per MLA head (bf16)
            wqbr = wqb_v.rearrange("(c p) n -> p c n", p=128)
            for h in range(H):
                wqb_h = p2w.tile([128, 8, DN + DR], BF16, tag="wqb_h")
                for c in range(8):
                    nc.gpsimd.dma_start(
                        out=wqb_h[:, c, :],
                        in_=wqbr[:, c, h * (DN + DR):(h + 1) * (DN + DR)])
                for q in range(NQT):
                    ps_q = p2ps.tile([128, DN + DR], F32, tag="ps_q")
                    for ch in range(8):
                        nc.tensor.matmul(
                            ps_q, qrT_hi[:, ch, q * 128:(q + 1) * 128],
                            wqb_h[:, ch, :],
                            start=(ch == 0), stop=(ch == 7))
                    qn_bf = p2.tile([128, DN], BF16, tag="qn_bf")
                    nc.vector.tensor_scalar(out=qn_bf, in0=ps_q[:, 0:DN],
                                            scalar1=SCALE_MLA, scalar2=None,
                                            op0=mybir.AluOpType.mult)
                    qp32 = p2.tile([128, DR], F32, tag="qp32")
                    _rope_int(nc, qp32, ps_q[:, DN:],
                              cosb[:, q, :], sinb[:, q, :])
                    qp_bf = p2.tile([128, DR], BF16, tag="qp_bf")
                    nc.vector.tensor_scalar(out=qp_bf, in0=qp32,
                                            scalar1=SCALE_MLA, scalar2=None,
                                            op0=mybir.AluOpType.mult)
                    ptr = p2tr.tile([128, 128], BF16, tag="ptr2")
                    nc.tensor.transpose(ptr, qn_bf, ident)
                    nc.scalar.copy(out=qTn[:, h, q * 128:(q + 1) * 128],
                                   in_=ptr)
                    ptr = p2tr.tile([128, 128], BF16, tag="ptr2")
                    nc.tensor.transpose(ptr[:64, :], qp_bf, ident)
                    nc.scalar.copy(out=qTp[:, h, q * 128:(q + 1) * 128],
                                   in_=ptr[:64, :])
            # indexer q heads: 3-pass hi/lo, rope, * gate, split, transpose
            iwqbh_r = iwqbh_v.rearrange("(c p) n -> p c n", p=128)
            iwqbl_r = iwqbl_v.rearrange("(c p) n -> p c n", p=128)
            for ih in range(IH):
                wiq_cat = p2w.tile([128, 8, 2 * IHD], BF16, tag="wiq_cat")
                for c in range(8):
                    nc.gpsimd.dma_start(
                        out=wiq_cat[:, c, 0:IHD],
                        in_=iwqbh_r[:, c, ih * IHD:(ih + 1) * IHD])
                    nc.gpsimd.dma_start(
                        out=wiq_cat[:, c, IHD:],
                        in_=iwqbl_r[:, c, ih * IHD:(ih + 1) * IHD])
                for q in range(NQT):
                    ps_qc = p2ps.tile([128, 2 * IHD], F32, tag="ps_q")
                    for ch in range(8):
                        nc.tensor.matmul(
                            ps_qc[:, 0:2 * IHD],
                            qrT_hi[:, ch, q * 128:(q + 1) * 128],
                            wiq_cat[:, ch, :],
                            start=(ch == 0), stop=False)
                        nc.tensor.matmul(
                            ps_qc[:, 0:IHD],
                            qrT_lo[:, ch, q * 128:(q + 1) * 128],
                            wiq_cat[:, ch, 0:IHD],
                            start=False, stop=(ch == 7))
                    qi32 = p2.tile([128, IHD], F32, tag="qi32")
                    nc.scalar.copy(out=qi32, in_=ps_qc[:, 0:IHD])
                    nc.vector.tensor_add(qi32, qi32, ps_qc[:, IHD:2 * IHD])
                    qi_r = p2.tile([128, IHD], F32, tag="qi_r")
                    _rope_ni(nc, qi_r, qi32, cosb[:, q, :], sinb[:, q, :])
                    nc.vector.tensor_scalar(out=qi_r, in0=qi_r,
                                            scalar1=gate_sb[:, q, ih:ih + 1],
                                            scalar2=None,
                                            op0=mybir.AluOpType.mult)
                    qi_hi, qi_lo = _split(nc, p2, qi_r, IHD, "qi")
                    ptr = p2tr.tile([128, 128], BF16, tag="ptr2")
                    nc.tensor.transpose(ptr[:64, :], qi_hi, ident)
                    nc.scalar.copy(out=qiT_hi[:, ih, q * 128:(q + 1) * 128],
                                   in_=ptr[:64, :])
                    ptr = p2tr.tile([128, 128], BF16, tag="ptr2")
                    nc.tensor.transpose(ptr[:64, :], qi_lo, ident)
                    nc.scalar.copy(out=qiT_lo[:, ih, q * 128:(q + 1) * 128],
                                   in_=ptr[:64, :])
            if dbg:
                nc.gpsimd.dma_start(out=d_qrT[:, 0:8 * NB],
                                    in_=qrT_hi.rearrange("p a b -> p (a b)"))
                nc.gpsimd.dma_start(out=d_qrT[:, 8 * NB:],
                                    in_=qrT_lo.rearrange("p a b -> p (a b)"))
                nc.gpsimd.dma_start(out=d_gate,
                                    in_=gate_sb.rearrange("p a b -> p (a b)"))
                nc.gpsimd.dma_start(out=d_qiT[:, 0:IH * NB],
                                    in_=qiT_hi.rearrange("p a b -> p (a b)"))
                nc.gpsimd.dma_start(out=d_qiT[:, IH * NB:],
                                    in_=qiT_lo.rearrange("p a b -> p (a b)"))

        # ---------------- P3: index scores + EXACT top-k ----------------
        maskNEG = mid.tile([128, NQT, S], F32)
        with tc.tile_pool(name="p3", bufs=1) as p3, \
             tc.tile_pool(name="p3ps", bufs=4, space="PSUM") as p3ps:
            for q in range(NQT):
                cm = p3.tile([128, S], F32, tag="cm")
                if causal:
                    # cmask = (col > row) * NEG
                    rq = p3.tile([128, 1], F32, tag="rq")
                    nc.vector.tensor_scalar(out=rq, in0=rowid,
                                            scalar1=float(q * 128),
                                            scalar2=None,
                                            op0=mybir.AluOpType.add)
                    nc.vector.tensor_scalar(out=cm, in0=iota_sb,
                                            scalar1=rq, scalar2=NEG,
                                            op0=mybir.AluOpType.is_gt,
                                            op1=mybir.AluOpType.mult)
                else:
                    nc.gpsimd.dma_start(
                        out=cm, in_=amask_d[q * 128:(q + 1) * 128, :])
                isc = p3.tile([128, S], F32, tag="isc")
                for kc in range(4):
                    cols = slice(kc * 512, (kc + 1) * 512)
                    ps = p3ps.tile([128, 512], F32, tag="ps_isc")
                    for ih in range(IH):
                        qcols = slice(q * 128, (q + 1) * 128)
                        nc.tensor.matmul(ps, qiT_hi[:, ih, qcols],
                                         kiT_hi[:, cols],
                                         start=(ih == 0), stop=False)
                        nc.tensor.matmul(ps, qiT_hi[:, ih, qcols],
                                         kiT_lo[:, cols],
                                         start=False, stop=False)
                        nc.tensor.matmul(ps, qiT_lo[:, ih, qcols],
                                         kiT_hi[:, cols],
                                         start=False, stop=(ih == IH - 1))
                    nc.vector.tensor_add(isc[:, cols], ps, cm[:, cols])
                # clamp; masked cols sit at -200 (amask re-kills them later)
                nc.vector.tensor_scalar(out=isc, in0=isc, scalar1=-200.0,
                                        scalar2=None, op0=mybir.AluOpType.max)
                # exact top-256 threshold: 32 rounds of top-8 + replace
                scr = p3.tile([128, S], F32, tag="sel_scr")
                nc.vector.tensor_copy(scr, isc)
                mx = p3.tile([128, 8], F32, tag="mx")
                for r in range(TOPK // 8):
                    nc.vector.max(out=mx, in_=scr)
                    if r < TOPK // 8 - 1:
                        nc.vector.match_replace(out=scr, in_to_replace=mx,
                                                in_values=scr, imm_value=-3e9)
                nc.vector.tensor_scalar(out=maskNEG[:, q, :], in0=isc,
                                        scalar1=mx[:, 7:8], scalar2=NEG,
                                        op0=mybir.AluOpType.is_lt,
                                        op1=mybir.AluOpType.mult)
                nc.vector.tensor_add(maskNEG[:, q, :], maskNEG[:, q, :], cm)
                if dbg:
                    nc.gpsimd.dma_start(out=d_thr[:, q:q + 1], in_=mx[:, 7:8])

        if dbg:
            nc.gpsimd.dma_start(out=d_mask,
                                in_=maskNEG.rearrange("p a b -> p (a b)"))

        # ---------------- P4: sparse MLA attention per head ----------------
        out_hT = mid.tile([128, H, NB], BF16)
        with tc.tile_pool(name="p4w", bufs=2) as p4w, \
             tc.tile_pool(name="p4k", bufs=2) as p4k, \
             tc.tile_pool(name="p4p", bufs=2) as p4p, \
             tc.tile_pool(name="p4ps", bufs=2, space="PSUM") as p4ps, \
             tc.tile_pool(name="p4po", bufs=2, space="PSUM") as p4po:
            wbr = wkvb_v.rearrange("(c p) n -> p c n", p=128)
            for h in range(H):
                wb_k = p4w.tile([128, 4, DN], BF16, tag="wb_k")
                wb_v = p4w.tile([128, 4, DV], BF16, tag="wb_v")
                for c in range(4):
                    nc.gpsimd.dma_start(
                        out=wb_k[:, c, :],
                        in_=wbr[:, c, h * (DN + DV):h * (DN + DV) + DN])
                    nc.gpsimd.dma_start(
                        out=wb_v[:, c, :],
                        in_=wbr[:, c, h * (DN + DV) + DN:(h + 1) * (DN + DV)])
                knT = p4k.tile([128, S], BF16, tag="knT")
                for kc in range(4):
                    ps = p4ps.tile([128, 512], F32, tag="ps_kn")
                    for c in range(4):
                        nc.tensor.matmul(
                            ps, wb_k[:, c, :],
                            ckvT[:, c, kc * 512:(kc + 1) * 512],
                            start=(c == 0), stop=(c == 3))
                    nc.scalar.copy(out=knT[:, kc * 512:(kc + 1) * 512], in_=ps)
                v_sb = p4k.tile([128, NT, DV], BF16, tag="v_sb")
                for kt in range(NT):
                    ps = p4ps.tile([128, DV], F32, tag="ps_v")
                    for c in range(4):
                        nc.tensor.matmul(
                            ps,
                            ckvT[:, c, kt * 128:(kt + 1) * 128],
                            wb_v[:, c, :],
                            start=(c == 0), stop=(c == 3))
                    nc.scalar.copy(out=v_sb[:, kt, :], in_=ps)
                ps_o = p4po.tile([128, NB], F32, tag="ps_o")
                for q in range(NQT):
                    probs = p4p.tile([128, S], F32, tag="probs", bufs=1)
                    for kc in range(4):
                        cols = slice(kc * 512, (kc + 1) * 512)
                        ps = p4ps.tile([128, 512], F32, tag="ps_s")
                        nc.tensor.matmul(
                            ps, qTn[:, h, q * 128:(q + 1) * 128],
                            knT[:, cols], start=True, stop=False)
                        nc.tensor.matmul(
                            ps, qTp[:, h, q * 128:(q + 1) * 128],
                            kpeT[:, cols], start=False, stop=True)
                        nc.vector.tensor_add(probs[:, cols], ps,
                                             maskNEG[:, q, cols])
                    den = p4p.tile([128, 2], F32, tag="den")
                    nc.scalar.activation(out=probs, in_=probs,
                                         func=mybir.ActivationFunctionType.Exp,
                                         accum_out=den[:, 0:1])
                    nc.vector.reciprocal(out=den[:, 1:2], in_=den[:, 0:1])
                    pb = p4p.tile([128, S], BF16, tag="pb")
                    nc.vector.tensor_scalar(out=pb, in0=probs,
                                            scalar1=den[:, 1:2], scalar2=None,
                                            op0=mybir.AluOpType.mult)
                    pT = p4p.tile([128, NT, 128], BF16, tag="pT", bufs=1)
                    for kt in range(NT):
                        nc.scalar.dma_start_transpose(
                            out=pT[:, kt, :],
                            in_=pb[:, kt * 128:(kt + 1) * 128])
                    for kt in range(NT):
                        nc.tensor.matmul(
                            ps_o[:, q * 128:(q + 1) * 128],
                            v_sb[:, kt, :], pT[:, kt, :],
                            start=(kt == 0), stop=(kt == NT - 1))
                nc.scalar.copy(out=out_hT[:, h, :], in_=ps_o)

        if dbg:
            nc.gpsimd.dma_start(out=d_ohT,
                                in_=out_hT.rearrange("p a b -> p (a b)"))

        # ---------------- P5: output projection ----------------
        with tc.tile_pool(name="p5w", bufs=3) as p5w, \
             tc.tile_pool(name="p5", bufs=3) as p5, \
             tc.tile_pool(name="p5ps", bufs=4, space="PSUM") as p5ps:
            wor = wo_v.rearrange("(hh p) n -> p hh n", p=128)
            for g in range(NT):
                wo_g = p5w.tile([128, H, 128], BF16, tag="wo_g")
                for c in range(H):
                    nc.gpsimd.dma_start(
                        out=wo_g[:, c, :],
                        in_=wor[:, c, g * 128:(g + 1) * 128])
                ps = p5ps.tile([128, NB], F32, tag="ps_w")
                for h in range(H):
                    nc.tensor.matmul(ps, wo_g[:, h, :],
                                     out_hT[:, h, :],
                                     start=(h == 0), stop=(h == H - 1))
                ot = p5.tile([128, NB], FP16, tag="ot")
                nc.scalar.copy(out=ot, in_=ps)
                nc.gpsimd.dma_start(out=outT[g * 128:(g + 1) * 128, :], in_=ot)

        mid.release()
        consts.release()
        dram.release()
    nc.compile()
    return nc


_NC_CACHE = {}


def _get_nc(causal=True):
    if causal not in _NC_CACHE:
        _NC_CACHE[causal] = build_nc(causal)
    return _NC_CACHE[causal]


def _split_np(a):
    hi = a.astype(ml_dtypes.bfloat16)
    lo = (a - hi.astype(np.float32)).astype(ml_dtypes.bfloat16)
    return hi, lo


def _is_causal(am):
    s = am.shape[-1]
    r = np.arange(s, dtype=np.int64)
    causal = np.where(r[:, None] >= r[None, :], np.float32(0.0),
                      np.float32(NEG))
    return np.array_equal(am.reshape(s, s), causal)


def make_core_inputs(x, cos, sin, attn_mask, wq_a, q_norm_w, wq_b, wkv_a,
                     kv_norm_w, wkv_b, wo, idx_wq_b, idx_wk, idx_knorm_w,
                     idx_knorm_b, idx_gate):
    causal = _is_causal(np.asarray(attn_mask, np.float32))
    F32_OFF, F32_TOT = _f32_layout(causal)

    blob_bf = np.zeros(BF_TOT, ml_dtypes.bfloat16)

    def put_bf(name, arr):
        o = BF_OFF[name]
        blob_bf[o:o + arr.size] = np.ascontiguousarray(arr).reshape(-1)

    xT = np.ascontiguousarray(x[0].astype(np.float32).T)
    xh, xl = _split_np(xT)
    put_bf("xT_hi", xh)
    put_bf("xT_lo", xl)
    wh, wl = _split_np(np.asarray(wq_a, np.float32))
    put_bf("wqa_hi", wh)
    put_bf("wqa_lo", wl)
    put_bf("wq_b", np.asarray(wq_b, np.float32).astype(ml_dtypes.bfloat16))
    put_bf("wkv_a", np.asarray(wkv_a, np.float32).astype(ml_dtypes.bfloat16))
    put_bf("wkv_b", np.asarray(wkv_b, np.float32).astype(ml_dtypes.bfloat16))
    put_bf("wo", np.asarray(wo, np.float32).astype(ml_dtypes.bfloat16))
    ih_, il_ = _split_np(np.asarray(idx_wq_b, np.float32))
    put_bf("iwqb_hi", ih_)
    put_bf("iwqb_lo", il_)
    kh, kl = _split_np(np.asarray(idx_wk, np.float32))
    put_bf("iwk_hi", kh)
    put_bf("iwk_lo", kl)
    gh, gl = _split_np(np.asarray(idx_gate, np.float32))
    put_bf("igate_hi", gh)
    put_bf("igate_lo", gl)
    put_bf("ident", np.eye(128, dtype=np.float32))

    blob_f32 = np.zeros(F32_TOT, np.float32)

    def put_f(name, arr):
        o = F32_OFF[name]
        blob_f32[o:o + arr.size] = np.ascontiguousarray(
            arr, np.float32).reshape(-1)

    put_f("cos", cos[0])
    put_f("sin", sin[0])
    put_f("q_norm_w", q_norm_w)
    put_f("kv_norm_w", kv_norm_w)
    put_f("idx_knorm_w", idx_knorm_w)
    put_f("idx_knorm_b", idx_knorm_b)
    put_f("iota", np.arange(S, dtype=np.float32))
    rows = (np.arange(NCORES)[:, None] * NB
            + np.arange(128)[None, :]).astype(np.float32)
    put_f("rows", rows)

    lb, lf = BF_TOT // NCORES, F32_TOT // NCORES
    maps = []
    am = np.ascontiguousarray(attn_mask[0, 0], np.float32)
    for c in range(NCORES):
        m = {
            "shard_bf": np.ascontiguousarray(blob_bf[c * lb:(c + 1) * lb]),
            "shard_f32": np.ascontiguousarray(blob_f32[c * lf:(c + 1) * lf]),
        }
        if not causal:
            m["amask_rows"] = np.ascontiguousarray(
                am[c * NB:(c + 1) * NB])
        maps.append(m)
    return maps, causal


def kernel(x, cos, sin, attn_mask, wq_a, q_norm_w, wq_b, wkv_a, kv_norm_w,
           wkv_b, wo, idx_wq_b, idx_wk, idx_knorm_w, idx_knorm_b, idx_gate):
    from concourse.bass_utils import run_bass_kernel_spmd
    args = [np.asarray(a, np.float32) for a in (
        x, cos, sin, attn_mask, wq_a, q_norm_w, wq_b, wkv_a, kv_norm_w,
        wkv_b, wo, idx_wq_b, idx_wk, idx_knorm_w, idx_knorm_b, idx_gate)]
    maps, causal = make_core_inputs(*args)
    nc = _get_nc(causal)
    res = run_bass_kernel_spmd(nc, maps, list(range(NCORES)))
    outs = [np.asarray(r["outT"]).astype(np.float32).T for r in res.results]
    out = np.concatenate(outs, axis=0)[None]                   # [1, S, HID]
    return out.astype(np.float32)


# revision 15
# speedup vs baseline: 10.8837x; 1.0101x over previous
"""DSA sparse MLA attention kernel for TRN2, 8 NeuronCores.

Transfer-optimized SPMD design. The host->device tunnel moves ~52 MB/s,
so every input byte is shipped exactly ONCE: each core receives a 1/8
slice of two packed blobs (bf16 + f32) and the cores AllGather them
on-device (HBM-to-HBM over on-chip links, ~GB/ms). Per-core query-block
slices are carved out of the gathered blobs at runtime with
partition_id()-based dynamic DMA offsets, so no per-core host tensors
are needed at all.

Precision plan (harness gate: rel_err < 2e-2; this lands ~6e-3):
 - Indexer path (x, wq_a, idx_*) is selection-critical: tensors are
   shipped as hi/lo bf16 pairs (same bytes as f32) and matmuls use a
   3-pass hi/lo bf16 emulation (~1e-5 rel, 4x faster than fp32r which
   is only ~1e-3 accurate).
 - Top-256 selection is EXACT: 32 rounds of vector.max + match_replace
   give the true 256th-largest index score per row.
 - Attention path (wq_b, wkv_a, wkv_b, wo, q/k/v, probs) is plain bf16.
 - Output is fp16 (halves the donated-zeros upload + fetch).

Sharding: sequence-parallel over query rows; core c owns rows
[256c, 256(c+1)). KV/indexer-key expansion over all 2048 keys is
replicated (compute is ~free vs transfer).
"""

import numpy as np
import ml_dtypes

import jax

# Persistent XLA compilation cache: run_bass_kernel_spmd re-jits (and
# would re-run the walrus NEFF compile, ~1s) on every call; the disk
# cache turns that into a deserialize+load.
jax.config.update("jax_compilation_cache_dir", "/tmp/jax_cc_cache")
jax.config.update("jax_persistent_cache_min_compile_time_secs", 0.0)
jax.config.update("jax_persistent_cache_min_entry_size_bytes", 0)

import concourse.bass as bass
import concourse.bacc as bacc
import concourse.mybir as mybir
from concourse.tile import TileContext

F32 = mybir.dt.float32
BF16 = mybir.dt.bfloat16
FP16 = mybir.dt.float16

S, HID = 2048, 2048
H, DN, DR, DV = 16, 128, 64, 128
QLR, KVLR = 1024, 512
IH, IHD, TOPK = 8, 64, 256
NEG = -1e9
NB = 256            # query rows per core
NCORES = 8
NT = S // 128       # 16 token tiles
NQT = NB // 128     # 2 query tiles per core
SCALE_MLA = float((DN + DR) ** -0.5)
SCALE_IDX = float(IHD ** -0.5)
SCALE_GATE = float(IH ** -0.5)
ALIGN = 512         # element alignment for blob entries

I8_LAYOUT = [
    ("xT_lo", (HID, S)),
]
I8_OFF, I8_TOT = None, None  # filled below

BF_LAYOUT = [
    ("xT_hi", (HID, S)),
    ("wqa_hi", (HID, QLR)), ("wqa_lo", (HID, QLR)),
    ("wq_b", (QLR, H * (DN + DR))),
    ("wkv_a", (HID, KVLR + DR)),
    ("wkv_b", (KVLR, H * (DN + DV))),
    ("wo", (H * DV, HID)),
    ("iwqb_hi", (QLR, IH * IHD)), ("iwqb_lo", (QLR, IH * IHD)),
    ("iwk_hi", (HID, IHD)), ("iwk_lo", (HID, IHD)),
    ("igate_hi", (HID, IH)), ("igate_lo", (HID, IH)),
    ("ident", (128, 128)),
]

F32_LAYOUT_BASE = [
    ("cos", (S, DR)), ("sin", (S, DR)),
    ("q_norm_w", (1, QLR)), ("kv_norm_w", (1, KVLR)),
    ("idx_knorm_w", (1, IHD)), ("idx_knorm_b", (1, IHD)),
    ("iota", (1, S)),
    ("rows", (NCORES * 128, 1)),
    ("xlo_scale", (HID, 1)),
]


def _mk_layout(entries):
    offs, off = {}, 0
    for name, shape in entries:
        offs[name] = off
        n = int(np.prod(shape))
        off += (n + ALIGN - 1) // ALIGN * ALIGN
    tot = (off + NCORES * ALIGN - 1) // (NCORES * ALIGN) * (NCORES * ALIGN)
    return offs, tot


def _f32_layout(causal):
    return _mk_layout(list(F32_LAYOUT_BASE))


BF_OFF, BF_TOT = _mk_layout(BF_LAYOUT)
LB = BF_TOT // NCORES
I8_OFF, I8_TOT = _mk_layout(I8_LAYOUT)
LI = I8_TOT // NCORES


def _v(blob1d, off, r, c):
    """[r, c] row-major view at element offset off of a 1-D DRAM AP."""
    return blob1d[off:off + r * c].rearrange("(r c) -> r c", c=c)


def _vb(blob1d, off, n, parts=128):
    """Partition-broadcast view [parts, n] of n elements at offset off."""
    return bass.AP(tensor=blob1d.tensor, offset=blob1d.offset + off,
                   ap=[[0, parts], [1, n]])


def _rmsnorm_from_psum(nc, pool, out_sb, psums, wb, d, eps=1e-6):
    """out_sb[p, d] = psum * rsqrt(mean(psum^2)+eps) * w."""
    ssq = pool.tile([128, len(psums)], F32)
    for i, ps in enumerate(psums):
        w = ps.shape[-1]
        scr = pool.tile([128, 512], F32, tag="rms_scr")
        nc.scalar.activation(out=scr[:, :w], in_=ps,
                             func=mybir.ActivationFunctionType.Square,
                             accum_out=ssq[:, i:i + 1])
    tot = pool.tile([128, 1], F32)
    if len(psums) == 1:
        nc.vector.tensor_scalar(out=tot, in0=ssq, scalar1=1.0 / d,
                                scalar2=eps, op0=mybir.AluOpType.mult,
                                op1=mybir.AluOpType.add)
    else:
        nc.vector.tensor_reduce(out=tot, in_=ssq, axis=mybir.AxisListType.X,
                                op=mybir.AluOpType.add)
        nc.vector.tensor_scalar(out=tot, in0=tot, scalar1=1.0 / d,
                                scalar2=eps, op0=mybir.AluOpType.mult,
                                op1=mybir.AluOpType.add)
    nc.scalar.activation(out=tot, in_=tot,
                         func=mybir.ActivationFunctionType.Sqrt)
    rinv = pool.tile([128, 1], F32)
    nc.vector.reciprocal(out=rinv, in_=tot)
    off = 0
    for ps in psums:
        w = ps.shape[-1]
        nc.vector.tensor_scalar(out=out_sb[:, off:off + w], in0=ps,
                                scalar1=rinv, scalar2=None,
                                op0=mybir.AluOpType.mult)
        off += w
    nc.vector.tensor_mul(out_sb[:, :d], out_sb[:, :d], wb[:, :d])


def _rope_int(nc, out, in_, cos, sin):
    """Interleaved (GPT-J) rope, token-major [128, 64] -> out[128, 64]."""
    xp = in_.rearrange("p (a b) -> p a b", b=2)
    op = out.rearrange("p (a b) -> p a b", b=2)
    c, s = cos[:, 0:32], sin[:, 0:32]
    x1, x2 = xp[:, :, 0], xp[:, :, 1]
    nc.vector.tensor_mul(op[:, :, 0], x1, c)
    nc.vector.tensor_mul(op[:, :, 1], x2, c)
    t = nc._rope_scr.tile([128, 32], F32, tag="rope_t")
    nc.vector.tensor_mul(t, x2, s)
    nc.vector.tensor_sub(op[:, :, 0], op[:, :, 0], t)
    nc.vector.tensor_mul(t, x1, s)
    nc.vector.tensor_add(op[:, :, 1], op[:, :, 1], t)


def _rope_ni(nc, out, in_, cos, sin):
    """Non-interleaved (rotate_half) rope, [128, 64]."""
    x1, x2 = in_[:, 0:32], in_[:, 32:64]
    c1, c2 = cos[:, 0:32], cos[:, 32:64]
    s1, s2 = sin[:, 0:32], sin[:, 32:64]
    nc.vector.tensor_mul(out[:, 0:32], x1, c1)
    nc.vector.tensor_mul(out[:, 32:64], x2, c2)
    t = nc._rope_scr.tile([128, 32], F32, tag="rope_t")
    nc.vector.tensor_mul(t, x2, s1)
    nc.vector.tensor_sub(out[:, 0:32], out[:, 0:32], t)
    nc.vector.tensor_mul(t, x1, s2)
    nc.vector.tensor_add(out[:, 32:64], out[:, 32:64], t)


def _split(nc, pool, src_f32, n, tag):
    """f32 [128, n] -> (hi bf16, lo bf16) with hi+lo ~= src."""
    hi = pool.tile([128, n], BF16, tag=tag + "_hi")
    nc.vector.tensor_copy(hi, src_f32)
    hi32 = pool.tile([128, n], F32, tag=tag + "_h32")
    nc.vector.tensor_copy(hi32, hi)
    lo32 = pool.tile([128, n], F32, tag=tag + "_l32")
    nc.vector.tensor_sub(lo32, src_f32, hi32)
    lo = pool.tile([128, n], BF16, tag=tag + "_lo")
    nc.vector.tensor_copy(lo, lo32)
    return hi, lo


class _Bacc(bacc.Bacc):
    """Bacc with memoized BIR serialization: run_bass_kernel_spmd re-lowers
    (and re-serializes the ~9 MB BIR) on every call; the module is frozen
    after compile(), so the bytes are reusable."""
    _json_cache = None

    def to_json_bytes(self):
        if self._json_cache is None:
            self._json_cache = super().to_json_bytes()
        return self._json_cache


def build_nc(causal=True, dbg=False):
    F32_OFF, F32_TOT = _f32_layout(causal)
    lf = F32_TOT // NCORES

    nc = _Bacc("TRN2", target_bir_lowering=False, debug=False)
    shard_bf = nc.dram_tensor("shard_bf", [LB], BF16, kind="ExternalInput").ap()
    shard_i8 = nc.dram_tensor("shard_i8", [LI], mybir.dt.int8,
                              kind="ExternalInput").ap()
    shard_f32 = nc.dram_tensor("shard_f32", [lf], F32,
                               kind="ExternalInput").ap()
    outT = nc.dram_tensor("outT", [HID, NB], FP16, kind="ExternalOutput").ap()
    amask_d = None
    if not causal:
        amask_d = nc.dram_tensor("amask_rows", [NB, S], F32,
                                 kind="ExternalInput").ap()
    if dbg:
        d_ckvT = nc.dram_tensor("d_ckvT", [128, 4 * S], BF16,
                                kind="ExternalOutput").ap()
        d_kpeT = nc.dram_tensor("d_kpeT", [64, S], BF16,
                                kind="ExternalOutput").ap()
        d_kiT = nc.dram_tensor("d_kiT", [64, 2 * S], BF16,
                               kind="ExternalOutput").ap()
        d_qrT = nc.dram_tensor("d_qrT", [128, 2 * 8 * NB], BF16,
                               kind="ExternalOutput").ap()
        d_gate = nc.dram_tensor("d_gate", [128, NQT * IH], F32,
                                kind="ExternalOutput").ap()
        d_mask = nc.dram_tensor("d_mask", [128, NQT * S], F32,
                                kind="ExternalOutput").ap()
        d_thr = nc.dram_tensor("d_thr", [128, NQT], F32,
                               kind="ExternalOutput").ap()
        d_qiT = nc.dram_tensor("d_qiT", [64, 2 * IH * NB], BF16,
                               kind="ExternalOutput").ap()
        d_ohT = nc.dram_tensor("d_ohT", [128, H * NB], BF16,
                               kind="ExternalOutput").ap()

    with TileContext(nc) as tc:
        pid = nc.partition_id()
        r0 = pid * NB

        dram = tc.alloc_tile_pool(name="dram", bufs=1, space="DRAM")
        bounce_bf = dram.tile([LB], BF16)
        bounce_i8 = dram.tile([LI], mybir.dt.int8)
        bounce_f32 = dram.tile([lf], F32)
        gath_bf = nc.dram_tensor("gath_bf", [NCORES, LB], BF16,
                                 kind="Internal", addr_space="Shared").ap()
        gath_f32 = nc.dram_tensor("gath_f32", [NCORES, lf], F32,
                                  kind="Internal", addr_space="Shared").ap()
        gath_i8 = nc.dram_tensor("gath_i8", [NCORES, LI], mybir.dt.int8,
                                 kind="Internal", addr_space="Shared").ap()
        nc.gpsimd.dma_start(out=bounce_i8, in_=shard_i8)
        nc.gpsimd.collective_compute(
            "AllGather", mybir.AluOpType.bypass,
            replica_groups=[list(range(NCORES))],
            ins=[bounce_i8[:].opt()], outs=[gath_i8[:].opt()])
        nc.gpsimd.dma_start(out=bounce_bf, in_=shard_bf)
        nc.gpsimd.dma_start(out=bounce_f32, in_=shard_f32)
        nc.gpsimd.collective_compute(
            "AllGather", mybir.AluOpType.bypass,
            replica_groups=[list(range(NCORES))],
            ins=[bounce_bf[:].opt()], outs=[gath_bf[:].opt()])
        nc.gpsimd.collective_compute(
            "AllGather", mybir.AluOpType.bypass,
            replica_groups=[list(range(NCORES))],
            ins=[bounce_f32[:].opt()], outs=[gath_f32[:].opt()])
        gb = gath_bf.rearrange("a b -> (a b)")
        gf = gath_f32.rearrange("a b -> (a b)")
        gi = gath_i8.rearrange("a b -> (a b)")

        xTh_v = _v(gb, BF_OFF["xT_hi"], HID, S)
        xTl_v = _v(gi, I8_OFF["xT_lo"], HID, S)
        wqah_v = _v(gb, BF_OFF["wqa_hi"], HID, QLR)
        wqal_v = _v(gb, BF_OFF["wqa_lo"], HID, QLR)
        wqb_v = _v(gb, BF_OFF["wq_b"], QLR, H * (DN + DR))
        wkva_v = _v(gb, BF_OFF["wkv_a"], HID, KVLR + DR)
        wkvb_v = _v(gb, BF_OFF["wkv_b"], KVLR, H * (DN + DV))
        wo_v = _v(gb, BF_OFF["wo"], H * DV, HID)
        iwqbh_v = _v(gb, BF_OFF["iwqb_hi"], QLR, IH * IHD)
        iwqbl_v = _v(gb, BF_OFF["iwqb_lo"], QLR, IH * IHD)
        iwkh_v = _v(gb, BF_OFF["iwk_hi"], HID, IHD)
        iwkl_v = _v(gb, BF_OFF["iwk_lo"], HID, IHD)
        igh_v = _v(gb, BF_OFF["igate_hi"], HID, IH)
        igl_v = _v(gb, BF_OFF["igate_lo"], HID, IH)
        ident_v = _v(gb, BF_OFF["ident"], 128, 128)
        cos_v = _v(gf, F32_OFF["cos"], S, DR)
        sin_v = _v(gf, F32_OFF["sin"], S, DR)
        rows_v = _v(gf, F32_OFF["rows"], NCORES * 128, 1)

        consts = tc.alloc_tile_pool(name="consts", bufs=1)
        nc._rope_scr = consts

        ident = consts.tile([128, 128], BF16)
        nc.gpsimd.dma_start(out=ident, in_=ident_v)
        kvnw = consts.tile([128, KVLR], F32)
        nc.gpsimd.dma_start(out=kvnw, in_=_vb(gf, F32_OFF["kv_norm_w"], KVLR))
        knw = consts.tile([128, IHD], F32)
        nc.gpsimd.dma_start(out=knw, in_=_vb(gf, F32_OFF["idx_knorm_w"], IHD))
        knb = consts.tile([128, IHD], F32)
        nc.gpsimd.dma_start(out=knb, in_=_vb(gf, F32_OFF["idx_knorm_b"], IHD))
        iota_sb = consts.tile([128, S], F32)
        nc.gpsimd.dma_start(out=iota_sb, in_=_vb(gf, F32_OFF["iota"], S))
        rowid = consts.tile([128, 1], F32)
        nc.gpsimd.dma_start(out=rowid, in_=rows_v[bass.ds(pid * 128, 128), :])
        xls = consts.tile([128, NT], F32)
        xls_v = _v(gf, F32_OFF["xlo_scale"], HID, 1)
        nc.gpsimd.dma_start(
            out=xls, in_=xls_v.rearrange("(c p) o -> p (c o)", p=128))

        ckvT = consts.tile([128, 4, S], BF16)      # [ckv_chunk, 4, tok]
        kpeT = consts.tile([64, S], BF16)
        kiT_hi = consts.tile([64, S], BF16)
        kiT_lo = consts.tile([64, S], BF16)

        # ---------------- P1: KV / indexer-key expansion ----------------
        with tc.tile_pool(name="p1w", bufs=1) as p1w, \
             tc.tile_pool(name="p1", bufs=3) as p1, \
             tc.tile_pool(name="p1ps", bufs=2, space="PSUM") as p1ps, \
             tc.tile_pool(name="p1tr", bufs=2, space="PSUM") as p1tr:
            cos_t = p1w.tile([128, NT, DR], F32)
            sin_t = p1w.tile([128, NT, DR], F32)
            cr = cos_v.rearrange("(t p) d -> p t d", p=128)
            sr = sin_v.rearrange("(t p) d -> p t d", p=128)
            wkva_sb = p1w.tile([128, NT, KVLR], BF16)
            wr = wkva_v.rearrange("(c p) n -> p c n", p=128)
            # wcat: [k_pe cols of wkv_a | iwk_hi | iwk_lo]
            wcat = p1w.tile([128, NT, DR + 2 * IHD], BF16)
            ikh = iwkh_v.rearrange("(c p) n -> p c n", p=128)
            ikl = iwkl_v.rearrange("(c p) n -> p c n", p=128)
            for c in range(NT):
                nc.gpsimd.dma_start(out=cos_t[:, c, :], in_=cr[:, c, :])
                nc.gpsimd.dma_start(out=sin_t[:, c, :], in_=sr[:, c, :])
                nc.gpsimd.dma_start(out=wkva_sb[:, c, :],
                                    in_=wr[:, c, 0:KVLR])
                nc.gpsimd.dma_start(out=wcat[:, c, 0:DR],
                                    in_=wr[:, c, KVLR:])
                nc.gpsimd.dma_start(out=wcat[:, c, DR:DR + IHD],
                                    in_=ikh[:, c, :])
                nc.gpsimd.dma_start(out=wcat[:, c, DR + IHD:],
                                    in_=ikl[:, c, :])

            xrh = xTh_v.rearrange("(c p) (u q) -> p c u q", p=128, q=128)
            xrl = xTl_v.rearrange("(c p) (u q) -> p c u q", p=128, q=128)
            for t in range(NT):
                xt_hi = p1.tile([128, NT, 128], BF16, tag="xt_hi")
                xt_q = p1.tile([128, NT, 128], mybir.dt.int8, tag="xt_q")
                xt_lo = p1.tile([128, NT, 128], BF16, tag="xt_lo")
                nc.gpsimd.dma_start(out=xt_hi, in_=xrh[:, :, t, :])
                nc.gpsimd.dma_start(out=xt_q, in_=xrl[:, :, t, :])
                for c in range(NT):
                    nc.vector.tensor_scalar(
                        out=xt_lo[:, c, :], in0=xt_q[:, c, :],
                        scalar1=xls[:, c:c + 1], scalar2=None,
                        op0=mybir.AluOpType.mult)
                ps_kv = p1ps.tile([128, KVLR], F32, tag="ps_kv")
                ps_x = p1ps.tile([128, DR + 2 * IHD], F32, tag="ps_x")
                ps_kl = p1ps.tile([128, IHD], F32, tag="ps_kl")
                for f in range(NT):
                    st, sp = (f == 0), (f == NT - 1)
                    nc.tensor.matmul(ps_kv, xt_hi[:, f, :],
                                     wkva_sb[:, f, :], start=st, stop=sp)
                    nc.tensor.matmul(ps_x, xt_hi[:, f, :],
                                     wcat[:, f, :], start=st, stop=sp)
                    nc.tensor.matmul(ps_kl, xt_lo[:, f, :],
                                     wcat[:, f, DR:DR + IHD],
                                     start=st, stop=sp)
                # ckv rmsnorm -> bf16 -> transpose into ckvT
                ckv_sb = p1.tile([128, KVLR], F32, tag="ckv_sb")
                _rmsnorm_from_psum(nc, p1, ckv_sb, [ps_kv], kvnw, KVLR)
                ckv_bf = p1.tile([128, KVLR], BF16, tag="ckv_bf")
                nc.vector.tensor_copy(ckv_bf, ckv_sb)
                for ch in range(4):
                    ptr = p1tr.tile([128, 128], BF16, tag="ptr")
                    nc.tensor.transpose(ptr, ckv_bf[:, ch * 128:(ch + 1) * 128],
                                        ident)
                    nc.scalar.copy(out=ckvT[:, ch, t * 128:(t + 1) * 128],
                                   in_=ptr)
                # k_pe rope -> bf16 -> transpose into kpeT
                pe_sb = p1.tile([128, DR], F32, tag="pe_sb")
                _rope_int(nc, pe_sb, ps_x[:, 0:DR],
                          cos_t[:, t, :], sin_t[:, t, :])
                pe_bf = p1.tile([128, DR], BF16, tag="pe_bf")
                nc.vector.tensor_copy(pe_bf, pe_sb)
                ptr = p1tr.tile([128, 128], BF16, tag="ptr")
                nc.tensor.transpose(ptr[:64, :], pe_bf, ident)
                nc.scalar.copy(out=kpeT[:, t * 128:(t + 1) * 128],
                               in_=ptr[:64, :])
                # ki = layernorm(3-pass sum) + rope -> split -> transpose
                ki32 = p1.tile([128, IHD], F32, tag="ki32")
                nc.scalar.copy(out=ki32, in_=ps_x[:, DR:DR + IHD])
                nc.vector.tensor_add(ki32, ki32, ps_x[:, DR + IHD:])
                nc.vector.tensor_add(ki32, ki32, ps_kl)
                s1 = p1.tile([128, 2], F32, tag="ki_s")
                scr = p1.tile([128, IHD], F32, tag="ki_scr")
                nc.scalar.activation(out=scr, in_=ki32,
                                     func=mybir.ActivationFunctionType.Copy,
                                     accum_out=s1[:, 0:1])
                nc.scalar.activation(out=scr, in_=ki32,
                                     func=mybir.ActivationFunctionType.Square,
                                     accum_out=s1[:, 1:2])
                mom = p1.tile([128, 4], F32, tag="ki_m")
                nc.vector.tensor_scalar(out=mom[:, 0:1], in0=s1[:, 0:1],
                                        scalar1=1.0 / IHD, scalar2=None,
                                        op0=mybir.AluOpType.mult)
                nc.vector.tensor_scalar(out=mom[:, 1:2], in0=s1[:, 1:2],
                                        scalar1=1.0 / IHD, scalar2=None,
                                        op0=mybir.AluOpType.mult)
                nc.vector.tensor_mul(mom[:, 2:3], mom[:, 0:1], mom[:, 0:1])
                nc.vector.tensor_sub(mom[:, 2:3], mom[:, 1:2], mom[:, 2:3])
                nc.vector.tensor_scalar(out=mom[:, 2:3], in0=mom[:, 2:3],
                                        scalar1=1e-5, scalar2=None,
                                        op0=mybir.AluOpType.add)
                nc.scalar.activation(out=mom[:, 2:3], in_=mom[:, 2:3],
                                     func=mybir.ActivationFunctionType.Sqrt)
                nc.vector.reciprocal(out=mom[:, 3:4], in_=mom[:, 2:3])
                ki_n = p1.tile([128, IHD], F32, tag="ki_n")
                nc.vector.tensor_scalar(out=ki_n, in0=ki32,
                                        scalar1=mom[:, 0:1],
                                        scalar2=mom[:, 3:4],
                                        op0=mybir.AluOpType.subtract,
                                        op1=mybir.AluOpType.mult)
                nc.vector.tensor_mul(ki_n, ki_n, knw)
                nc.vector.tensor_add(ki_n, ki_n, knb)
                ki_r = p1.tile([128, IHD], F32, tag="ki_r")
                _rope_ni(nc, ki_r, ki_n, cos_t[:, t, :], sin_t[:, t, :])
                ki_hi, ki_lo = _split(nc, p1, ki_r, IHD, "ki")
                ptr = p1tr.tile([128, 128], BF16, tag="ptr")
                nc.tensor.transpose(ptr[:64, :], ki_hi, ident)
                nc.scalar.copy(out=kiT_hi[:, t * 128:(t + 1) * 128],
                               in_=ptr[:64, :])
                ptr = p1tr.tile([128, 128], BF16, tag="ptr")
                nc.tensor.transpose(ptr[:64, :], ki_lo, ident)
                nc.scalar.copy(out=kiT_lo[:, t * 128:(t + 1) * 128],
                               in_=ptr[:64, :])

        if dbg:
            nc.gpsimd.dma_start(out=d_ckvT,
                                in_=ckvT.rearrange("p a b -> p (a b)"))
            nc.gpsimd.dma_start(out=d_kpeT, in_=kpeT)
            nc.gpsimd.dma_start(out=d_kiT[:, 0:S], in_=kiT_hi)
            nc.gpsimd.dma_start(out=d_kiT[:, S:], in_=kiT_lo)

        # ---------------- P2: query-block projections ----------------
        mid = tc.alloc_tile_pool(name="mid", bufs=1)
        qTn = mid.tile([128, H, NB], BF16)       # nope part, feature-major
        qTp = mid.tile([64, H, NB], BF16)        # rope part
        qiT_hi = mid.tile([64, IH, NB], BF16)
        qiT_lo = mid.tile([64, IH, NB], BF16)

        with tc.tile_pool(name="p2w", bufs=2) as p2w, \
             tc.tile_pool(name="p2", bufs=2) as p2, \
             tc.tile_pool(name="p2ps", bufs=1, space="PSUM") as p2ps, \
             tc.tile_pool(name="p2tr", bufs=1, space="PSUM") as p2tr:
            cosb = p2.tile([128, NQT, DR], F32, tag="cosb", bufs=1)
            sinb = p2.tile([128, NQT, DR], F32, tag="sinb", bufs=1)
            for q in range(NQT):
                nc.gpsimd.dma_start(
                    out=cosb[:, q, :],
                    in_=cos_v[bass.ds(r0 + q * 128, 128), :])
                nc.gpsimd.dma_start(
                    out=sinb[:, q, :],
                    in_=sin_v[bass.ds(r0 + q * 128, 128), :])
            qnw = p2.tile([128, QLR], F32, tag="qnw", bufs=1)
            nc.gpsimd.dma_start(out=qnw, in_=_vb(gf, F32_OFF["q_norm_w"], QLR))
            gcat_w = p2.tile([128, NT, 2 * IH], BF16, tag="gcat", bufs=1)
            igh_r = igh_v.rearrange("(c p) n -> p c n", p=128)
            igl_r = igl_v.rearrange("(c p) n -> p c n", p=128)
            for c in range(NT):
                nc.gpsimd.dma_start(out=gcat_w[:, c, 0:IH], in_=igh_r[:, c, :])
                nc.gpsimd.dma_start(out=gcat_w[:, c, IH:], in_=igl_r[:, c, :])
            ps_qr = [p2ps.tile([128, 512], F32, tag=f"ps_qr{q}{i}",
                               name=f"ps_qr{q}{i}")
                     for q in range(NQT) for i in range(2)]
            ps_g = [p2ps.tile([128, 2 * IH], F32, tag=f"ps_g{q}",
                              name=f"ps_g{q}") for q in range(NQT)]
            for f in range(NT):
                wqah_f = p2w.tile([128, QLR], BF16, tag="wqah_f")
                nc.gpsimd.dma_start(out=wqah_f,
                                    in_=wqah_v[f * 128:(f + 1) * 128, :])
                wqal_f = p2w.tile([128, QLR], BF16, tag="wqal_f")
                nc.gpsimd.dma_start(out=wqal_f,
                                    in_=wqal_v[f * 128:(f + 1) * 128, :])
                xq_hi = p2w.tile([128, NB], BF16, tag="xq_hi", bufs=3)
                nc.gpsimd.dma_start(
                    out=xq_hi,
                    in_=xTh_v[f * 128:(f + 1) * 128, bass.ds(r0, NB)])
                xq_q = p2w.tile([128, NB], mybir.dt.int8, tag="xq_q",
                                bufs=3)
                nc.gpsimd.dma_start(
                    out=xq_q,
                    in_=xTl_v[f * 128:(f + 1) * 128, bass.ds(r0, NB)])
                xq_lo = p2w.tile([128, NB], BF16, tag="xq_lo", bufs=3)
                nc.vector.tensor_scalar(out=xq_lo, in0=xq_q,
                                        scalar1=xls[:, f:f + 1], scalar2=None,
                                        op0=mybir.AluOpType.mult)
                st, sp = (f == 0), (f == NT - 1)
                for q in range(NQT):
                    lhs_hi = xq_hi[:, q * 128:(q + 1) * 128]
                    lhs_lo = xq_lo[:, q * 128:(q + 1) * 128]
                    for i in range(2):
                        cols = slice(i * 512, (i + 1) * 512)
                        nc.tensor.matmul(ps_qr[2 * q + i], lhs_hi,
                                         wqah_f[:, cols], start=st, stop=False)
                        nc.tensor.matmul(ps_qr[2 * q + i], lhs_hi,
                                         wqal_f[:, cols], start=False,
                                         stop=False)
                        nc.tensor.matmul(ps_qr[2 * q + i], lhs_lo,
                                         wqah_f[:, cols], start=False, stop=sp)
                    nc.tensor.matmul(ps_g[q][:, 0:2 * IH], lhs_hi,
                                     gcat_w[:, f, :], start=st, stop=False)
                    nc.tensor.matmul(ps_g[q][:, 0:IH], lhs_lo,
                                     gcat_w[:, f, 0:IH], start=False, stop=sp)
            qrT_hi = p2.tile([128, 8, NB], BF16, tag="qrT_hi", bufs=1)
            qrT_lo = p2.tile([128, 8, NB], BF16, tag="qrT_lo", bufs=1)
            gate_sb = p2.tile([128, NQT, IH], F32, tag="gate_sb", bufs=1)
            for q in range(NQT):
                qr_sb = p2.tile([128, QLR], F32, tag="qr_sb")
                _rmsnorm_from_psum(nc, p2, qr_sb,
                                   [ps_qr[2 * q], ps_qr[2 * q + 1]], qnw, QLR)
                nc.scalar.copy(out=gate_sb[:, q, :], in_=ps_g[q][:, 0:IH])
                nc.vector.tensor_add(gate_sb[:, q, :], gate_sb[:, q, :],
                                     ps_g[q][:, IH:2 * IH])
                nc.vector.tensor_scalar(out=gate_sb[:, q, :],
                                        in0=gate_sb[:, q, :],
                                        scalar1=SCALE_GATE * SCALE_IDX,
                                        scalar2=None,
                                        op0=mybir.AluOpType.mult)
                qr_hi, qr_lo = _split(nc, p2, qr_sb, QLR, "qr")
                for ch in range(8):
                    cols = slice(ch * 128, (ch + 1) * 128)
                    ptr = p2tr.tile([128, 128], BF16, tag="ptr2")
                    nc.tensor.transpose(ptr, qr_hi[:, cols], ident)
                    nc.scalar.copy(out=qrT_hi[:, ch, q * 128:(q + 1) * 128],
                                   in_=ptr)
                    ptr = p2tr.tile([128, 128], BF16, tag="ptr2")
                    nc.tensor.transpose(ptr, qr_lo[:, cols], ident)
                    nc.scalar.copy(out=qrT_lo[:, ch, q * 128:(q + 1) * 128],
                                   in_=ptr)
            # q projection per MLA head (bf16)
            wqbr = wqb_v.rearrange("(c p) n -> p c n", p=128)
            for h in range(H):
                wqb_h = p2w.tile([128, 8, DN + DR], BF16, tag="wqb_h")
                for c in range(8):
                    nc.gpsimd.dma_start(
                        out=wqb_h[:, c, :],
                        in_=wqbr[:, c, h * (DN + DR):(h + 1) * (DN + DR)])
                for q in range(NQT):
                    ps_q = p2ps.tile([128, DN + DR], F32, tag="ps_q")
                    for ch in range(8):
                        nc.tensor.matmul(
                            ps_q, qrT_hi[:, ch, q * 128:(q + 1) * 128],
                            wqb_h[:, ch, :],
                            start=(ch == 0), stop=(ch == 7))
                    qn_bf = p2.tile([128, DN], BF16, tag="qn_bf")
                    nc.vector.tensor_scalar(out=qn_bf, in0=ps_q[:, 0:DN],
                                            scalar1=SCALE_MLA, scalar2=None,
                                            op0=mybir.AluOpType.mult)
                    qp32 = p2.tile([128, DR], F32, tag="qp32")
                    _rope_int(nc, qp32, ps_q[:, DN:],
                              cosb[:, q, :], sinb[:, q, :])
                    qp_bf = p2.tile([128, DR], BF16, tag="qp_bf")
                    nc.vector.tensor_scalar(out=qp_bf, in0=qp32,
                                            scalar1=SCALE_MLA, scalar2=None,
                                            op0=mybir.AluOpType.mult)
                    ptr = p2tr.tile([128, 128], BF16, tag="ptr2")
                    nc.tensor.transpose(ptr, qn_bf, ident)
                    nc.scalar.copy(out=qTn[:, h, q * 128:(q + 1) * 128],
                                   in_=ptr)
                    ptr = p2tr.tile([128, 128], BF16, tag="ptr2")
                    nc.tensor.transpose(ptr[:64, :], qp_bf, ident)
                    nc.scalar.copy(out=qTp[:, h, q * 128:(q + 1) * 128],
                                   in_=ptr[:64, :])
            # indexer q heads: 3-pass hi/lo, rope, * gate, split, transpose
            iwqbh_r = iwqbh_v.rearrange("(c p) n -> p c n", p=128)
            iwqbl_r = iwqbl_v.rearrange("(c p) n -> p c n", p=128)
            for ih in range(IH):
                wiq_cat = p2w.tile([128, 8, 2 * IHD], BF16, tag="wiq_cat")
                for c in range(8):
                    nc.gpsimd.dma_start(
                        out=wiq_cat[:, c, 0:IHD],
                        in_=iwqbh_r[:, c, ih * IHD:(ih + 1) * IHD])
                    nc.gpsimd.dma_start(
                        out=wiq_cat[:, c, IHD:],
                        in_=iwqbl_r[:, c, ih * IHD:(ih + 1) * IHD])
                for q in range(NQT):
                    ps_qc = p2ps.tile([128, 2 * IHD], F32, tag="ps_q")
                    for ch in range(8):
                        nc.tensor.matmul(
                            ps_qc[:, 0:2 * IHD],
                            qrT_hi[:, ch, q * 128:(q + 1) * 128],
                            wiq_cat[:, ch, :],
                            start=(ch == 0), stop=False)
                        nc.tensor.matmul(
                            ps_qc[:, 0:IHD],
                            qrT_lo[:, ch, q * 128:(q + 1) * 128],
                            wiq_cat[:, ch, 0:IHD],
                            start=False, stop=(ch == 7))
                    qi32 = p2.tile([128, IHD], F32, tag="qi32")
                    nc.scalar.copy(out=qi32, in_=ps_qc[:, 0:IHD])
                    nc.vector.tensor_add(qi32, qi32, ps_qc[:, IHD:2 * IHD])
                    qi_r = p2.tile([128, IHD], F32, tag="qi_r")
                    _rope_ni(nc, qi_r, qi32, cosb[:, q, :], sinb[:, q, :])
                    nc.vector.tensor_scalar(out=qi_r, in0=qi_r,
                                            scalar1=gate_sb[:, q, ih:ih + 1],
                                            scalar2=None,
                                            op0=mybir.AluOpType.mult)
                    qi_hi, qi_lo = _split(nc, p2, qi_r, IHD, "qi")
                    ptr = p2tr.tile([128, 128], BF16, tag="ptr2")
                    nc.tensor.transpose(ptr[:64, :], qi_hi, ident)
                    nc.scalar.copy(out=qiT_hi[:, ih, q * 128:(q + 1) * 128],
                                   in_=ptr[:64, :])
                    ptr = p2tr.tile([128, 128], BF16, tag="ptr2")
                    nc.tensor.transpose(ptr[:64, :], qi_lo, ident)
                    nc.scalar.copy(out=qiT_lo[:, ih, q * 128:(q + 1) * 128],
                                   in_=ptr[:64, :])
            if dbg:
                nc.gpsimd.dma_start(out=d_qrT[:, 0:8 * NB],
                                    in_=qrT_hi.rearrange("p a b -> p (a b)"))
                nc.gpsimd.dma_start(out=d_qrT[:, 8 * NB:],
                                    in_=qrT_lo.rearrange("p a b -> p (a b)"))
                nc.gpsimd.dma_start(out=d_gate,
                                    in_=gate_sb.rearrange("p a b -> p (a b)"))
                nc.gpsimd.dma_start(out=d_qiT[:, 0:IH * NB],
                                    in_=qiT_hi.rearrange("p a b -> p (a b)"))
                nc.gpsimd.dma_start(out=d_qiT[:, IH * NB:],
                                    in_=qiT_lo.rearrange("p a b -> p (a b)"))

        # ---------------- P3: index scores + EXACT top-k ----------------
        maskNEG = mid.tile([128, NQT, S], F32)
        with tc.tile_pool(name="p3", bufs=1) as p3, \
             tc.tile_pool(name="p3ps", bufs=4, space="PSUM") as p3ps:
            for q in range(NQT):
                cm = p3.tile([128, S], F32, tag="cm")
                if causal:
                    # cmask = (col > row) * NEG
                    rq = p3.tile([128, 1], F32, tag="rq")
                    nc.vector.tensor_scalar(out=rq, in0=rowid,
                                            scalar1=float(q * 128),
                                            scalar2=None,
                                            op0=mybir.AluOpType.add)
                    nc.vector.tensor_scalar(out=cm, in0=iota_sb,
                                            scalar1=rq, scalar2=NEG,
                                            op0=mybir.AluOpType.is_gt,
                                            op1=mybir.AluOpType.mult)
                else:
                    nc.gpsimd.dma_start(
                        out=cm, in_=amask_d[q * 128:(q + 1) * 128, :])
                isc = p3.tile([128, S], F32, tag="isc")
                for kc in range(4):
                    cols = slice(kc * 512, (kc + 1) * 512)
                    ps = p3ps.tile([128, 512], F32, tag="ps_isc")
                    for ih in range(IH):
                        qcols = slice(q * 128, (q + 1) * 128)
                        nc.tensor.matmul(ps, qiT_hi[:, ih, qcols],
                                         kiT_hi[:, cols],
                                         start=(ih == 0), stop=False)
                        nc.tensor.matmul(ps, qiT_hi[:, ih, qcols],
                                         kiT_lo[:, cols],
                                         start=False, stop=False)
                        nc.tensor.matmul(ps, qiT_lo[:, ih, qcols],
                                         kiT_hi[:, cols],
                                         start=False, stop=(ih == IH - 1))
                    nc.vector.tensor_add(isc[:, cols], ps, cm[:, cols])
                # clamp; masked cols sit at -200 (amask re-kills them later)
                nc.vector.tensor_scalar(out=isc, in0=isc, scalar1=-200.0,
                                        scalar2=None, op0=mybir.AluOpType.max)
                # exact top-256 threshold: 32 rounds of top-8 + replace
                scr = p3.tile([128, S], F32, tag="sel_scr")
                nc.vector.tensor_copy(scr, isc)
                mx = p3.tile([128, 8], F32, tag="mx")
                for r in range(TOPK // 8):
                    nc.vector.max(out=mx, in_=scr)
                    if r < TOPK // 8 - 1:
                        nc.vector.match_replace(out=scr, in_to_replace=mx,
                                                in_values=scr, imm_value=-3e9)
                nc.vector.tensor_scalar(out=maskNEG[:, q, :], in0=isc,
                                        scalar1=mx[:, 7:8], scalar2=NEG,
                                        op0=mybir.AluOpType.is_lt,
                                        op1=mybir.AluOpType.mult)
                nc.vector.tensor_add(maskNEG[:, q, :], maskNEG[:, q, :], cm)
                if dbg:
                    nc.gpsimd.dma_start(out=d_thr[:, q:q + 1], in_=mx[:, 7:8])

        if dbg:
            nc.gpsimd.dma_start(out=d_mask,
                                in_=maskNEG.rearrange("p a b -> p (a b)"))

        # ---------------- P4: sparse MLA attention per head ----------------
        out_hT = mid.tile([128, H, NB], BF16)
        with tc.tile_pool(name="p4w", bufs=2) as p4w, \
             tc.tile_pool(name="p4k", bufs=2) as p4k, \
             tc.tile_pool(name="p4p", bufs=2) as p4p, \
             tc.tile_pool(name="p4ps", bufs=2, space="PSUM") as p4ps, \
             tc.tile_pool(name="p4po", bufs=2, space="PSUM") as p4po:
            wbr = wkvb_v.rearrange("(c p) n -> p c n", p=128)
            for h in range(H):
                wb_k = p4w.tile([128, 4, DN], BF16, tag="wb_k")
                wb_v = p4w.tile([128, 4, DV], BF16, tag="wb_v")
                for c in range(4):
                    nc.gpsimd.dma_start(
                        out=wb_k[:, c, :],
                        in_=wbr[:, c, h * (DN + DV):h * (DN + DV) + DN])
                    nc.gpsimd.dma_start(
                        out=wb_v[:, c, :],
                        in_=wbr[:, c, h * (DN + DV) + DN:(h + 1) * (DN + DV)])
                knT = p4k.tile([128, S], BF16, tag="knT")
                for kc in range(4):
                    ps = p4ps.tile([128, 512], F32, tag="ps_kn")
                    for c in range(4):
                        nc.tensor.matmul(
                            ps, wb_k[:, c, :],
                            ckvT[:, c, kc * 512:(kc + 1) * 512],
                            start=(c == 0), stop=(c == 3))
                    nc.scalar.copy(out=knT[:, kc * 512:(kc + 1) * 512], in_=ps)
                v_sb = p4k.tile([128, NT, DV], BF16, tag="v_sb")
                for kt in range(NT):
                    ps = p4ps.tile([128, DV], F32, tag="ps_v")
                    for c in range(4):
                        nc.tensor.matmul(
                            ps,
                            ckvT[:, c, kt * 128:(kt + 1) * 128],
                            wb_v[:, c, :],
                            start=(c == 0), stop=(c == 3))
                    nc.scalar.copy(out=v_sb[:, kt, :], in_=ps)
                ps_o = p4po.tile([128, NB], F32, tag="ps_o")
                for q in range(NQT):
                    probs = p4p.tile([128, S], F32, tag="probs", bufs=1)
                    for kc in range(4):
                        cols = slice(kc * 512, (kc + 1) * 512)
                        ps = p4ps.tile([128, 512], F32, tag="ps_s")
                        nc.tensor.matmul(
                            ps, qTn[:, h, q * 128:(q + 1) * 128],
                            knT[:, cols], start=True, stop=False)
                        nc.tensor.matmul(
                            ps, qTp[:, h, q * 128:(q + 1) * 128],
                            kpeT[:, cols], start=False, stop=True)
                        nc.vector.tensor_add(probs[:, cols], ps,
                                             maskNEG[:, q, cols])
                    den = p4p.tile([128, 2], F32, tag="den")
                    nc.scalar.activation(out=probs, in_=probs,
                                         func=mybir.ActivationFunctionType.Exp,
                                         accum_out=den[:, 0:1])
                    nc.vector.reciprocal(out=den[:, 1:2], in_=den[:, 0:1])
                    pb = p4p.tile([128, S], BF16, tag="pb")
                    nc.vector.tensor_scalar(out=pb, in0=probs,
                                            scalar1=den[:, 1:2], scalar2=None,
                                            op0=mybir.AluOpType.mult)
                    pT = p4p.tile([128, NT, 128], BF16, tag="pT", bufs=1)
                    for kt in range(NT):
                        nc.scalar.dma_start_transpose(
                            out=pT[:, kt, :],
                            in_=pb[:, kt * 128:(kt + 1) * 128])
                    for kt in range(NT):
                        nc.tensor.matmul(
                            ps_o[:, q * 128:(q + 1) * 128],
                            v_sb[:, kt, :], pT[:, kt, :],
                            start=(kt == 0), stop=(kt == NT - 1))
                nc.scalar.copy(out=out_hT[:, h, :], in_=ps_o)

        if dbg:
            nc.gpsimd.dma_start(out=d_ohT,
                                in_=out_hT.rearrange("p a b -> p (a b)"))

        # ---------------- P5: output projection ----------------
        with tc.tile_pool(name="p5w", bufs=3) as p5w, \
             tc.tile_pool(name="p5", bufs=3) as p5, \
             tc.tile_pool(name="p5ps", bufs=4, space="PSUM") as p5ps:
            wor = wo_v.rearrange("(hh p) n -> p hh n", p=128)
            for g in range(NT):
                wo_g = p5w.tile([128, H, 128], BF16, tag="wo_g")
                for c in range(H):
                    nc.gpsimd.dma_start(
                        out=wo_g[:, c, :],
                        in_=wor[:, c, g * 128:(g + 1) * 128])
                ps = p5ps.tile([128, NB], F32, tag="ps_w")
                for h in range(H):
                    nc.tensor.matmul(ps, wo_g[:, h, :],
                                     out_hT[:, h, :],
                                     start=(h == 0), stop=(h == H - 1))
                ot = p5.tile([128, NB], FP16, tag="ot")
                nc.scalar.copy(out=ot, in_=ps)
                nc.gpsimd.dma_start(out=outT[g * 128:(g + 1) * 128, :], in_=ot)

        mid.release()
        consts.release()
        dram.release()
    nc.compile()
    return nc


_NC_CACHE = {}


def _get_nc(causal=True):
    if causal not in _NC_CACHE:
        _NC_CACHE[causal] = build_nc(causal)
    return _NC_CACHE[causal]


def _split_np(a):
    hi = a.astype(ml_dtypes.bfloat16)
    lo = (a - hi.astype(np.float32)).astype(ml_dtypes.bfloat16)
    return hi, lo


def _is_causal(am):
    s = am.shape[-1]
    r = np.arange(s, dtype=np.int64)
    causal = np.where(r[:, None] >= r[None, :], np.float32(0.0),
                      np.float32(NEG))
    return np.array_equal(am.reshape(s, s), causal)


def make_core_inputs(x, cos, sin, attn_mask, wq_a, q_norm_w, wq_b, wkv_a,
                     kv_norm_w, wkv_b, wo, idx_wq_b, idx_wk, idx_knorm_w,
                     idx_knorm_b, idx_gate):
    causal = _is_causal(np.asarray(attn_mask, np.float32))
    F32_OFF, F32_TOT = _f32_layout(causal)

    blob_bf = np.zeros(BF_TOT, ml_dtypes.bfloat16)

    def put_bf(name, arr):
        o = BF_OFF[name]
        blob_bf[o:o + arr.size] = np.ascontiguousarray(arr).reshape(-1)

    xT = np.ascontiguousarray(x[0].astype(np.float32).T)
    xh = xT.astype(ml_dtypes.bfloat16)
    put_bf("xT_hi", xh)
    lo32 = xT - xh.astype(np.float32)
    xls_np = (np.abs(lo32).max(axis=1, keepdims=True) / 127.0
              ).astype(np.float32) + 1e-30
    blob_i8 = np.zeros(I8_TOT, np.int8)
    qv = np.clip(np.round(lo32 / xls_np), -127, 127).astype(np.int8)
    blob_i8[I8_OFF["xT_lo"]:I8_OFF["xT_lo"] + qv.size] = qv.reshape(-1)
    wh, wl = _split_np(np.asarray(wq_a, np.float32))
    put_bf("wqa_hi", wh)
    put_bf("wqa_lo", wl)
    put_bf("wq_b", np.asarray(wq_b, np.float32).astype(ml_dtypes.bfloat16))
    put_bf("wkv_a", np.asarray(wkv_a, np.float32).astype(ml_dtypes.bfloat16))
    put_bf("wkv_b", np.asarray(wkv_b, np.float32).astype(ml_dtypes.bfloat16))
    put_bf("wo", np.asarray(wo, np.float32).astype(ml_dtypes.bfloat16))
    ih_, il_ = _split_np(np.asarray(idx_wq_b, np.float32))
    put_bf("iwqb_hi", ih_)
    put_bf("iwqb_lo", il_)
    kh, kl = _split_np(np.asarray(idx_wk, np.float32))
    put_bf("iwk_hi", kh)
    put_bf("iwk_lo", kl)
    gh, gl = _split_np(np.asarray(idx_gate, np.float32))
    put_bf("igate_hi", gh)
    put_bf("igate_lo", gl)
    put_bf("ident", np.eye(128, dtype=np.float32))

    blob_f32 = np.zeros(F32_TOT, np.float32)

    def put_f(name, arr):
        o = F32_OFF[name]
        blob_f32[o:o + arr.size] = np.ascontiguousarray(
            arr, np.float32).reshape(-1)

    put_f("cos", cos[0])
    put_f("sin", sin[0])
    put_f("q_norm_w", q_norm_w)
    put_f("kv_norm_w", kv_norm_w)
    put_f("idx_knorm_w", idx_knorm_w)
    put_f("idx_knorm_b", idx_knorm_b)
    put_f("iota", np.arange(S, dtype=np.float32))
    put_f("xlo_scale", xls_np)
    rows = (np.arange(NCORES)[:, None] * NB
            + np.arange(128)[None, :]).astype(np.float32)
    put_f("rows", rows)

    lb, lf = BF_TOT // NCORES, F32_TOT // NCORES
    maps = []
    am = np.ascontiguousarray(attn_mask[0, 0], np.float32)
    for c in range(NCORES):
        li = I8_TOT // NCORES
        m = {
            "shard_bf": np.ascontiguousarray(blob_bf[c * lb:(c + 1) * lb]),
            "shard_i8": np.ascontiguousarray(blob_i8[c * li:(c + 1) * li]),
            "shard_f32": np.ascontiguousarray(blob_f32[c * lf:(c + 1) * lf]),
        }
        if not causal:
            m["amask_rows"] = np.ascontiguousarray(
                am[c * NB:(c + 1) * NB])
        maps.append(m)
    return maps, causal


def kernel(x, cos, sin, attn_mask, wq_a, q_norm_w, wq_b, wkv_a, kv_norm_w,
           wkv_b, wo, idx_wq_b, idx_wk, idx_knorm_w, idx_knorm_b, idx_gate):
    from concourse.bass_utils import run_bass_kernel_spmd
    args = [np.asarray(a, np.float32) for a in (
        x, cos, sin, attn_mask, wq_a, q_norm_w, wq_b, wkv_a, kv_norm_w,
        wkv_b, wo, idx_wq_b, idx_wk, idx_knorm_w, idx_knorm_b, idx_gate)]
    maps, causal = make_core_inputs(*args)
    nc = _get_nc(causal)
    res = run_bass_kernel_spmd(nc, maps, list(range(NCORES)))
    outs = [np.asarray(r["outT"]).astype(np.float32).T for r in res.results]
    out = np.concatenate(outs, axis=0)[None]                   # [1, S, HID]
    return out.astype(np.float32)


# revision 16
# speedup vs baseline: 11.8812x; 1.0917x over previous
"""DSA sparse MLA attention kernel for TRN2, 8 NeuronCores.

Transfer-optimized SPMD design. The host->device tunnel moves ~52 MB/s,
so every input byte is shipped exactly ONCE: each core receives a 1/8
slice of two packed blobs (bf16 + f32) and the cores AllGather them
on-device (HBM-to-HBM over on-chip links, ~GB/ms). Per-core query-block
slices are carved out of the gathered blobs at runtime with
partition_id()-based dynamic DMA offsets, so no per-core host tensors
are needed at all.

Precision plan (harness gate: rel_err < 2e-2; this lands ~6e-3):
 - Indexer path (x, wq_a, idx_*) is selection-critical: tensors are
   shipped as hi/lo bf16 pairs (same bytes as f32) and matmuls use a
   3-pass hi/lo bf16 emulation (~1e-5 rel, 4x faster than fp32r which
   is only ~1e-3 accurate).
 - Top-256 selection is EXACT: 32 rounds of vector.max + match_replace
   give the true 256th-largest index score per row.
 - Attention path (wq_b, wkv_a, wkv_b, wo, q/k/v, probs) is plain bf16.
 - Output is fp16 (halves the donated-zeros upload + fetch).

Sharding: sequence-parallel over query rows; core c owns rows
[256c, 256(c+1)). KV/indexer-key expansion over all 2048 keys is
replicated (compute is ~free vs transfer).
"""

import numpy as np
import ml_dtypes

import jax

# Persistent XLA compilation cache: run_bass_kernel_spmd re-jits (and
# would re-run the walrus NEFF compile, ~1s) on every call; the disk
# cache turns that into a deserialize+load.
jax.config.update("jax_compilation_cache_dir", "/tmp/jax_cc_cache")
jax.config.update("jax_persistent_cache_min_compile_time_secs", 0.0)
jax.config.update("jax_persistent_cache_min_entry_size_bytes", 0)

import concourse.bass as bass
import concourse.bacc as bacc
import concourse.mybir as mybir
from concourse.tile import TileContext

F32 = mybir.dt.float32
BF16 = mybir.dt.bfloat16
FP16 = mybir.dt.float16

S, HID = 2048, 2048
H, DN, DR, DV = 16, 128, 64, 128
QLR, KVLR = 1024, 512
IH, IHD, TOPK = 8, 64, 256
NEG = -1e9
NB = 256            # query rows per core
NCORES = 8
NT = S // 128       # 16 token tiles
NQT = NB // 128     # 2 query tiles per core
SCALE_MLA = float((DN + DR) ** -0.5)
SCALE_IDX = float(IHD ** -0.5)
SCALE_GATE = float(IH ** -0.5)
ALIGN = 512         # element alignment for blob entries

I8_LAYOUT = [
    ("xT_lo", (HID, S)),
    ("wqa_lo", (HID, QLR)),
]
I8_OFF, I8_TOT = None, None  # filled below

BF_LAYOUT = [
    ("xT_hi", (HID, S)),
    ("wqa_hi", (HID, QLR)),
    ("wq_b", (QLR, H * (DN + DR))),
    ("wkv_a", (HID, KVLR + DR)),
    ("wkv_b", (KVLR, H * (DN + DV))),
    ("wo", (H * DV, HID)),
    ("iwqb_hi", (QLR, IH * IHD)), ("iwqb_lo", (QLR, IH * IHD)),
    ("iwk_hi", (HID, IHD)), ("iwk_lo", (HID, IHD)),
    ("igate_hi", (HID, IH)), ("igate_lo", (HID, IH)),
    ("ident", (128, 128)),
]

F32_LAYOUT_BASE = [
    ("cos", (S, DR)), ("sin", (S, DR)),
    ("q_norm_w", (1, QLR)), ("kv_norm_w", (1, KVLR)),
    ("idx_knorm_w", (1, IHD)), ("idx_knorm_b", (1, IHD)),
    ("iota", (1, S)),
    ("rows", (NCORES * 128, 1)),
    ("xlo_scale", (HID, 1)),
    ("walo_scale", (HID, 1)),
]


def _mk_layout(entries):
    offs, off = {}, 0
    for name, shape in entries:
        offs[name] = off
        n = int(np.prod(shape))
        off += (n + ALIGN - 1) // ALIGN * ALIGN
    tot = (off + NCORES * ALIGN - 1) // (NCORES * ALIGN) * (NCORES * ALIGN)
    return offs, tot


def _f32_layout(causal):
    return _mk_layout(list(F32_LAYOUT_BASE))


BF_OFF, BF_TOT = _mk_layout(BF_LAYOUT)
LB = BF_TOT // NCORES
I8_OFF, I8_TOT = _mk_layout(I8_LAYOUT)
LI = I8_TOT // NCORES


def _v(blob1d, off, r, c):
    """[r, c] row-major view at element offset off of a 1-D DRAM AP."""
    return blob1d[off:off + r * c].rearrange("(r c) -> r c", c=c)


def _vb(blob1d, off, n, parts=128):
    """Partition-broadcast view [parts, n] of n elements at offset off."""
    return bass.AP(tensor=blob1d.tensor, offset=blob1d.offset + off,
                   ap=[[0, parts], [1, n]])


def _rmsnorm_from_psum(nc, pool, out_sb, psums, wb, d, eps=1e-6):
    """out_sb[p, d] = psum * rsqrt(mean(psum^2)+eps) * w."""
    ssq = pool.tile([128, len(psums)], F32)
    for i, ps in enumerate(psums):
        w = ps.shape[-1]
        scr = pool.tile([128, 512], F32, tag="rms_scr")
        nc.scalar.activation(out=scr[:, :w], in_=ps,
                             func=mybir.ActivationFunctionType.Square,
                             accum_out=ssq[:, i:i + 1])
    tot = pool.tile([128, 1], F32)
    if len(psums) == 1:
        nc.vector.tensor_scalar(out=tot, in0=ssq, scalar1=1.0 / d,
                                scalar2=eps, op0=mybir.AluOpType.mult,
                                op1=mybir.AluOpType.add)
    else:
        nc.vector.tensor_reduce(out=tot, in_=ssq, axis=mybir.AxisListType.X,
                                op=mybir.AluOpType.add)
        nc.vector.tensor_scalar(out=tot, in0=tot, scalar1=1.0 / d,
                                scalar2=eps, op0=mybir.AluOpType.mult,
                                op1=mybir.AluOpType.add)
    nc.scalar.activation(out=tot, in_=tot,
                         func=mybir.ActivationFunctionType.Sqrt)
    rinv = pool.tile([128, 1], F32)
    nc.vector.reciprocal(out=rinv, in_=tot)
    off = 0
    for ps in psums:
        w = ps.shape[-1]
        nc.vector.tensor_scalar(out=out_sb[:, off:off + w], in0=ps,
                                scalar1=rinv, scalar2=None,
                                op0=mybir.AluOpType.mult)
        off += w
    nc.vector.tensor_mul(out_sb[:, :d], out_sb[:, :d], wb[:, :d])


def _rope_int(nc, out, in_, cos, sin):
    """Interleaved (GPT-J) rope, token-major [128, 64] -> out[128, 64]."""
    xp = in_.rearrange("p (a b) -> p a b", b=2)
    op = out.rearrange("p (a b) -> p a b", b=2)
    c, s = cos[:, 0:32], sin[:, 0:32]
    x1, x2 = xp[:, :, 0], xp[:, :, 1]
    nc.vector.tensor_mul(op[:, :, 0], x1, c)
    nc.vector.tensor_mul(op[:, :, 1], x2, c)
    t = nc._rope_scr.tile([128, 32], F32, tag="rope_t")
    nc.vector.tensor_mul(t, x2, s)
    nc.vector.tensor_sub(op[:, :, 0], op[:, :, 0], t)
    nc.vector.tensor_mul(t, x1, s)
    nc.vector.tensor_add(op[:, :, 1], op[:, :, 1], t)


def _rope_ni(nc, out, in_, cos, sin):
    """Non-interleaved (rotate_half) rope, [128, 64]."""
    x1, x2 = in_[:, 0:32], in_[:, 32:64]
    c1, c2 = cos[:, 0:32], cos[:, 32:64]
    s1, s2 = sin[:, 0:32], sin[:, 32:64]
    nc.vector.tensor_mul(out[:, 0:32], x1, c1)
    nc.vector.tensor_mul(out[:, 32:64], x2, c2)
    t = nc._rope_scr.tile([128, 32], F32, tag="rope_t")
    nc.vector.tensor_mul(t, x2, s1)
    nc.vector.tensor_sub(out[:, 0:32], out[:, 0:32], t)
    nc.vector.tensor_mul(t, x1, s2)
    nc.vector.tensor_add(out[:, 32:64], out[:, 32:64], t)


def _split(nc, pool, src_f32, n, tag):
    """f32 [128, n] -> (hi bf16, lo bf16) with hi+lo ~= src."""
    hi = pool.tile([128, n], BF16, tag=tag + "_hi")
    nc.vector.tensor_copy(hi, src_f32)
    hi32 = pool.tile([128, n], F32, tag=tag + "_h32")
    nc.vector.tensor_copy(hi32, hi)
    lo32 = pool.tile([128, n], F32, tag=tag + "_l32")
    nc.vector.tensor_sub(lo32, src_f32, hi32)
    lo = pool.tile([128, n], BF16, tag=tag + "_lo")
    nc.vector.tensor_copy(lo, lo32)
    return hi, lo


class _Bacc(bacc.Bacc):
    """Bacc with memoized BIR serialization: run_bass_kernel_spmd re-lowers
    (and re-serializes the ~9 MB BIR) on every call; the module is frozen
    after compile(), so the bytes are reusable."""
    _json_cache = None

    def to_json_bytes(self):
        if self._json_cache is None:
            self._json_cache = super().to_json_bytes()
        return self._json_cache


def build_nc(causal=True, dbg=False):
    F32_OFF, F32_TOT = _f32_layout(causal)
    lf = F32_TOT // NCORES

    nc = _Bacc("TRN2", target_bir_lowering=False, debug=False)
    shard_bf = nc.dram_tensor("shard_bf", [LB], BF16, kind="ExternalInput").ap()
    shard_i8 = nc.dram_tensor("shard_i8", [LI], mybir.dt.int8,
                              kind="ExternalInput").ap()
    shard_f32 = nc.dram_tensor("shard_f32", [lf], F32,
                               kind="ExternalInput").ap()
    outT = nc.dram_tensor("outT", [HID, NB], FP16, kind="ExternalOutput").ap()
    amask_d = None
    if not causal:
        amask_d = nc.dram_tensor("amask_rows", [NB, S], F32,
                                 kind="ExternalInput").ap()
    if dbg:
        d_ckvT = nc.dram_tensor("d_ckvT", [128, 4 * S], BF16,
                                kind="ExternalOutput").ap()
        d_kpeT = nc.dram_tensor("d_kpeT", [64, S], BF16,
                                kind="ExternalOutput").ap()
        d_kiT = nc.dram_tensor("d_kiT", [64, 2 * S], BF16,
                               kind="ExternalOutput").ap()
        d_qrT = nc.dram_tensor("d_qrT", [128, 2 * 8 * NB], BF16,
                               kind="ExternalOutput").ap()
        d_gate = nc.dram_tensor("d_gate", [128, NQT * IH], F32,
                                kind="ExternalOutput").ap()
        d_mask = nc.dram_tensor("d_mask", [128, NQT * S], F32,
                                kind="ExternalOutput").ap()
        d_thr = nc.dram_tensor("d_thr", [128, NQT], F32,
                               kind="ExternalOutput").ap()
        d_qiT = nc.dram_tensor("d_qiT", [64, 2 * IH * NB], BF16,
                               kind="ExternalOutput").ap()
        d_ohT = nc.dram_tensor("d_ohT", [128, H * NB], BF16,
                               kind="ExternalOutput").ap()

    with TileContext(nc) as tc:
        pid = nc.partition_id()
        r0 = pid * NB

        dram = tc.alloc_tile_pool(name="dram", bufs=1, space="DRAM")
        bounce_bf = dram.tile([LB], BF16)
        bounce_i8 = dram.tile([LI], mybir.dt.int8)
        bounce_f32 = dram.tile([lf], F32)
        gath_bf = nc.dram_tensor("gath_bf", [NCORES, LB], BF16,
                                 kind="Internal", addr_space="Shared").ap()
        gath_f32 = nc.dram_tensor("gath_f32", [NCORES, lf], F32,
                                  kind="Internal", addr_space="Shared").ap()
        gath_i8 = nc.dram_tensor("gath_i8", [NCORES, LI], mybir.dt.int8,
                                 kind="Internal", addr_space="Shared").ap()
        nc.gpsimd.dma_start(out=bounce_i8, in_=shard_i8)
        nc.gpsimd.collective_compute(
            "AllGather", mybir.AluOpType.bypass,
            replica_groups=[list(range(NCORES))],
            ins=[bounce_i8[:].opt()], outs=[gath_i8[:].opt()])
        nc.gpsimd.dma_start(out=bounce_bf, in_=shard_bf)
        nc.gpsimd.dma_start(out=bounce_f32, in_=shard_f32)
        nc.gpsimd.collective_compute(
            "AllGather", mybir.AluOpType.bypass,
            replica_groups=[list(range(NCORES))],
            ins=[bounce_bf[:].opt()], outs=[gath_bf[:].opt()])
        nc.gpsimd.collective_compute(
            "AllGather", mybir.AluOpType.bypass,
            replica_groups=[list(range(NCORES))],
            ins=[bounce_f32[:].opt()], outs=[gath_f32[:].opt()])
        gb = gath_bf.rearrange("a b -> (a b)")
        gf = gath_f32.rearrange("a b -> (a b)")
        gi = gath_i8.rearrange("a b -> (a b)")

        xTh_v = _v(gb, BF_OFF["xT_hi"], HID, S)
        xTl_v = _v(gi, I8_OFF["xT_lo"], HID, S)
        wqah_v = _v(gb, BF_OFF["wqa_hi"], HID, QLR)
        wqal_v = _v(gi, I8_OFF["wqa_lo"], HID, QLR)
        wqb_v = _v(gb, BF_OFF["wq_b"], QLR, H * (DN + DR))
        wkva_v = _v(gb, BF_OFF["wkv_a"], HID, KVLR + DR)
        wkvb_v = _v(gb, BF_OFF["wkv_b"], KVLR, H * (DN + DV))
        wo_v = _v(gb, BF_OFF["wo"], H * DV, HID)
        iwqbh_v = _v(gb, BF_OFF["iwqb_hi"], QLR, IH * IHD)
        iwqbl_v = _v(gb, BF_OFF["iwqb_lo"], QLR, IH * IHD)
        iwkh_v = _v(gb, BF_OFF["iwk_hi"], HID, IHD)
        iwkl_v = _v(gb, BF_OFF["iwk_lo"], HID, IHD)
        igh_v = _v(gb, BF_OFF["igate_hi"], HID, IH)
        igl_v = _v(gb, BF_OFF["igate_lo"], HID, IH)
        ident_v = _v(gb, BF_OFF["ident"], 128, 128)
        cos_v = _v(gf, F32_OFF["cos"], S, DR)
        sin_v = _v(gf, F32_OFF["sin"], S, DR)
        rows_v = _v(gf, F32_OFF["rows"], NCORES * 128, 1)

        consts = tc.alloc_tile_pool(name="consts", bufs=1)
        nc._rope_scr = consts

        ident = consts.tile([128, 128], BF16)
        nc.gpsimd.dma_start(out=ident, in_=ident_v)
        kvnw = consts.tile([128, KVLR], F32)
        nc.gpsimd.dma_start(out=kvnw, in_=_vb(gf, F32_OFF["kv_norm_w"], KVLR))
        knw = consts.tile([128, IHD], F32)
        nc.gpsimd.dma_start(out=knw, in_=_vb(gf, F32_OFF["idx_knorm_w"], IHD))
        knb = consts.tile([128, IHD], F32)
        nc.gpsimd.dma_start(out=knb, in_=_vb(gf, F32_OFF["idx_knorm_b"], IHD))
        iota_sb = consts.tile([128, S], F32)
        nc.gpsimd.dma_start(out=iota_sb, in_=_vb(gf, F32_OFF["iota"], S))
        rowid = consts.tile([128, 1], F32)
        nc.gpsimd.dma_start(out=rowid, in_=rows_v[bass.ds(pid * 128, 128), :])
        xls = consts.tile([128, NT], F32)
        xls_v = _v(gf, F32_OFF["xlo_scale"], HID, 1)
        nc.gpsimd.dma_start(
            out=xls, in_=xls_v.rearrange("(c p) o -> p (c o)", p=128))
        wals = consts.tile([128, NT], F32)
        wals_v = _v(gf, F32_OFF["walo_scale"], HID, 1)
        nc.gpsimd.dma_start(
            out=wals, in_=wals_v.rearrange("(c p) o -> p (c o)", p=128))

        ckvT = consts.tile([128, 4, S], BF16)      # [ckv_chunk, 4, tok]
        kpeT = consts.tile([64, S], BF16)
        kiT_hi = consts.tile([64, S], BF16)
        kiT_lo = consts.tile([64, S], BF16)

        # ---------------- P1: KV / indexer-key expansion ----------------
        with tc.tile_pool(name="p1w", bufs=1) as p1w, \
             tc.tile_pool(name="p1", bufs=3) as p1, \
             tc.tile_pool(name="p1ps", bufs=2, space="PSUM") as p1ps, \
             tc.tile_pool(name="p1tr", bufs=2, space="PSUM") as p1tr:
            cos_t = p1w.tile([128, NT, DR], F32)
            sin_t = p1w.tile([128, NT, DR], F32)
            cr = cos_v.rearrange("(t p) d -> p t d", p=128)
            sr = sin_v.rearrange("(t p) d -> p t d", p=128)
            wkva_sb = p1w.tile([128, NT, KVLR], BF16)
            wr = wkva_v.rearrange("(c p) n -> p c n", p=128)
            # wcat: [k_pe cols of wkv_a | iwk_hi | iwk_lo]
            wcat = p1w.tile([128, NT, DR + 2 * IHD], BF16)
            ikh = iwkh_v.rearrange("(c p) n -> p c n", p=128)
            ikl = iwkl_v.rearrange("(c p) n -> p c n", p=128)
            for c in range(NT):
                nc.gpsimd.dma_start(out=cos_t[:, c, :], in_=cr[:, c, :])
                nc.gpsimd.dma_start(out=sin_t[:, c, :], in_=sr[:, c, :])
                nc.gpsimd.dma_start(out=wkva_sb[:, c, :],
                                    in_=wr[:, c, 0:KVLR])
                nc.gpsimd.dma_start(out=wcat[:, c, 0:DR],
                                    in_=wr[:, c, KVLR:])
                nc.gpsimd.dma_start(out=wcat[:, c, DR:DR + IHD],
                                    in_=ikh[:, c, :])
                nc.gpsimd.dma_start(out=wcat[:, c, DR + IHD:],
                                    in_=ikl[:, c, :])

            xrh = xTh_v.rearrange("(c p) (u q) -> p c u q", p=128, q=128)
            xrl = xTl_v.rearrange("(c p) (u q) -> p c u q", p=128, q=128)
            for t in range(NT):
                xt_hi = p1.tile([128, NT, 128], BF16, tag="xt_hi")
                xt_q = p1.tile([128, NT, 128], mybir.dt.int8, tag="xt_q")
                xt_lo = p1.tile([128, NT, 128], BF16, tag="xt_lo")
                nc.gpsimd.dma_start(out=xt_hi, in_=xrh[:, :, t, :])
                nc.gpsimd.dma_start(out=xt_q, in_=xrl[:, :, t, :])
                for c in range(NT):
                    nc.vector.tensor_scalar(
                        out=xt_lo[:, c, :], in0=xt_q[:, c, :],
                        scalar1=xls[:, c:c + 1], scalar2=None,
                        op0=mybir.AluOpType.mult)
                ps_kv = p1ps.tile([128, KVLR], F32, tag="ps_kv")
                ps_x = p1ps.tile([128, DR + 2 * IHD], F32, tag="ps_x")
                ps_kl = p1ps.tile([128, IHD], F32, tag="ps_kl")
                for f in range(NT):
                    st, sp = (f == 0), (f == NT - 1)
                    nc.tensor.matmul(ps_kv, xt_hi[:, f, :],
                                     wkva_sb[:, f, :], start=st, stop=sp)
                    nc.tensor.matmul(ps_x, xt_hi[:, f, :],
                                     wcat[:, f, :], start=st, stop=sp)
                    nc.tensor.matmul(ps_kl, xt_lo[:, f, :],
                                     wcat[:, f, DR:DR + IHD],
                                     start=st, stop=sp)
                # ckv rmsnorm -> bf16 -> transpose into ckvT
                ckv_sb = p1.tile([128, KVLR], F32, tag="ckv_sb")
                _rmsnorm_from_psum(nc, p1, ckv_sb, [ps_kv], kvnw, KVLR)
                ckv_bf = p1.tile([128, KVLR], BF16, tag="ckv_bf")
                nc.vector.tensor_copy(ckv_bf, ckv_sb)
                for ch in range(4):
                    ptr = p1tr.tile([128, 128], BF16, tag="ptr")
                    nc.tensor.transpose(ptr, ckv_bf[:, ch * 128:(ch + 1) * 128],
                                        ident)
                    nc.scalar.copy(out=ckvT[:, ch, t * 128:(t + 1) * 128],
                                   in_=ptr)
                # k_pe rope -> bf16 -> transpose into kpeT
                pe_sb = p1.tile([128, DR], F32, tag="pe_sb")
                _rope_int(nc, pe_sb, ps_x[:, 0:DR],
                          cos_t[:, t, :], sin_t[:, t, :])
                pe_bf = p1.tile([128, DR], BF16, tag="pe_bf")
                nc.vector.tensor_copy(pe_bf, pe_sb)
                ptr = p1tr.tile([128, 128], BF16, tag="ptr")
                nc.tensor.transpose(ptr[:64, :], pe_bf, ident)
                nc.scalar.copy(out=kpeT[:, t * 128:(t + 1) * 128],
                               in_=ptr[:64, :])
                # ki = layernorm(3-pass sum) + rope -> split -> transpose
                ki32 = p1.tile([128, IHD], F32, tag="ki32")
                nc.scalar.copy(out=ki32, in_=ps_x[:, DR:DR + IHD])
                nc.vector.tensor_add(ki32, ki32, ps_x[:, DR + IHD:])
                nc.vector.tensor_add(ki32, ki32, ps_kl)
                s1 = p1.tile([128, 2], F32, tag="ki_s")
                scr = p1.tile([128, IHD], F32, tag="ki_scr")
                nc.scalar.activation(out=scr, in_=ki32,
                                     func=mybir.ActivationFunctionType.Copy,
                                     accum_out=s1[:, 0:1])
                nc.scalar.activation(out=scr, in_=ki32,
                                     func=mybir.ActivationFunctionType.Square,
                                     accum_out=s1[:, 1:2])
                mom = p1.tile([128, 4], F32, tag="ki_m")
                nc.vector.tensor_scalar(out=mom[:, 0:1], in0=s1[:, 0:1],
                                        scalar1=1.0 / IHD, scalar2=None,
                                        op0=mybir.AluOpType.mult)
                nc.vector.tensor_scalar(out=mom[:, 1:2], in0=s1[:, 1:2],
                                        scalar1=1.0 / IHD, scalar2=None,
                                        op0=mybir.AluOpType.mult)
                nc.vector.tensor_mul(mom[:, 2:3], mom[:, 0:1], mom[:, 0:1])
                nc.vector.tensor_sub(mom[:, 2:3], mom[:, 1:2], mom[:, 2:3])
                nc.vector.tensor_scalar(out=mom[:, 2:3], in0=mom[:, 2:3],
                                        scalar1=1e-5, scalar2=None,
                                        op0=mybir.AluOpType.add)
                nc.scalar.activation(out=mom[:, 2:3], in_=mom[:, 2:3],
                                     func=mybir.ActivationFunctionType.Sqrt)
                nc.vector.reciprocal(out=mom[:, 3:4], in_=mom[:, 2:3])
                ki_n = p1.tile([128, IHD], F32, tag="ki_n")
                nc.vector.tensor_scalar(out=ki_n, in0=ki32,
                                        scalar1=mom[:, 0:1],
                                        scalar2=mom[:, 3:4],
                                        op0=mybir.AluOpType.subtract,
                                        op1=mybir.AluOpType.mult)
                nc.vector.tensor_mul(ki_n, ki_n, knw)
                nc.vector.tensor_add(ki_n, ki_n, knb)
                ki_r = p1.tile([128, IHD], F32, tag="ki_r")
                _rope_ni(nc, ki_r, ki_n, cos_t[:, t, :], sin_t[:, t, :])
                ki_hi, ki_lo = _split(nc, p1, ki_r, IHD, "ki")
                ptr = p1tr.tile([128, 128], BF16, tag="ptr")
                nc.tensor.transpose(ptr[:64, :], ki_hi, ident)
                nc.scalar.copy(out=kiT_hi[:, t * 128:(t + 1) * 128],
                               in_=ptr[:64, :])
                ptr = p1tr.tile([128, 128], BF16, tag="ptr")
                nc.tensor.transpose(ptr[:64, :], ki_lo, ident)
                nc.scalar.copy(out=kiT_lo[:, t * 128:(t + 1) * 128],
                               in_=ptr[:64, :])

        if dbg:
            nc.gpsimd.dma_start(out=d_ckvT,
                                in_=ckvT.rearrange("p a b -> p (a b)"))
            nc.gpsimd.dma_start(out=d_kpeT, in_=kpeT)
            nc.gpsimd.dma_start(out=d_kiT[:, 0:S], in_=kiT_hi)
            nc.gpsimd.dma_start(out=d_kiT[:, S:], in_=kiT_lo)

        # ---------------- P2: query-block projections ----------------
        mid = tc.alloc_tile_pool(name="mid", bufs=1)
        qTn = mid.tile([128, H, NB], BF16)       # nope part, feature-major
        qTp = mid.tile([64, H, NB], BF16)        # rope part
        qiT_hi = mid.tile([64, IH, NB], BF16)
        qiT_lo = mid.tile([64, IH, NB], BF16)

        with tc.tile_pool(name="p2w", bufs=2) as p2w, \
             tc.tile_pool(name="p2", bufs=2) as p2, \
             tc.tile_pool(name="p2ps", bufs=1, space="PSUM") as p2ps, \
             tc.tile_pool(name="p2tr", bufs=1, space="PSUM") as p2tr:
            cosb = p2.tile([128, NQT, DR], F32, tag="cosb", bufs=1)
            sinb = p2.tile([128, NQT, DR], F32, tag="sinb", bufs=1)
            for q in range(NQT):
                nc.gpsimd.dma_start(
                    out=cosb[:, q, :],
                    in_=cos_v[bass.ds(r0 + q * 128, 128), :])
                nc.gpsimd.dma_start(
                    out=sinb[:, q, :],
                    in_=sin_v[bass.ds(r0 + q * 128, 128), :])
            qnw = p2.tile([128, QLR], F32, tag="qnw", bufs=1)
            nc.gpsimd.dma_start(out=qnw, in_=_vb(gf, F32_OFF["q_norm_w"], QLR))
            gcat_w = p2.tile([128, NT, 2 * IH], BF16, tag="gcat", bufs=1)
            igh_r = igh_v.rearrange("(c p) n -> p c n", p=128)
            igl_r = igl_v.rearrange("(c p) n -> p c n", p=128)
            for c in range(NT):
                nc.gpsimd.dma_start(out=gcat_w[:, c, 0:IH], in_=igh_r[:, c, :])
                nc.gpsimd.dma_start(out=gcat_w[:, c, IH:], in_=igl_r[:, c, :])
            ps_qr = [p2ps.tile([128, 512], F32, tag=f"ps_qr{q}{i}",
                               name=f"ps_qr{q}{i}")
                     for q in range(NQT) for i in range(2)]
            ps_g = [p2ps.tile([128, 2 * IH], F32, tag=f"ps_g{q}",
                              name=f"ps_g{q}") for q in range(NQT)]
            for f in range(NT):
                wqah_f = p2w.tile([128, QLR], BF16, tag="wqah_f")
                nc.gpsimd.dma_start(out=wqah_f,
                                    in_=wqah_v[f * 128:(f + 1) * 128, :])
                wqal_q = p2w.tile([128, QLR], mybir.dt.int8,
                                  tag="wqal_q")
                nc.gpsimd.dma_start(out=wqal_q,
                                    in_=wqal_v[f * 128:(f + 1) * 128, :])
                wqal_f = p2w.tile([128, QLR], BF16, tag="wqal_f")
                nc.vector.tensor_scalar(out=wqal_f, in0=wqal_q,
                                        scalar1=wals[:, f:f + 1],
                                        scalar2=None,
                                        op0=mybir.AluOpType.mult)
                xq_hi = p2w.tile([128, NB], BF16, tag="xq_hi", bufs=3)
                nc.gpsimd.dma_start(
                    out=xq_hi,
                    in_=xTh_v[f * 128:(f + 1) * 128, bass.ds(r0, NB)])
                xq_q = p2w.tile([128, NB], mybir.dt.int8, tag="xq_q",
                                bufs=3)
                nc.gpsimd.dma_start(
                    out=xq_q,
                    in_=xTl_v[f * 128:(f + 1) * 128, bass.ds(r0, NB)])
                xq_lo = p2w.tile([128, NB], BF16, tag="xq_lo", bufs=3)
                nc.vector.tensor_scalar(out=xq_lo, in0=xq_q,
                                        scalar1=xls[:, f:f + 1], scalar2=None,
                                        op0=mybir.AluOpType.mult)
                st, sp = (f == 0), (f == NT - 1)
                for q in range(NQT):
                    lhs_hi = xq_hi[:, q * 128:(q + 1) * 128]
                    lhs_lo = xq_lo[:, q * 128:(q + 1) * 128]
                    for i in range(2):
                        cols = slice(i * 512, (i + 1) * 512)
                        nc.tensor.matmul(ps_qr[2 * q + i], lhs_hi,
                                         wqah_f[:, cols], start=st, stop=False)
                        nc.tensor.matmul(ps_qr[2 * q + i], lhs_hi,
                                         wqal_f[:, cols], start=False,
                                         stop=False)
                        nc.tensor.matmul(ps_qr[2 * q + i], lhs_lo,
                                         wqah_f[:, cols], start=False, stop=sp)
                    nc.tensor.matmul(ps_g[q][:, 0:2 * IH], lhs_hi,
                                     gcat_w[:, f, :], start=st, stop=False)
                    nc.tensor.matmul(ps_g[q][:, 0:IH], lhs_lo,
                                     gcat_w[:, f, 0:IH], start=False, stop=sp)
            qrT_hi = p2.tile([128, 8, NB], BF16, tag="qrT_hi", bufs=1)
            qrT_lo = p2.tile([128, 8, NB], BF16, tag="qrT_lo", bufs=1)
            gate_sb = p2.tile([128, NQT, IH], F32, tag="gate_sb", bufs=1)
            for q in range(NQT):
                qr_sb = p2.tile([128, QLR], F32, tag="qr_sb")
                _rmsnorm_from_psum(nc, p2, qr_sb,
                                   [ps_qr[2 * q], ps_qr[2 * q + 1]], qnw, QLR)
                nc.scalar.copy(out=gate_sb[:, q, :], in_=ps_g[q][:, 0:IH])
                nc.vector.tensor_add(gate_sb[:, q, :], gate_sb[:, q, :],
                                     ps_g[q][:, IH:2 * IH])
                nc.vector.tensor_scalar(out=gate_sb[:, q, :],
                                        in0=gate_sb[:, q, :],
                                        scalar1=SCALE_GATE * SCALE_IDX,
                                        scalar2=None,
                                        op0=mybir.AluOpType.mult)
                qr_hi, qr_lo = _split(nc, p2, qr_sb, QLR, "qr")
                for ch in range(8):
                    cols = slice(ch * 128, (ch + 1) * 128)
                    ptr = p2tr.tile([128, 128], BF16, tag="ptr2")
                    nc.tensor.transpose(ptr, qr_hi[:, cols], ident)
                    nc.scalar.copy(out=qrT_hi[:, ch, q * 128:(q + 1) * 128],
                                   in_=ptr)
                    ptr = p2tr.tile([128, 128], BF16, tag="ptr2")
                    nc.tensor.transpose(ptr, qr_lo[:, cols], ident)
                    nc.scalar.copy(out=qrT_lo[:, ch, q * 128:(q + 1) * 128],
                                   in_=ptr)
            # q projection per MLA head (bf16)
            wqbr = wqb_v.rearrange("(c p) n -> p c n", p=128)
            for h in range(H):
                wqb_h = p2w.tile([128, 8, DN + DR], BF16, tag="wqb_h")
                for c in range(8):
                    nc.gpsimd.dma_start(
                        out=wqb_h[:, c, :],
                        in_=wqbr[:, c, h * (DN + DR):(h + 1) * (DN + DR)])
                for q in range(NQT):
                    ps_q = p2ps.tile([128, DN + DR], F32, tag="ps_q")
                    for ch in range(8):
                        nc.tensor.matmul(
                            ps_q, qrT_hi[:, ch, q * 128:(q + 1) * 128],
                            wqb_h[:, ch, :],
                            start=(ch == 0), stop=(ch == 7))
                    qn_bf = p2.tile([128, DN], BF16, tag="qn_bf")
                    nc.vector.tensor_scalar(out=qn_bf, in0=ps_q[:, 0:DN],
                                            scalar1=SCALE_MLA, scalar2=None,
                                            op0=mybir.AluOpType.mult)
                    qp32 = p2.tile([128, DR], F32, tag="qp32")
                    _rope_int(nc, qp32, ps_q[:, DN:],
                              cosb[:, q, :], sinb[:, q, :])
                    qp_bf = p2.tile([128, DR], BF16, tag="qp_bf")
                    nc.vector.tensor_scalar(out=qp_bf, in0=qp32,
                                            scalar1=SCALE_MLA, scalar2=None,
                                            op0=mybir.AluOpType.mult)
                    ptr = p2tr.tile([128, 128], BF16, tag="ptr2")
                    nc.tensor.transpose(ptr, qn_bf, ident)
                    nc.scalar.copy(out=qTn[:, h, q * 128:(q + 1) * 128],
                                   in_=ptr)
                    ptr = p2tr.tile([128, 128], BF16, tag="ptr2")
                    nc.tensor.transpose(ptr[:64, :], qp_bf, ident)
                    nc.scalar.copy(out=qTp[:, h, q * 128:(q + 1) * 128],
                                   in_=ptr[:64, :])
            # indexer q heads: 3-pass hi/lo, rope, * gate, split, transpose
            iwqbh_r = iwqbh_v.rearrange("(c p) n -> p c n", p=128)
            iwqbl_r = iwqbl_v.rearrange("(c p) n -> p c n", p=128)
            for ih in range(IH):
                wiq_cat = p2w.tile([128, 8, 2 * IHD], BF16, tag="wiq_cat")
                for c in range(8):
                    nc.gpsimd.dma_start(
                        out=wiq_cat[:, c, 0:IHD],
                        in_=iwqbh_r[:, c, ih * IHD:(ih + 1) * IHD])
                    nc.gpsimd.dma_start(
                        out=wiq_cat[:, c, IHD:],
                        in_=iwqbl_r[:, c, ih * IHD:(ih + 1) * IHD])
                for q in range(NQT):
                    ps_qc = p2ps.tile([128, 2 * IHD], F32, tag="ps_q")
                    for ch in range(8):
                        nc.tensor.matmul(
                            ps_qc[:, 0:2 * IHD],
                            qrT_hi[:, ch, q * 128:(q + 1) * 128],
                            wiq_cat[:, ch, :],
                            start=(ch == 0), stop=False)
                        nc.tensor.matmul(
                            ps_qc[:, 0:IHD],
                            qrT_lo[:, ch, q * 128:(q + 1) * 128],
                            wiq_cat[:, ch, 0:IHD],
                            start=False, stop=(ch == 7))
                    qi32 = p2.tile([128, IHD], F32, tag="qi32")
                    nc.scalar.copy(out=qi32, in_=ps_qc[:, 0:IHD])
                    nc.vector.tensor_add(qi32, qi32, ps_qc[:, IHD:2 * IHD])
                    qi_r = p2.tile([128, IHD], F32, tag="qi_r")
                    _rope_ni(nc, qi_r, qi32, cosb[:, q, :], sinb[:, q, :])
                    nc.vector.tensor_scalar(out=qi_r, in0=qi_r,
                                            scalar1=gate_sb[:, q, ih:ih + 1],
                                            scalar2=None,
                                            op0=mybir.AluOpType.mult)
                    qi_hi, qi_lo = _split(nc, p2, qi_r, IHD, "qi")
                    ptr = p2tr.tile([128, 128], BF16, tag="ptr2")
                    nc.tensor.transpose(ptr[:64, :], qi_hi, ident)
                    nc.scalar.copy(out=qiT_hi[:, ih, q * 128:(q + 1) * 128],
                                   in_=ptr[:64, :])
                    ptr = p2tr.tile([128, 128], BF16, tag="ptr2")
                    nc.tensor.transpose(ptr[:64, :], qi_lo, ident)
                    nc.scalar.copy(out=qiT_lo[:, ih, q * 128:(q + 1) * 128],
                                   in_=ptr[:64, :])
            if dbg:
                nc.gpsimd.dma_start(out=d_qrT[:, 0:8 * NB],
                                    in_=qrT_hi.rearrange("p a b -> p (a b)"))
                nc.gpsimd.dma_start(out=d_qrT[:, 8 * NB:],
                                    in_=qrT_lo.rearrange("p a b -> p (a b)"))
                nc.gpsimd.dma_start(out=d_gate,
                                    in_=gate_sb.rearrange("p a b -> p (a b)"))
                nc.gpsimd.dma_start(out=d_qiT[:, 0:IH * NB],
                                    in_=qiT_hi.rearrange("p a b -> p (a b)"))
                nc.gpsimd.dma_start(out=d_qiT[:, IH * NB:],
                                    in_=qiT_lo.rearrange("p a b -> p (a b)"))

        # ---------------- P3: index scores + EXACT top-k ----------------
        maskNEG = mid.tile([128, NQT, S], F32)
        with tc.tile_pool(name="p3", bufs=1) as p3, \
             tc.tile_pool(name="p3ps", bufs=4, space="PSUM") as p3ps:
            for q in range(NQT):
                cm = p3.tile([128, S], F32, tag="cm")
                if causal:
                    # cmask = (col > row) * NEG
                    rq = p3.tile([128, 1], F32, tag="rq")
                    nc.vector.tensor_scalar(out=rq, in0=rowid,
                                            scalar1=float(q * 128),
                                            scalar2=None,
                                            op0=mybir.AluOpType.add)
                    nc.vector.tensor_scalar(out=cm, in0=iota_sb,
                                            scalar1=rq, scalar2=NEG,
                                            op0=mybir.AluOpType.is_gt,
                                            op1=mybir.AluOpType.mult)
                else:
                    nc.gpsimd.dma_start(
                        out=cm, in_=amask_d[q * 128:(q + 1) * 128, :])
                isc = p3.tile([128, S], F32, tag="isc")
                for kc in range(4):
                    cols = slice(kc * 512, (kc + 1) * 512)
                    ps = p3ps.tile([128, 512], F32, tag="ps_isc")
                    for ih in range(IH):
                        qcols = slice(q * 128, (q + 1) * 128)
                        nc.tensor.matmul(ps, qiT_hi[:, ih, qcols],
                                         kiT_hi[:, cols],
                                         start=(ih == 0), stop=False)
                        nc.tensor.matmul(ps, qiT_hi[:, ih, qcols],
                                         kiT_lo[:, cols],
                                         start=False, stop=False)
                        nc.tensor.matmul(ps, qiT_lo[:, ih, qcols],
                                         kiT_hi[:, cols],
                                         start=False, stop=(ih == IH - 1))
                    nc.vector.tensor_add(isc[:, cols], ps, cm[:, cols])
                # clamp; masked cols sit at -200 (amask re-kills them later)
                nc.vector.tensor_scalar(out=isc, in0=isc, scalar1=-200.0,
                                        scalar2=None, op0=mybir.AluOpType.max)
                # exact top-256 threshold: 32 rounds of top-8 + replace
                scr = p3.tile([128, S], F32, tag="sel_scr")
                nc.vector.tensor_copy(scr, isc)
                mx = p3.tile([128, 8], F32, tag="mx")
                for r in range(TOPK // 8):
                    nc.vector.max(out=mx, in_=scr)
                    if r < TOPK // 8 - 1:
                        nc.vector.match_replace(out=scr, in_to_replace=mx,
                                                in_values=scr, imm_value=-3e9)
                nc.vector.tensor_scalar(out=maskNEG[:, q, :], in0=isc,
                                        scalar1=mx[:, 7:8], scalar2=NEG,
                                        op0=mybir.AluOpType.is_lt,
                                        op1=mybir.AluOpType.mult)
                nc.vector.tensor_add(maskNEG[:, q, :], maskNEG[:, q, :], cm)
                if dbg:
                    nc.gpsimd.dma_start(out=d_thr[:, q:q + 1], in_=mx[:, 7:8])

        if dbg:
            nc.gpsimd.dma_start(out=d_mask,
                                in_=maskNEG.rearrange("p a b -> p (a b)"))

        # ---------------- P4: sparse MLA attention per head ----------------
        out_hT = mid.tile([128, H, NB], BF16)
        with tc.tile_pool(name="p4w", bufs=2) as p4w, \
             tc.tile_pool(name="p4k", bufs=2) as p4k, \
             tc.tile_pool(name="p4p", bufs=2) as p4p, \
             tc.tile_pool(name="p4ps", bufs=2, space="PSUM") as p4ps, \
             tc.tile_pool(name="p4po", bufs=2, space="PSUM") as p4po:
            wbr = wkvb_v.rearrange("(c p) n -> p c n", p=128)
            for h in range(H):
                wb_k = p4w.tile([128, 4, DN], BF16, tag="wb_k")
                wb_v = p4w.tile([128, 4, DV], BF16, tag="wb_v")
                for c in range(4):
                    nc.gpsimd.dma_start(
                        out=wb_k[:, c, :],
                        in_=wbr[:, c, h * (DN + DV):h * (DN + DV) + DN])
                    nc.gpsimd.dma_start(
                        out=wb_v[:, c, :],
                        in_=wbr[:, c, h * (DN + DV) + DN:(h + 1) * (DN + DV)])
                knT = p4k.tile([128, S], BF16, tag="knT")
                for kc in range(4):
                    ps = p4ps.tile([128, 512], F32, tag="ps_kn")
                    for c in range(4):
                        nc.tensor.matmul(
                            ps, wb_k[:, c, :],
                            ckvT[:, c, kc * 512:(kc + 1) * 512],
                            start=(c == 0), stop=(c == 3))
                    nc.scalar.copy(out=knT[:, kc * 512:(kc + 1) * 512], in_=ps)
                v_sb = p4k.tile([128, NT, DV], BF16, tag="v_sb")
                for kt in range(NT):
                    ps = p4ps.tile([128, DV], F32, tag="ps_v")
                    for c in range(4):
                        nc.tensor.matmul(
                            ps,
                            ckvT[:, c, kt * 128:(kt + 1) * 128],
                            wb_v[:, c, :],
                            start=(c == 0), stop=(c == 3))
                    nc.scalar.copy(out=v_sb[:, kt, :], in_=ps)
                ps_o = p4po.tile([128, NB], F32, tag="ps_o")
                for q in range(NQT):
                    probs = p4p.tile([128, S], F32, tag="probs", bufs=1)
                    for kc in range(4):
                        cols = slice(kc * 512, (kc + 1) * 512)
                        ps = p4ps.tile([128, 512], F32, tag="ps_s")
                        nc.tensor.matmul(
                            ps, qTn[:, h, q * 128:(q + 1) * 128],
                            knT[:, cols], start=True, stop=False)
                        nc.tensor.matmul(
                            ps, qTp[:, h, q * 128:(q + 1) * 128],
                            kpeT[:, cols], start=False, stop=True)
                        nc.vector.tensor_add(probs[:, cols], ps,
                                             maskNEG[:, q, cols])
                    den = p4p.tile([128, 2], F32, tag="den")
                    nc.scalar.activation(out=probs, in_=probs,
                                         func=mybir.ActivationFunctionType.Exp,
                                         accum_out=den[:, 0:1])
                    nc.vector.reciprocal(out=den[:, 1:2], in_=den[:, 0:1])
                    pb = p4p.tile([128, S], BF16, tag="pb")
                    nc.vector.tensor_scalar(out=pb, in0=probs,
                                            scalar1=den[:, 1:2], scalar2=None,
                                            op0=mybir.AluOpType.mult)
                    pT = p4p.tile([128, NT, 128], BF16, tag="pT", bufs=1)
                    for kt in range(NT):
                        nc.scalar.dma_start_transpose(
                            out=pT[:, kt, :],
                            in_=pb[:, kt * 128:(kt + 1) * 128])
                    for kt in range(NT):
                        nc.tensor.matmul(
                            ps_o[:, q * 128:(q + 1) * 128],
                            v_sb[:, kt, :], pT[:, kt, :],
                            start=(kt == 0), stop=(kt == NT - 1))
                nc.scalar.copy(out=out_hT[:, h, :], in_=ps_o)

        if dbg:
            nc.gpsimd.dma_start(out=d_ohT,
                                in_=out_hT.rearrange("p a b -> p (a b)"))

        # ---------------- P5: output projection ----------------
        with tc.tile_pool(name="p5w", bufs=3) as p5w, \
             tc.tile_pool(name="p5", bufs=3) as p5, \
             tc.tile_pool(name="p5ps", bufs=4, space="PSUM") as p5ps:
            wor = wo_v.rearrange("(hh p) n -> p hh n", p=128)
            for g in range(NT):
                wo_g = p5w.tile([128, H, 128], BF16, tag="wo_g")
                for c in range(H):
                    nc.gpsimd.dma_start(
                        out=wo_g[:, c, :],
                        in_=wor[:, c, g * 128:(g + 1) * 128])
                ps = p5ps.tile([128, NB], F32, tag="ps_w")
                for h in range(H):
                    nc.tensor.matmul(ps, wo_g[:, h, :],
                                     out_hT[:, h, :],
                                     start=(h == 0), stop=(h == H - 1))
                ot = p5.tile([128, NB], FP16, tag="ot")
                nc.scalar.copy(out=ot, in_=ps)
                nc.gpsimd.dma_start(out=outT[g * 128:(g + 1) * 128, :], in_=ot)

        mid.release()
        consts.release()
        dram.release()
    nc.compile()
    return nc


_NC_CACHE = {}


def _get_nc(causal=True):
    if causal not in _NC_CACHE:
        _NC_CACHE[causal] = build_nc(causal)
    return _NC_CACHE[causal]


def _split_np(a):
    hi = a.astype(ml_dtypes.bfloat16)
    lo = (a - hi.astype(np.float32)).astype(ml_dtypes.bfloat16)
    return hi, lo


def _is_causal(am):
    s = am.shape[-1]
    r = np.arange(s, dtype=np.int64)
    causal = np.where(r[:, None] >= r[None, :], np.float32(0.0),
                      np.float32(NEG))
    return np.array_equal(am.reshape(s, s), causal)


def make_core_inputs(x, cos, sin, attn_mask, wq_a, q_norm_w, wq_b, wkv_a,
                     kv_norm_w, wkv_b, wo, idx_wq_b, idx_wk, idx_knorm_w,
                     idx_knorm_b, idx_gate):
    causal = _is_causal(np.asarray(attn_mask, np.float32))
    F32_OFF, F32_TOT = _f32_layout(causal)

    blob_bf = np.zeros(BF_TOT, ml_dtypes.bfloat16)

    def put_bf(name, arr):
        o = BF_OFF[name]
        blob_bf[o:o + arr.size] = np.ascontiguousarray(arr).reshape(-1)

    xT = np.ascontiguousarray(x[0].astype(np.float32).T)
    xh = xT.astype(ml_dtypes.bfloat16)
    put_bf("xT_hi", xh)
    lo32 = xT - xh.astype(np.float32)
    xls_np = (np.abs(lo32).max(axis=1, keepdims=True) / 127.0
              ).astype(np.float32) + 1e-30
    blob_i8 = np.zeros(I8_TOT, np.int8)
    qv = np.clip(np.round(lo32 / xls_np), -127, 127).astype(np.int8)
    blob_i8[I8_OFF["xT_lo"]:I8_OFF["xT_lo"] + qv.size] = qv.reshape(-1)
    wa = np.asarray(wq_a, np.float32)
    wah = wa.astype(ml_dtypes.bfloat16)
    put_bf("wqa_hi", wah)
    walo32 = wa - wah.astype(np.float32)
    wals_np = (np.abs(walo32).max(axis=1, keepdims=True) / 127.0
               ).astype(np.float32) + 1e-30
    wq = np.clip(np.round(walo32 / wals_np), -127, 127).astype(np.int8)
    blob_i8[I8_OFF["wqa_lo"]:I8_OFF["wqa_lo"] + wq.size] = wq.reshape(-1)
    put_bf("wq_b", np.asarray(wq_b, np.float32).astype(ml_dtypes.bfloat16))
    put_bf("wkv_a", np.asarray(wkv_a, np.float32).astype(ml_dtypes.bfloat16))
    put_bf("wkv_b", np.asarray(wkv_b, np.float32).astype(ml_dtypes.bfloat16))
    put_bf("wo", np.asarray(wo, np.float32).astype(ml_dtypes.bfloat16))
    ih_, il_ = _split_np(np.asarray(idx_wq_b, np.float32))
    put_bf("iwqb_hi", ih_)
    put_bf("iwqb_lo", il_)
    kh, kl = _split_np(np.asarray(idx_wk, np.float32))
    put_bf("iwk_hi", kh)
    put_bf("iwk_lo", kl)
    gh, gl = _split_np(np.asarray(idx_gate, np.float32))
    put_bf("igate_hi", gh)
    put_bf("igate_lo", gl)
    put_bf("ident", np.eye(128, dtype=np.float32))

    blob_f32 = np.zeros(F32_TOT, np.float32)

    def put_f(name, arr):
        o = F32_OFF[name]
        blob_f32[o:o + arr.size] = np.ascontiguousarray(
            arr, np.float32).reshape(-1)

    put_f("cos", cos[0])
    put_f("sin", sin[0])
    put_f("q_norm_w", q_norm_w)
    put_f("kv_norm_w", kv_norm_w)
    put_f("idx_knorm_w", idx_knorm_w)
    put_f("idx_knorm_b", idx_knorm_b)
    put_f("iota", np.arange(S, dtype=np.float32))
    put_f("xlo_scale", xls_np)
    put_f("walo_scale", wals_np)
    rows = (np.arange(NCORES)[:, None] * NB
            + np.arange(128)[None, :]).astype(np.float32)
    put_f("rows", rows)

    lb, lf = BF_TOT // NCORES, F32_TOT // NCORES
    maps = []
    am = np.ascontiguousarray(attn_mask[0, 0], np.float32)
    for c in range(NCORES):
        li = I8_TOT // NCORES
        m = {
            "shard_bf": np.ascontiguousarray(blob_bf[c * lb:(c + 1) * lb]),
            "shard_i8": np.ascontiguousarray(blob_i8[c * li:(c + 1) * li]),
            "shard_f32": np.ascontiguousarray(blob_f32[c * lf:(c + 1) * lf]),
        }
        if not causal:
            m["amask_rows"] = np.ascontiguousarray(
                am[c * NB:(c + 1) * NB])
        maps.append(m)
    return maps, causal


def kernel(x, cos, sin, attn_mask, wq_a, q_norm_w, wq_b, wkv_a, kv_norm_w,
           wkv_b, wo, idx_wq_b, idx_wk, idx_knorm_w, idx_knorm_b, idx_gate):
    from concourse.bass_utils import run_bass_kernel_spmd
    args = [np.asarray(a, np.float32) for a in (
        x, cos, sin, attn_mask, wq_a, q_norm_w, wq_b, wkv_a, kv_norm_w,
        wkv_b, wo, idx_wq_b, idx_wk, idx_knorm_w, idx_knorm_b, idx_gate)]
    maps, causal = make_core_inputs(*args)
    nc = _get_nc(causal)
    res = run_bass_kernel_spmd(nc, maps, list(range(NCORES)))
    outs = [np.asarray(r["outT"]).astype(np.float32).T for r in res.results]
    out = np.concatenate(outs, axis=0)[None]                   # [1, S, HID]
    return out.astype(np.float32)
